# revision 1
# baseline (speedup 1.0000x reference)
"""AF-LSTM Trainium2 kernel: 8-way batch-parallel, no collectives.

Per core (8 batch rows): gather embeddings, LSTM recurrence in transposed
(gate-major) layout, circular-correlation attention via per-batch circulant
matmul folded into W_y, final MLP + softmax. Host concatenates per-core
[8,256] outputs.
"""

import numpy as np
import ml_dtypes

import concourse.bacc as bacc
import concourse.tile as tile
from concourse import bass, mybir
from concourse.bass import IndirectOffsetOnAxis
from concourse.bass_utils import run_bass_kernel_spmd
from concourse.masks import make_identity
from concourse.bass import _add_dep_helper

F32 = mybir.dt.float32
BF16 = mybir.dt.bfloat16
I32 = mybir.dt.int32
AF = mybir.ActivationFunctionType
ALU = mybir.AluOpType

V, D, B = 50000, 256, 64
NCORES, BL = 8, 8
G4 = 4 * D
bf16 = ml_dtypes.bfloat16

# gate blocks of 128 rows reordered to [i0,i1,f0,f1,o0,o1,g0,g1]
_PERM = np.concatenate([
    np.arange(0, 256),        # i
    np.arange(256, 512),      # f
    np.arange(768, 1024),     # o
    np.arange(512, 768),      # g
])


VARIANT = "c"


def build(T_steps=512, variant=None):
    variant = VARIANT if variant is None else variant
    nc = bacc.Bacc(None, target_bir_lowering=False)
    NT = T_steps * BL // 128          # gather tiles of 128 tokens
    NCH = T_steps * BL // 512         # 512-token chunks for xs matmul

    emb_e = nc.declare_dram_parameter("emb", [V, D], F32, isOutput=False)
    xp_e = nc.declare_dram_parameter("xp", [128, NT], I32, isOutput=False)
    sidx_e = nc.declare_dram_parameter("sidx", [64, 1], I32, isOutput=False)
    wihT_e = nc.declare_dram_parameter("wihT", [128, 2048], BF16, isOutput=False)
    whhT_e = nc.declare_dram_parameter("whhT", [128, 2048], BF16, isOutput=False)
    whh8_e = nc.declare_dram_parameter("whh8", [128, 2048], mybir.dt.float8e4, isOutput=False)
    bl_e = nc.declare_dram_parameter("bl", [128, 8], F32, isOutput=False)
    wy_e = nc.declare_dram_parameter("wy", [128, 512], BF16, isOutput=False)
    wtoh_e = nc.declare_dram_parameter("wtoh", [128, 128], BF16, isOutput=False)
    sel_e = nc.declare_dram_parameter("sel", [8, 1024], BF16, isOutput=False)
    wpT_e = nc.declare_dram_parameter("wpT", [128, 512], BF16, isOutput=False)
    wxT_e = nc.declare_dram_parameter("wxT", [128, 512], BF16, isOutput=False)
    wfT_e = nc.declare_dram_parameter("wfT", [128, 512], BF16, isOutput=False)
    bf_e = nc.declare_dram_parameter("bf", [128, 2], F32, isOutput=False)
    out_e = nc.declare_dram_parameter("out", [8, 256], F32, isOutput=True)
    s2d = nc.dram_tensor("s2d", [8, 512], BF16)

    with tile.TileContext(nc) as tc:
        with (
            tc.tile_pool(name="const", bufs=1) as cp,
            tc.tile_pool(name="big", bufs=1) as bigp,
        ):
            # ---- constants / weights to SBUF ----
            xp_sb = cp.tile([128, NT], I32)
            sidx_sb = cp.tile([64, 1], I32)
            wihT_sb = cp.tile([128, 2048], BF16)
            whhT_sb = cp.tile([128, 2048], BF16)
            whh8_sb = cp.tile([128, 2048], mybir.dt.float8e4)
            bl_sb = cp.tile([128, 8], F32)
            wy_sb = cp.tile([128, 512], BF16)
            wtoh_sb = cp.tile([128, 128], BF16)
            sel_sb = cp.tile([8, 1024], BF16)
            wpT_sb = cp.tile([128, 512], BF16)
            wxT_sb = cp.tile([128, 512], BF16)
            wfT_sb = cp.tile([128, 512], BF16)
            bf_sb = cp.tile([128, 2], F32)
            for dst, src in [(xp_sb, xp_e), (sidx_sb, sidx_e), (wihT_sb, wihT_e),
                             (whhT_sb, whhT_e), (whh8_sb, whh8_e), (bl_sb, bl_e), (wy_sb, wy_e),
                             (wtoh_sb, wtoh_e), (sel_sb, sel_e), (wpT_sb, wpT_e),
                             (wxT_sb, wxT_e), (wfT_sb, wfT_e), (bf_sb, bf_e)]:
                nc.sync.dma_start(dst[:], src[:])
            ident = cp.tile([128, 128], F32)
            make_identity(nc, ident[:])
            ident_bf = cp.tile([128, 128], BF16)
            nc.vector.tensor_copy(ident_bf[:], ident[:])
            ones64 = cp.tile([64, 1], F32)
            nc.gpsimd.memset(ones64[:], 1.0)
            ones1w = cp.tile([1, 128], F32)
            nc.gpsimd.memset(ones1w[:], 1.0)
            ones128 = cp.tile([128, 1], F32)
            nc.gpsimd.memset(ones128[:], 1.0)

            # ---- persistent big tensors ----
            eT0 = bigp.tile([128, T_steps * 8], BF16)
            eT1 = bigp.tile([128, T_steps * 8], BF16)
            xsT = bigp.tile([128, T_steps * 64], BF16)
            hT = bigp.tile([128, (T_steps + 1) * 16], BF16)
            s2_sb = bigp.tile([8, 512], BF16)
            rT_sb = bigp.tile([128, 16], F32)
            rT_bf = bigp.tile([128, 16], BF16)
            a_sb = bigp.tile([8, T_steps], BF16)

            # ================= s-branch (batchnormed aspect embedding) ======
            with (
                tc.tile_pool(name="swork", bufs=1) as sw,
                tc.tile_pool(name="spsum", bufs=1, space="PSUM") as sps,
            ):
                semb = sw.tile([64, 256], F32)
                nc.gpsimd.indirect_dma_start(
                    out=semb[:], out_offset=None, in_=emb_e[:],
                    in_offset=IndirectOffsetOnAxis(ap=sidx_sb[:, :1], axis=0))
                mu_ps = sps.tile([1, 256], F32, space="PSUM")
                nc.tensor.matmul(mu_ps[:], ones64[:], semb[:], start=True, stop=True)
                mu = sw.tile([1, 256], F32)
                nc.vector.tensor_scalar_mul(mu[:], mu_ps[:], 1.0 / 64)
                sq = sw.tile([64, 256], F32)
                nc.vector.tensor_mul(sq[:], semb[:], semb[:])
                ms_ps = sps.tile([1, 256], F32, space="PSUM")
                nc.tensor.matmul(ms_ps[:], ones64[:], sq[:], start=True, stop=True)
                msq = sw.tile([1, 256], F32)
                nc.vector.tensor_scalar_mul(msq[:], ms_ps[:], 1.0 / 64)
                mu2 = sw.tile([1, 256], F32)
                nc.vector.tensor_mul(mu2[:], mu[:], mu[:])
                var = sw.tile([1, 256], F32)
                nc.vector.tensor_tensor(var[:], msq[:], mu2[:], op=ALU.subtract)
                nc.vector.tensor_scalar_add(var[:], var[:], 1e-5)
                std = sw.tile([1, 256], F32)
                nc.scalar.sqrt(std[:], var[:])
                istd = sw.tile([1, 256], F32)
                nc.vector.reciprocal(istd[:], std[:])
                mub_ps = sps.tile([64, 256], F32, space="PSUM")
                nc.tensor.matmul(mub_ps[:], ones1w[:1, :64], mu[:], start=True, stop=True)
                ib_ps = sps.tile([64, 256], F32, space="PSUM")
                nc.tensor.matmul(ib_ps[:], ones1w[:1, :64], istd[:], start=True, stop=True)
                d8 = sw.tile([8, 256], F32)
                nc.vector.tensor_tensor(d8[:], semb[0:8, :], mub_ps[0:8, :], op=ALU.subtract)
                nc.vector.tensor_tensor(s2_sb[:, 0:256], d8[:], ib_ps[0:8, :], op=ALU.mult)
                nc.vector.tensor_copy(s2_sb[:, 256:512], s2_sb[:, 0:256])
                nc.gpsimd.dma_start(s2d[:], s2_sb[:])

            # ========= gather/xs/Weff/Y/score interleaved with recurrence ===
            xs_v = xsT[:].rearrange("p (t q) -> p t q", q=64)
            hT_v = hT[:].rearrange("p (t dj b) -> p dj b t", dj=2, b=8)
            weff_all = bigp.tile([128, 4096], BF16)
            nc.gpsimd.memset(hT[:, 0:16], 0.0)
            with (
                tc.tile_pool(name="gat", bufs=8) as gp,
                tc.tile_pool(name="xps", bufs=1, space="PSUM") as xpsm,
                tc.tile_pool(name="rec", bufs=3) as rp,
                tc.tile_pool(name="cst", bufs=3) as cpp,
                tc.tile_pool(name="rps", bufs=2, space="PSUM") as rpsm,
                tc.tile_pool(name="yps", bufs=2, space="PSUM") as ypsm,
                tc.tile_pool(name="ytp", bufs=3) as ytp,
                tc.tile_pool(name="scp", bufs=1, space="PSUM") as scpsm,
                tc.tile_pool(name="att", bufs=2) as ap_,
                tc.tile_pool(name="sm", bufs=1) as smp,
            ):
                sc_ps = scpsm.tile([8, T_steps], F32, space="PSUM")

                def emit_gather(g):
                    egath = gp.tile([128, 256], F32)
                    nc.gpsimd.indirect_dma_start(
                        out=egath[:], out_offset=None, in_=emb_e[:],
                        in_offset=IndirectOffsetOnAxis(ap=xp_sb[:, g:g + 1], axis=0))
                    return egath

                def emit_trcopy(egath, g, dc):
                    eT = eT0 if dc == 0 else eT1
                    tps = xpsm.tile([128, 128], F32, space="PSUM", tag="ps")
                    nc.tensor.transpose(tps[:], egath[:, dc * 128:(dc + 1) * 128], ident[:])
                    nc.vector.tensor_copy(eT[:, g * 128:(g + 1) * 128], tps[:])

                def emit_xs_mm(nch, gb, holder):
                    xps = xpsm.tile([128, 512], F32, space="PSUM", tag="ps")
                    nc.tensor.matmul(xps[:], wihT_sb[:, gb * 128:(gb + 1) * 128],
                                     eT0[:, nch * 512:(nch + 1) * 512],
                                     start=True, stop=False)
                    nc.tensor.matmul(xps[:], wihT_sb[:, 1024 + gb * 128:1024 + (gb + 1) * 128],
                                     eT1[:, nch * 512:(nch + 1) * 512],
                                     start=False, stop=True)
                    holder['xps'] = xps

                def emit_xs_ts(nch, gb, holder):
                    nc.vector.tensor_scalar(
                        xs_v[:, nch * 64:(nch + 1) * 64, gb * 8:(gb + 1) * 8],
                        holder['xps'], bl_sb[:, gb:gb + 1], None, op0=ALU.add)

                def emit_xs(nch, gb):
                    h = {}
                    emit_xs_mm(nch, gb, h)
                    emit_xs_ts(nch, gb, h)

                egaths = {}

                def chunk_items(k):
                    items = []
                    for g in range(4 * k, 4 * k + 4):
                        if variant == "g":
                            for dc in range(2):
                                items.append(lambda g=g, dc=dc: emit_trcopy(egaths.pop(g) if dc else egaths[g], g, dc))
                        else:
                            holder = {}
                            items.append(lambda g=g, h=holder: h.__setitem__('e', emit_gather(g)))
                            for dc in range(2):
                                items.append(lambda g=g, dc=dc, h=holder: emit_trcopy(h['e'], g, dc))
                    for gb in range(8):
                        h = {}
                        items.append(lambda k=k, gb=gb, h=h: emit_xs_mm(k, gb, h))
                        items.append(lambda k=k, gb=gb, h=h: emit_xs_ts(k, gb, h))
                    return items

                if variant == "g":
                    for g in range(NT):
                        egaths[g] = emit_gather(g)

                # --- Weff = C_b @ W_y (only needs the s-branch) ---
                def emit_cw(b, holder):
                    cw = ap_.tile([128, 384], BF16)
                    for j in range(3):
                        win = bass.AP(s2d[:].tensor, b * 512 + j * 128, [[1, 128], [1, 128]])
                        nc.gpsimd.dma_start(cw[:, j * 128:(j + 1) * 128], win)
                    holder['cw'] = cw

                def emit_weff_mm(b, mj, holder):
                    cw = holder['cw']
                    wps = ypsm.tile([128, 256], F32, space="PSUM", tag="yps")
                    for kc in range(2):
                        nc.tensor.matmul(wps[:], cw[:, (mj + kc) * 128:(mj + kc + 1) * 128],
                                         wy_sb[:, kc * 256:(kc + 1) * 256],
                                         start=(kc == 0), stop=(kc == 1))
                    holder['wps' + str(mj)] = wps

                def emit_weff_cp(b, mj, holder):
                    nc.vector.tensor_copy(weff_all[:, b * 512 + mj * 256: b * 512 + (mj + 1) * 256],
                                          holder['wps' + str(mj)])

                def emit_ygroup(k, b, ec):
                    yc = ypsm.tile([128, 64], F32, space="PSUM", tag="yps")
                    for kc in range(2):
                        nc.tensor.matmul(
                            yc[:], weff_all[:, b * 512 + kc * 256 + ec * 128: b * 512 + kc * 256 + (ec + 1) * 128],
                            hT_v[:, kc, b, 1 + 64 * k: 1 + 64 * (k + 1)],
                            start=(kc == 0), stop=(kc == 1))
                    yt = ytp.tile([128, 64], BF16)
                    nc.scalar.activation(yt[:], yc[:], AF.Tanh)
                    nc.tensor.matmul(sc_ps[:, 64 * k:64 * (k + 1)],
                                     wtoh_sb[:, ec * 64 + b * 8: ec * 64 + (b + 1) * 8],
                                     yt[:], start=(b == 0 and ec == 0), stop=(b == 7 and ec == 1))

                # chunk 0 of gather/xs up front; everything else trickles in
                for it in chunk_items(0):
                    it()
                xs_work = []
                for k in range(1, NCH):
                    xs_work.extend(chunk_items(k))
                y_work = []   # (avail_step, fn)
                for b in range(8):
                    holder = {}
                    y_work.append((0, lambda b=b, h=holder: emit_cw(b, h)))
                    for mj in range(2):
                        y_work.append((0, lambda b=b, mj=mj, h=holder: emit_weff_mm(b, mj, h)))
                        y_work.append((0, lambda b=b, mj=mj, h=holder: emit_weff_cp(b, mj, h)))
                for k in range(T_steps // 64):
                    for b in range(8):
                        for ec in range(2):
                            y_work.append((64 * (k + 1),
                                           lambda k=k, b=b, ec=ec: emit_ygroup(k, b, ec)))
                xi = yi = 0

                c_prev = cpp.tile([128, 16], F32)
                nc.vector.memset(c_prev[:], 0.0)
                def emit_imm(tt):
                    gi = rpsm.tile([128, 48], F32, space="PSUM", tag="gpi")
                    gg_ = rpsm.tile([128, 16], F32, space="PSUM", tag="gpg")
                    nc.tensor.matmul(gi[:], ident_bf[:], xsT[:, tt * 64: tt * 64 + 48],
                                     start=True, stop=False)
                    nc.tensor.matmul(gg_[:], ident_bf[:], xsT[:, tt * 64 + 48: tt * 64 + 64],
                                     start=True, stop=False)
                    return gi, gg_

                if variant == "d":
                    gpi, gpg = emit_imm(0)
                for t in range(T_steps):
                    if variant != "d":
                        gpi, gpg = emit_imm(t)
                    gb_order = (6, 7, 0, 1, 2, 3, 4, 5) if variant == "d" else (0, 1, 2, 3, 4, 5, 6, 7)
                    w_sb = whh8_sb if variant == "f" else whhT_sb
                    for gb in gb_order:
                        out = gpi[:, gb * 8:(gb + 1) * 8] if gb < 6 else gpg[:, (gb - 6) * 8:(gb - 5) * 8]
                        for dj in range(2):
                            nc.tensor.matmul(
                                out,
                                w_sb[:, dj * 1024 + gb * 128: dj * 1024 + (gb + 1) * 128],
                                hT[:, t * 16 + dj * 8: t * 16 + (dj + 1) * 8],
                                start=False, stop=(dj == 1 and gb in (5, 7)))
                    gpi_t, gpg_t = gpi, gpg
                    if variant == "d" and t + 1 < T_steps:
                        gpi, gpg = emit_imm(t + 1)
                    if variant == "d":
                        gg = rp.tile([128, 16], F32)
                        nc.scalar.activation(gg[:], gpg_t[:], AF.Tanh)
                        sig = rp.tile([128, 48], F32)
                        nc.scalar.activation(sig[:], gpi_t[:], AF.Sigmoid)
                    else:
                        sig = rp.tile([128, 48], F32)
                        nc.scalar.activation(sig[:], gpi_t[:], AF.Sigmoid)
                        gg = rp.tile([128, 16], F32)
                        nc.scalar.activation(gg[:], gpg_t[:], AF.Tanh)
                    m1 = rp.tile([128, 16], F32)
                    nc.vector.tensor_mul(m1[:], sig[:, 16:32], c_prev[:])
                    m2 = rp.tile([128, 16], F32)
                    nc.vector.tensor_mul(m2[:], sig[:, 0:16], gg[:])
                    c_new = cpp.tile([128, 16], F32)
                    nc.vector.tensor_tensor(c_new[:], m1[:], m2[:], op=ALU.add)
                    thc = rp.tile([128, 16], F32)
                    nc.scalar.activation(thc[:], c_new[:], AF.Tanh)
                    nc.vector.tensor_mul(hT[:, (t + 1) * 16:(t + 2) * 16], sig[:, 32:48], thc[:])
                    c_prev = c_new
                    if xi < len(xs_work):
                        xs_work[xi]()
                        xi += 1
                    if yi < len(y_work) and y_work[yi][0] <= t and (t >= 96 or t % 2 == 0):
                        y_work[yi][1]()
                        yi += 1
                while xi < len(xs_work):
                    xs_work[xi]()
                    xi += 1
                while yi < len(y_work):
                    y_work[yi][1]()
                    yi += 1

                # ---- softmax over T (free axis) ----
                mx = smp.tile([8, 1], F32)
                nc.vector.tensor_reduce(mx[:], sc_ps[:, 0:T_steps], axis=mybir.AxisListType.X, op=ALU.max)
                nmx = smp.tile([8, 1], F32)
                nc.vector.tensor_scalar_mul(nmx[:], mx[:], -1.0)
                esc = smp.tile([8, T_steps], F32)
                ssum = smp.tile([8, 1], F32)
                nc.scalar.activation(esc[:], sc_ps[:, 0:T_steps], AF.Exp,
                                     bias=nmx[:, 0:1], accum_out=ssum[:, 0:1])
                rcs = smp.tile([8, 1], F32)
                nc.vector.reciprocal(rcs[:], ssum[:])
                nc.scalar.activation(a_sb[:], esc[:], AF.Copy, scale=rcs[:, 0:1])
                # ---- r = sum_t a_t * h_t ----
                for b in range(8):
                    abc = ypsm.tile([128, T_steps], F32, space="PSUM", tag="yps")
                    nc.tensor.matmul(abc[:, 0:T_steps], sel_sb[:, b * 128:(b + 1) * 128],
                                     a_sb[:], start=True, stop=True)
                    for dj in range(2):
                        wt_ = ap_.tile([128, T_steps], F32)
                        nc.vector.tensor_tensor(wt_[:, 0:T_steps], hT_v[:, dj, b, 1:T_steps + 1],
                                                abc[:, 0:T_steps], op=ALU.mult)
                        nc.vector.tensor_reduce(rT_sb[:, dj * 8 + b: dj * 8 + b + 1],
                                                wt_[:, 0:T_steps], axis=mybir.AxisListType.X,
                                                op=ALU.add)
                nc.vector.tensor_copy(rT_bf[:], rT_sb[:])

            # ================= final MLP + softmax ==========================
            with (
                tc.tile_pool(name="fin", bufs=1) as fp,
                tc.tile_pool(name="fps", bufs=1, space="PSUM") as fpsm,
            ):
                rr_ps = fpsm.tile([128, 16], F32, space="PSUM")
                hlast = hT[:, T_steps * 16:(T_steps + 1) * 16]
                for oc in range(2):
                    for kc in range(2):
                        nc.tensor.matmul(rr_ps[:, oc * 8:(oc + 1) * 8],
                                         wpT_sb[:, kc * 256 + oc * 128: kc * 256 + (oc + 1) * 128],
                                         rT_bf[:, kc * 8:(kc + 1) * 8],
                                         start=(kc == 0), stop=False)
                    for kc in range(2):
                        nc.tensor.matmul(rr_ps[:, oc * 8:(oc + 1) * 8],
                                         wxT_sb[:, kc * 256 + oc * 128: kc * 256 + (oc + 1) * 128],
                                         hlast[:, kc * 8:(kc + 1) * 8],
                                         start=False, stop=(kc == 1))
                rrT = fp.tile([128, 16], BF16)
                nc.scalar.activation(rrT[:], rr_ps[:], AF.Tanh)
                z_ps = fpsm.tile([128, 16], F32, space="PSUM")
                for oc in range(2):
                    for kc in range(2):
                        nc.tensor.matmul(z_ps[:, oc * 8:(oc + 1) * 8],
                                         wfT_sb[:, kc * 256 + oc * 128: kc * 256 + (oc + 1) * 128],
                                         rrT[:, kc * 8:(kc + 1) * 8],
                                         start=(kc == 0), stop=(kc == 1))
                e_sb = fp.tile([128, 16], F32)
                for oc in range(2):
                    nc.scalar.activation(e_sb[:, oc * 8:(oc + 1) * 8], z_ps[:, oc * 8:(oc + 1) * 8],
                                         AF.Exp, bias=bf_sb[:, oc:oc + 1])
                cs_ps = fpsm.tile([1, 16], F32, space="PSUM")
                nc.tensor.matmul(cs_ps[:], ones128[:], e_sb[:], start=True, stop=True)
                cs_sb = fp.tile([1, 16], F32)
                nc.vector.tensor_copy(cs_sb[:], cs_ps[:])
                s8 = fp.tile([1, 8], F32)
                nc.vector.tensor_tensor(s8[:], cs_sb[0:1, 0:8], cs_sb[0:1, 8:16], op=ALU.add)
                rc8 = fp.tile([1, 8], F32)
                nc.vector.reciprocal(rc8[:], s8[:])
                rc16 = fp.tile([1, 16], F32)
                nc.vector.tensor_copy(rc16[:, 0:8], rc8[:])
                nc.vector.tensor_copy(rc16[:, 8:16], rc8[:])
                rbc_ps = fpsm.tile([128, 16], F32, space="PSUM")
                nc.tensor.matmul(rbc_ps[:], ones1w[:], rc16[:], start=True, stop=True)
                yT_sb = fp.tile([128, 16], F32)
                nc.vector.tensor_tensor(yT_sb[:], e_sb[:], rbc_ps[:], op=ALU.mult)
                ytr_ps = fpsm.tile([16, 128], F32, space="PSUM")
                nc.tensor.transpose(ytr_ps[:], yT_sb[:], ident[:])
                ynat = fp.tile([16, 128], F32)
                nc.vector.tensor_copy(ynat[:], ytr_ps[:])
                for oc in range(2):
                    nc.sync.dma_start(out_e[0:8, oc * 128:(oc + 1) * 128],
                                      ynat[oc * 8:(oc + 1) * 8, :])

    nc.compile()
    return nc


_CACHE = {}


def _get_nc(T_steps=512, variant=None):
    key = (T_steps, VARIANT if variant is None else variant)
    if key not in _CACHE:
        _CACHE[key] = build(T_steps, variant=key[1])
    return _CACHE[key]


def make_in_maps(x, s, emb, w_ih, w_hh, b_lstm, W_y, w_t, W_p, W_x, W_f, b_f,
                 T_steps=512):
    x = np.asarray(x).astype(np.int32)[:, :T_steps]
    s = np.asarray(s).astype(np.int32).reshape(64)
    emb = np.ascontiguousarray(np.asarray(emb, dtype=np.float32))
    wih_p = np.asarray(w_ih, dtype=np.float32)[_PERM]
    whh_p = np.asarray(w_hh, dtype=np.float32)[_PERM]
    bl_p = np.asarray(b_lstm, dtype=np.float32)[_PERM]

    def wt2sb(wT):  # [256, 1024] -> [128, 2048]
        return np.concatenate([wT[0:128], wT[128:256]], axis=1)

    wihT = wt2sb(wih_p.T).astype(bf16)
    whhT = wt2sb(whh_p.T).astype(bf16)
    bl_sb = bl_p.reshape(8, 128).T.copy().astype(np.float32)  # [128, 8]
    wy_sb = np.concatenate([np.asarray(W_y, np.float32)[0:128],
                            np.asarray(W_y, np.float32)[128:256]], axis=1).astype(bf16)
    w_t = np.asarray(w_t, np.float32)
    wtoh = np.zeros((128, 128), np.float32)
    for ec in range(2):
        for b in range(8):
            wtoh[:, ec * 64 + b * 8 + b] = w_t[ec * 128:(ec + 1) * 128]
    wtoh = wtoh.astype(bf16)
    sel = np.zeros((8, 1024), np.float32)
    for b in range(8):
        sel[b, b * 128:(b + 1) * 128] = 1.0
    sel = sel.astype(bf16)

    def t2sb(w):  # W [do, din] -> lhsT layout [128, 512] free=kc*256+do
        wT = np.asarray(w, np.float32).T  # [din, do]
        return np.concatenate([wT[0:128], wT[128:256]], axis=1).astype(bf16)

    wpT = t2sb(W_p)
    wxT = t2sb(W_x)
    wfT = t2sb(W_f)
    bf_sb = np.asarray(b_f, np.float32).reshape(2, 128).T.copy()

    whh8 = wt2sb(whh_p.T).astype(ml_dtypes.float8_e4m3)
    common = dict(emb=emb, wihT=wihT, whhT=whhT, whh8=whh8, bl=bl_sb, wy=wy_sb, wtoh=wtoh,
                  sel=sel, wpT=wpT, wxT=wxT, wfT=wfT, bf=bf_sb)
    in_maps = []
    for c in range(NCORES):
        xs_c = x[c * BL:(c + 1) * BL]                      # [8, T]
        xflat = xs_c.T.reshape(-1)                         # t-major tokens
        xp = xflat.reshape(-1, 128).T.copy()               # [128, NT]
        sidx = np.roll(s, -BL * c).reshape(64, 1).copy()
        in_maps.append(dict(xp=xp, sidx=sidx, **common))
    return in_maps


def _install_trace_shim():
    """The agent image lacks antenv.axon_hooks; recreate it and install the
    ctypes NTFF hook from trn_boot so run_bass_kernel_spmd(trace=True) works."""
    import sys, types
    if "antenv.axon_hooks" not in sys.modules:
        mod = types.ModuleType("antenv.axon_hooks")
        mod._hook = None
        mod.set_axon_ntff_profile_hook = lambda h: setattr(mod, "_hook", h)
        mod.get_axon_ntff_profile_hook = lambda: mod._hook
        sys.modules["antenv.axon_hooks"] = mod
        import antenv
        antenv.axon_hooks = mod
    import antenv.axon_hooks as ah
    if ah.get_axon_ntff_profile_hook() is None:
        from trn_agent_boot.trn_boot import _ntff_profile_via_ctypes
        ah.set_axon_ntff_profile_hook(_ntff_profile_via_ctypes("/opt/axon/libaxon_pjrt.so"))
    import concourse.bass_utils as bu
    bu.upload_artifacts = lambda tmpdir: ""


def run(in_maps, T_steps=512, trace=False, tmpdir=None, variant=None):
    nc = _get_nc(T_steps, variant)
    if trace:
        _install_trace_shim()
    return run_bass_kernel_spmd(nc, in_maps, core_ids=list(range(NCORES)),
                                trace=trace, tmpdir=tmpdir)


def kernel(x, s, emb, w_ih, w_hh, b_lstm, W_y, w_t, W_p, W_x, W_f, b_f):
    in_maps = make_in_maps(x, s, emb, w_ih, w_hh, b_lstm, W_y, w_t, W_p, W_x,
                           W_f, b_f)
    res = run(in_maps)
    return np.concatenate([res.results[i]["out"] for i in range(NCORES)], axis=0)



# revision 11
# speedup vs baseline: 1.0759x; 1.0759x over previous
"""AF-LSTM Trainium2 kernel: 8-way batch-parallel, no collectives.

Per core (8 batch rows): gather embeddings, LSTM recurrence in transposed
(gate-major) layout, circular-correlation attention via per-batch circulant
matmul folded into W_y, final MLP + softmax. Host concatenates per-core
[8,256] outputs.

Variant "h": latency-tuned recurrence chain — g-gates matmul first so
tanh(g) hides under sigma(i,f); packed [sig_i|sig_f] * [tanh_g|c] product;
sigma(o) off the critical path; all embedding gathers issued up-front so the
PE queue never head-of-line blocks on gather DMAs; s-branch and circulant
window DMAs moved off the GpSimd/PE critical queues; fused
tensor_tensor_reduce for the attention readout.
"""

import numpy as np
import ml_dtypes

import concourse.bacc as bacc
import concourse.tile as tile
from concourse import bass, mybir
from concourse.bass import IndirectOffsetOnAxis
from concourse.bass_utils import run_bass_kernel_spmd
from concourse.masks import make_identity

F32 = mybir.dt.float32
BF16 = mybir.dt.bfloat16
I32 = mybir.dt.int32
AF = mybir.ActivationFunctionType
ALU = mybir.AluOpType

V, D, B = 50000, 256, 64
NCORES, BL = 8, 8
G4 = 4 * D
bf16 = ml_dtypes.bfloat16

# gate blocks of 128 rows reordered to [i0,i1,f0,f1,o0,o1,g0,g1]
_PERM = np.concatenate([
    np.arange(0, 256),        # i
    np.arange(256, 512),      # f
    np.arange(768, 1024),     # o
    np.arange(512, 768),      # g
])


VARIANT = "h"

import os as _os
H_CW_SYNC = _os.environ.get("H_CW_SYNC", "1") == "1"
H_TTR = _os.environ.get("H_TTR", "0") == "1"


def _declare(nc, T_steps):
    NT = T_steps * BL // 128
    ten = {}
    ten['emb'] = nc.declare_dram_parameter("emb", [V, D], F32, isOutput=False)
    ten['xp'] = nc.declare_dram_parameter("xp", [128, NT], I32, isOutput=False)
    ten['sidx'] = nc.declare_dram_parameter("sidx", [64, 1], I32, isOutput=False)
    ten['wihT'] = nc.declare_dram_parameter("wihT", [128, 2048], BF16, isOutput=False)
    ten['whhT'] = nc.declare_dram_parameter("whhT", [128, 2048], BF16, isOutput=False)
    ten['whh8'] = nc.declare_dram_parameter("whh8", [128, 2048], mybir.dt.float8e4, isOutput=False)
    ten['bl'] = nc.declare_dram_parameter("bl", [128, 8], F32, isOutput=False)
    ten['wy'] = nc.declare_dram_parameter("wy", [128, 512], BF16, isOutput=False)
    ten['wtoh'] = nc.declare_dram_parameter("wtoh", [128, 128], BF16, isOutput=False)
    ten['sel'] = nc.declare_dram_parameter("sel", [8, 1024], BF16, isOutput=False)
    ten['wpT'] = nc.declare_dram_parameter("wpT", [128, 512], BF16, isOutput=False)
    ten['wxT'] = nc.declare_dram_parameter("wxT", [128, 512], BF16, isOutput=False)
    ten['wfT'] = nc.declare_dram_parameter("wfT", [128, 512], BF16, isOutput=False)
    ten['bf'] = nc.declare_dram_parameter("bf", [128, 2], F32, isOutput=False)
    ten['out'] = nc.declare_dram_parameter("out", [8, 256], F32, isOutput=True)
    ten['s2d'] = nc.dram_tensor("s2d", [8, 512], BF16)
    return ten


def build_h(T_steps=512):
    nc = bacc.Bacc(None, target_bir_lowering=False)
    NT = T_steps * BL // 128          # gather tiles of 128 tokens
    NCH = T_steps * BL // 512         # 512-token chunks for xs matmul
    ten = _declare(nc, T_steps)
    emb_e, xp_e, sidx_e = ten['emb'], ten['xp'], ten['sidx']
    out_e, s2d = ten['out'], ten['s2d']

    with tile.TileContext(nc) as tc:
        with (
            tc.tile_pool(name="const", bufs=1) as cp,
            tc.tile_pool(name="big", bufs=1) as bigp,
        ):
            # ---- constants / weights to SBUF ----
            xp_sb = cp.tile([128, NT], I32)
            sidx_sb = cp.tile([64, 1], I32)
            wihT_sb = cp.tile([128, 2048], BF16)
            whhT_sb = cp.tile([128, 2048], BF16)
            bl_sb = cp.tile([128, 8], F32)
            wy_sb = cp.tile([128, 512], BF16)
            wtoh_sb = cp.tile([128, 128], BF16)
            sel_sb = cp.tile([8, 1024], BF16)
            wpT_sb = cp.tile([128, 512], BF16)
            wxT_sb = cp.tile([128, 512], BF16)
            wfT_sb = cp.tile([128, 512], BF16)
            bf_sb = cp.tile([128, 2], F32)
            for dst, src in [(xp_sb, ten['xp']), (sidx_sb, ten['sidx']),
                             (wihT_sb, ten['wihT']), (whhT_sb, ten['whhT']),
                             (bl_sb, ten['bl']), (wy_sb, ten['wy']),
                             (wtoh_sb, ten['wtoh']), (sel_sb, ten['sel']),
                             (wpT_sb, ten['wpT']), (wxT_sb, ten['wxT']),
                             (wfT_sb, ten['wfT']), (bf_sb, ten['bf'])]:
                nc.sync.dma_start(dst[:], src[:])
            ident = cp.tile([128, 128], F32)
            make_identity(nc, ident[:])
            ident_bf = cp.tile([128, 128], BF16)
            nc.vector.tensor_copy(ident_bf[:], ident[:])
            ones64 = cp.tile([64, 1], F32)
            nc.gpsimd.memset(ones64[:], 1.0)
            ones1w = cp.tile([1, 128], F32)
            nc.gpsimd.memset(ones1w[:], 1.0)
            ones128 = cp.tile([128, 1], F32)
            nc.gpsimd.memset(ones128[:], 1.0)

            # ---- persistent big tensors ----
            eT0 = bigp.tile([128, T_steps * 8], BF16)
            eT1 = bigp.tile([128, T_steps * 8], BF16)
            xsT = bigp.tile([128, T_steps * 64], BF16)
            hT = bigp.tile([128, (T_steps + 1) * 16], BF16)
            s2_sb = bigp.tile([8, 512], BF16)
            rT_sb = bigp.tile([128, 16], F32)
            rT_bf = bigp.tile([128, 16], BF16)
            a_sb = bigp.tile([8, T_steps], BF16)
            weff_all = bigp.tile([128, 4096], BF16)
            cgA = bigp.tile([128, 32], F32)   # cols 0:16 tanh(g_t), 16:32 c
            cgB = bigp.tile([128, 32], F32)

            xs_v = xsT[:].rearrange("p (t q) -> p t q", q=64)
            hT_v = hT[:].rearrange("p (t dj b) -> p dj b t", dj=2, b=8)
            nc.vector.memset(hT[:, 0:16], 0.0)
            nc.vector.memset(cgA[:, 16:32], 0.0)

            with (
                tc.tile_pool(name="gat", bufs=NT) as gp,
                tc.tile_pool(name="xps", bufs=1, space="PSUM") as xpsm,
                tc.tile_pool(name="rec", bufs=4) as rp,
                tc.tile_pool(name="rps", bufs=2, space="PSUM") as rpsm,
                tc.tile_pool(name="yps", bufs=2, space="PSUM") as ypsm,
                tc.tile_pool(name="ytp", bufs=3) as ytp,
                tc.tile_pool(name="scp", bufs=1, space="PSUM") as scpsm,
                tc.tile_pool(name="cwp", bufs=8) as cwp,
                tc.tile_pool(name="att", bufs=2) as ap_,
                tc.tile_pool(name="sm", bufs=1) as smp,
                tc.tile_pool(name="swork", bufs=1) as sw,
            ):
                sc_ps = scpsm.tile([8, T_steps], F32, space="PSUM")

                def emit_gather(g):
                    egath = gp.tile([128, 256], F32)
                    nc.gpsimd.indirect_dma_start(
                        out=egath[:], out_offset=None, in_=emb_e[:],
                        in_offset=IndirectOffsetOnAxis(ap=xp_sb[:, g:g + 1], axis=0))
                    return egath

                def emit_trcopy(egath, g, dc):
                    eT = eT0 if dc == 0 else eT1
                    tps = xpsm.tile([128, 128], F32, space="PSUM", tag="ps")
                    nc.tensor.transpose(tps[:], egath[:, dc * 128:(dc + 1) * 128], ident[:])
                    nc.vector.tensor_copy(eT[:, g * 128:(g + 1) * 128], tps[:])

                def emit_xs_mm(nch, gb, holder):
                    xps = xpsm.tile([128, 512], F32, space="PSUM", tag="ps")
                    nc.tensor.matmul(xps[:], wihT_sb[:, gb * 128:(gb + 1) * 128],
                                     eT0[:, nch * 512:(nch + 1) * 512],
                                     start=True, stop=False)
                    nc.tensor.matmul(xps[:], wihT_sb[:, 1024 + gb * 128:1024 + (gb + 1) * 128],
                                     eT1[:, nch * 512:(nch + 1) * 512],
                                     start=False, stop=True)
                    holder['xps'] = xps

                def emit_xs_ts_half(nch, gb, half, holder):
                    nc.vector.tensor_scalar(
                        xs_v[:, nch * 64 + half * 32:nch * 64 + (half + 1) * 32,
                             gb * 8:(gb + 1) * 8],
                        holder['xps'][:, half * 256:(half + 1) * 256],
                        bl_sb[:, gb:gb + 1], None, op0=ALU.add)

                egaths = {}

                def chunk_items(k):
                    items = []
                    for g in range(4 * k, 4 * k + 4):
                        for dc in range(2):
                            items.append(lambda g=g, dc=dc: emit_trcopy(
                                egaths.pop(g) if dc else egaths[g], g, dc))
                    for gb in range(8):
                        h = {}
                        items.append(lambda k=k, gb=gb, h=h: emit_xs_mm(k, gb, h))
                        for half in range(2):
                            items.append(lambda k=k, gb=gb, half=half, h=h:
                                         emit_xs_ts_half(k, gb, half, h))
                    return items

                # gathers: chunk-0 first, then aspect row, then the rest.
                for g in range(4):
                    egaths[g] = emit_gather(g)
                semb = sw.tile([64, 256], F32)
                nc.gpsimd.indirect_dma_start(
                    out=semb[:], out_offset=None, in_=emb_e[:],
                    in_offset=IndirectOffsetOnAxis(ap=sidx_sb[:, :1], axis=0))
                for g in range(4, NT):
                    egaths[g] = emit_gather(g)

                # chunk-0 transposes + xs up front
                for it in chunk_items(0):
                    it()

                # ---- s-branch (batchnormed aspect embedding); PSUM stats run
                # sequentially through the xpsm ring slot.
                mu_ps = xpsm.tile([64, 256], F32, space="PSUM", tag="ps")
                nc.tensor.matmul(mu_ps[0:1, :], ones64[:], semb[:], start=True, stop=True)
                mu = sw.tile([1, 256], F32)
                nc.vector.tensor_scalar_mul(mu[:], mu_ps[0:1, :], 1.0 / 64)
                sq = sw.tile([64, 256], F32)
                nc.vector.tensor_mul(sq[:], semb[:], semb[:])
                ms_ps = xpsm.tile([64, 256], F32, space="PSUM", tag="ps")
                nc.tensor.matmul(ms_ps[0:1, :], ones64[:], sq[:], start=True, stop=True)
                msq = sw.tile([1, 256], F32)
                nc.vector.tensor_scalar_mul(msq[:], ms_ps[0:1, :], 1.0 / 64)
                mu2 = sw.tile([1, 256], F32)
                nc.vector.tensor_mul(mu2[:], mu[:], mu[:])
                var = sw.tile([1, 256], F32)
                nc.vector.tensor_tensor(var[:], msq[:], mu2[:], op=ALU.subtract)
                nc.vector.tensor_scalar_add(var[:], var[:], 1e-5)
                std = sw.tile([1, 256], F32)
                nc.scalar.sqrt(std[:], var[:])
                istd = sw.tile([1, 256], F32)
                nc.vector.reciprocal(istd[:], std[:])
                mub_ps = xpsm.tile([64, 256], F32, space="PSUM", tag="ps")
                nc.tensor.matmul(mub_ps[:], ones1w[:1, :64], mu[:], start=True, stop=True)
                d8 = sw.tile([8, 256], F32)
                nc.vector.tensor_tensor(d8[:], semb[0:8, :], mub_ps[0:8, :], op=ALU.subtract)
                ib_ps = xpsm.tile([64, 256], F32, space="PSUM", tag="ps")
                nc.tensor.matmul(ib_ps[:], ones1w[:1, :64], istd[:], start=True, stop=True)
                nc.vector.tensor_tensor(s2_sb[:, 0:256], d8[:], ib_ps[0:8, :], op=ALU.mult)
                nc.vector.tensor_copy(s2_sb[:, 256:512], s2_sb[:, 0:256])
                nc.sync.dma_start(s2d[:], s2_sb[:])

                # circulant windows for all batch rows (sync DMA queue)
                cw_tiles = {}
                for b in range(8):
                    cw = cwp.tile([128, 384], BF16)
                    for j in range(3):
                        win = bass.AP(s2d[:].tensor, b * 512 + j * 128, [[1, 128], [1, 128]])
                        if H_CW_SYNC:
                            nc.sync.dma_start(cw[:, j * 128:(j + 1) * 128], win)
                        else:
                            nc.gpsimd.dma_start(cw[:, j * 128:(j + 1) * 128], win)
                    cw_tiles[b] = cw

                # --- Weff = C_b @ W_y ---
                def emit_weff_mm(b, mj, holder):
                    cw = cw_tiles[b]
                    wps = ypsm.tile([128, 256], F32, space="PSUM", tag="yps")
                    for kc in range(2):
                        nc.tensor.matmul(wps[:], cw[:, (mj + kc) * 128:(mj + kc + 1) * 128],
                                         wy_sb[:, kc * 256:(kc + 1) * 256],
                                         start=(kc == 0), stop=(kc == 1))
                    holder['wps' + str(mj)] = wps

                def emit_weff_cp(b, mj, holder):
                    nc.vector.tensor_copy(
                        weff_all[:, b * 512 + mj * 256: b * 512 + (mj + 1) * 256],
                        holder['wps' + str(mj)])

                def emit_ygroup(k, b, ec):
                    yc = ypsm.tile([128, 64], F32, space="PSUM", tag="yps")
                    for kc in range(2):
                        nc.tensor.matmul(
                            yc[:], weff_all[:, b * 512 + kc * 256 + ec * 128:
                                            b * 512 + kc * 256 + (ec + 1) * 128],
                            hT_v[:, kc, b, 1 + 64 * k: 1 + 64 * (k + 1)],
                            start=(kc == 0), stop=(kc == 1))
                    yt = ytp.tile([128, 64], BF16)
                    nc.scalar.activation(yt[:], yc[:], AF.Tanh)
                    nc.tensor.matmul(sc_ps[:, 64 * k:64 * (k + 1)],
                                     wtoh_sb[:, ec * 64 + b * 8: ec * 64 + (b + 1) * 8],
                                     yt[:], start=(b == 0 and ec == 0), stop=(b == 7 and ec == 1))

                xs_work = []
                for k in range(1, NCH):
                    xs_work.extend(chunk_items(k))
                y_work = []   # (avail_step, fn)
                for b in range(8):
                    holder = {}
                    for mj in range(2):
                        y_work.append((12, lambda b=b, mj=mj, h=holder: emit_weff_mm(b, mj, h)))
                        y_work.append((12, lambda b=b, mj=mj, h=holder: emit_weff_cp(b, mj, h)))
                for k in range(T_steps // 64):
                    for b in range(8):
                        for ec in range(2):
                            y_work.append((64 * (k + 1),
                                           lambda k=k, b=b, ec=ec: emit_ygroup(k, b, ec)))
                xi = yi = 0

                def emit_imm(tt):
                    gi = rpsm.tile([128, 48], F32, space="PSUM", tag="gpi")
                    gg_ = rpsm.tile([128, 16], F32, space="PSUM", tag="gpg")
                    nc.tensor.matmul(gi[:], ident_bf[:], xsT[:, tt * 64: tt * 64 + 48],
                                     start=True, stop=False)
                    nc.tensor.matmul(gg_[:], ident_bf[:], xsT[:, tt * 64 + 48: tt * 64 + 64],
                                     start=True, stop=False)
                    return gi, gg_

                cg = [cgA, cgB]
                for t in range(T_steps):
                    gpi, gpg = emit_imm(t)
                    # g-gates first so tanh(g) hides under sigma(i,f)
                    for gb in (6, 7, 0, 1, 2, 3, 4, 5):
                        out = (gpi[:, gb * 8:(gb + 1) * 8] if gb < 6
                               else gpg[:, (gb - 6) * 8:(gb - 5) * 8])
                        for dj in range(2):
                            nc.tensor.matmul(
                                out,
                                whhT_sb[:, dj * 1024 + gb * 128: dj * 1024 + (gb + 1) * 128],
                                hT[:, t * 16 + dj * 8: t * 16 + (dj + 1) * 8],
                                start=False, stop=(dj == 1 and gb in (5, 7)))
                    cgt = cg[t % 2]
                    cgn = cg[(t + 1) % 2]
                    # Scalar queue: tanh(g), sigma(i,f), sigma(o), [tanh(c) later]
                    nc.scalar.activation(cgt[:, 0:16], gpg[:], AF.Tanh)
                    sig_if = rp.tile([128, 32], F32)
                    nc.scalar.activation(sig_if[:], gpi[:, 0:32], AF.Sigmoid)
                    sig_o = rp.tile([128, 16], F32)
                    nc.scalar.activation(sig_o[:], gpi[:, 32:48], AF.Sigmoid)
                    # DVE: packed product, pair-sum, then h
                    m12 = rp.tile([128, 32], F32)
                    nc.vector.tensor_mul(m12[:], sig_if[:], cgt[:])
                    nc.vector.tensor_tensor(cgn[:, 16:32], m12[:, 0:16], m12[:, 16:32],
                                            op=ALU.add)
                    thc = rp.tile([128, 16], F32)
                    nc.scalar.activation(thc[:], cgn[:, 16:32], AF.Tanh)
                    nc.vector.tensor_mul(hT[:, (t + 1) * 16:(t + 2) * 16], sig_o[:], thc[:])
                    if xi < len(xs_work):
                        xs_work[xi]()
                        xi += 1
                    if yi < len(y_work) and y_work[yi][0] <= t:
                        y_work[yi][1]()
                        yi += 1
                while xi < len(xs_work):
                    xs_work[xi]()
                    xi += 1
                while yi < len(y_work):
                    y_work[yi][1]()
                    yi += 1

                # ---- softmax over T (free axis) ----
                mx = smp.tile([8, 1], F32)
                nc.vector.tensor_reduce(mx[:], sc_ps[:, 0:T_steps], axis=mybir.AxisListType.X, op=ALU.max)
                nmx = smp.tile([8, 1], F32)
                nc.vector.tensor_scalar_mul(nmx[:], mx[:], -1.0)
                esc = smp.tile([8, T_steps], F32)
                ssum = smp.tile([8, 1], F32)
                nc.scalar.activation(esc[:], sc_ps[:, 0:T_steps], AF.Exp,
                                     bias=nmx[:, 0:1], accum_out=ssum[:, 0:1])
                rcs = smp.tile([8, 1], F32)
                nc.vector.reciprocal(rcs[:], ssum[:])
                nc.scalar.activation(a_sb[:], esc[:], AF.Copy, scale=rcs[:, 0:1])
                # ---- r = sum_t a_t * h_t (fused multiply+reduce) ----
                for b in range(8):
                    abc = ypsm.tile([128, T_steps], F32, space="PSUM", tag="yps")
                    nc.tensor.matmul(abc[:, 0:T_steps], sel_sb[:, b * 128:(b + 1) * 128],
                                     a_sb[:], start=True, stop=True)
                    for dj in range(2):
                        wt_ = ap_.tile([128, T_steps], F32)
                        if H_TTR:
                            nc.vector.tensor_tensor_reduce(
                                out=wt_[:, 0:T_steps],
                                in0=hT_v[:, dj, b, 1:T_steps + 1],
                                in1=abc[:, 0:T_steps],
                                scale=1.0, scalar=0.0,
                                op0=ALU.mult, op1=ALU.add,
                                accum_out=rT_sb[:, dj * 8 + b: dj * 8 + b + 1])
                        else:
                            nc.vector.tensor_tensor(wt_[:, 0:T_steps], hT_v[:, dj, b, 1:T_steps + 1],
                                                    abc[:, 0:T_steps], op=ALU.mult)
                            nc.vector.tensor_reduce(rT_sb[:, dj * 8 + b: dj * 8 + b + 1],
                                                    wt_[:, 0:T_steps], axis=mybir.AxisListType.X,
                                                    op=ALU.add)
                nc.vector.tensor_copy(rT_bf[:], rT_sb[:])

            # ================= final MLP + softmax ==========================
            with (
                tc.tile_pool(name="fin", bufs=1) as fp,
                tc.tile_pool(name="fps", bufs=1, space="PSUM") as fpsm,
            ):
                rr_ps = fpsm.tile([128, 16], F32, space="PSUM")
                hlast = hT[:, T_steps * 16:(T_steps + 1) * 16]
                for oc in range(2):
                    for kc in range(2):
                        nc.tensor.matmul(rr_ps[:, oc * 8:(oc + 1) * 8],
                                         wpT_sb[:, kc * 256 + oc * 128: kc * 256 + (oc + 1) * 128],
                                         rT_bf[:, kc * 8:(kc + 1) * 8],
                                         start=(kc == 0), stop=False)
                    for kc in range(2):
                        nc.tensor.matmul(rr_ps[:, oc * 8:(oc + 1) * 8],
                                         wxT_sb[:, kc * 256 + oc * 128: kc * 256 + (oc + 1) * 128],
                                         hlast[:, kc * 8:(kc + 1) * 8],
                                         start=False, stop=(kc == 1))
                rrT = fp.tile([128, 16], BF16)
                nc.scalar.activation(rrT[:], rr_ps[:], AF.Tanh)
                z_ps = fpsm.tile([128, 16], F32, space="PSUM")
                for oc in range(2):
                    for kc in range(2):
                        nc.tensor.matmul(z_ps[:, oc * 8:(oc + 1) * 8],
                                         wfT_sb[:, kc * 256 + oc * 128: kc * 256 + (oc + 1) * 128],
                                         rrT[:, kc * 8:(kc + 1) * 8],
                                         start=(kc == 0), stop=(kc == 1))
                e_sb = fp.tile([128, 16], F32)
                for oc in range(2):
                    nc.scalar.activation(e_sb[:, oc * 8:(oc + 1) * 8], z_ps[:, oc * 8:(oc + 1) * 8],
                                         AF.Exp, bias=bf_sb[:, oc:oc + 1])
                cs_ps = fpsm.tile([1, 16], F32, space="PSUM")
                nc.tensor.matmul(cs_ps[:], ones128[:], e_sb[:], start=True, stop=True)
                cs_sb = fp.tile([1, 16], F32)
                nc.vector.tensor_copy(cs_sb[:], cs_ps[:])
                s8 = fp.tile([1, 8], F32)
                nc.vector.tensor_tensor(s8[:], cs_sb[0:1, 0:8], cs_sb[0:1, 8:16], op=ALU.add)
                rc8 = fp.tile([1, 8], F32)
                nc.vector.reciprocal(rc8[:], s8[:])
                rc16 = fp.tile([1, 16], F32)
                nc.vector.tensor_copy(rc16[:, 0:8], rc8[:])
                nc.vector.tensor_copy(rc16[:, 8:16], rc8[:])
                rbc_ps = fpsm.tile([128, 16], F32, space="PSUM")
                nc.tensor.matmul(rbc_ps[:], ones1w[:], rc16[:], start=True, stop=True)
                yT_sb = fp.tile([128, 16], F32)
                nc.vector.tensor_tensor(yT_sb[:], e_sb[:], rbc_ps[:], op=ALU.mult)
                ytr_ps = fpsm.tile([16, 128], F32, space="PSUM")
                nc.tensor.transpose(ytr_ps[:], yT_sb[:], ident[:])
                ynat = fp.tile([16, 128], F32)
                nc.vector.tensor_copy(ynat[:], ytr_ps[:])
                for oc in range(2):
                    nc.sync.dma_start(out_e[0:8, oc * 128:(oc + 1) * 128],
                                      ynat[oc * 8:(oc + 1) * 8, :])

    nc.compile()
    return nc


def build(T_steps=512, variant=None):
    variant = VARIANT if variant is None else variant
    if variant == "h":
        return build_h(T_steps)
    nc = bacc.Bacc(None, target_bir_lowering=False)
    NT = T_steps * BL // 128          # gather tiles of 128 tokens
    NCH = T_steps * BL // 512         # 512-token chunks for xs matmul

    ten = _declare(nc, T_steps)
    emb_e, xp_e, sidx_e = ten['emb'], ten['xp'], ten['sidx']
    wihT_e, whhT_e, whh8_e, bl_e = ten['wihT'], ten['whhT'], ten['whh8'], ten['bl']
    wy_e, wtoh_e, sel_e = ten['wy'], ten['wtoh'], ten['sel']
    wpT_e, wxT_e, wfT_e, bf_e = ten['wpT'], ten['wxT'], ten['wfT'], ten['bf']
    out_e, s2d = ten['out'], ten['s2d']

    with tile.TileContext(nc) as tc:
        with (
            tc.tile_pool(name="const", bufs=1) as cp,
            tc.tile_pool(name="big", bufs=1) as bigp,
        ):
            # ---- constants / weights to SBUF ----
            xp_sb = cp.tile([128, NT], I32)
            sidx_sb = cp.tile([64, 1], I32)
            wihT_sb = cp.tile([128, 2048], BF16)
            whhT_sb = cp.tile([128, 2048], BF16)
            whh8_sb = cp.tile([128, 2048], mybir.dt.float8e4)
            bl_sb = cp.tile([128, 8], F32)
            wy_sb = cp.tile([128, 512], BF16)
            wtoh_sb = cp.tile([128, 128], BF16)
            sel_sb = cp.tile([8, 1024], BF16)
            wpT_sb = cp.tile([128, 512], BF16)
            wxT_sb = cp.tile([128, 512], BF16)
            wfT_sb = cp.tile([128, 512], BF16)
            bf_sb = cp.tile([128, 2], F32)
            for dst, src in [(xp_sb, xp_e), (sidx_sb, sidx_e), (wihT_sb, wihT_e),
                             (whhT_sb, whhT_e), (whh8_sb, whh8_e), (bl_sb, bl_e), (wy_sb, wy_e),
                             (wtoh_sb, wtoh_e), (sel_sb, sel_e), (wpT_sb, wpT_e),
                             (wxT_sb, wxT_e), (wfT_sb, wfT_e), (bf_sb, bf_e)]:
                nc.sync.dma_start(dst[:], src[:])
            ident = cp.tile([128, 128], F32)
            make_identity(nc, ident[:])
            ident_bf = cp.tile([128, 128], BF16)
            nc.vector.tensor_copy(ident_bf[:], ident[:])
            ones64 = cp.tile([64, 1], F32)
            nc.gpsimd.memset(ones64[:], 1.0)
            ones1w = cp.tile([1, 128], F32)
            nc.gpsimd.memset(ones1w[:], 1.0)
            ones128 = cp.tile([128, 1], F32)
            nc.gpsimd.memset(ones128[:], 1.0)

            # ---- persistent big tensors ----
            eT0 = bigp.tile([128, T_steps * 8], BF16)
            eT1 = bigp.tile([128, T_steps * 8], BF16)
            xsT = bigp.tile([128, T_steps * 64], BF16)
            hT = bigp.tile([128, (T_steps + 1) * 16], BF16)
            s2_sb = bigp.tile([8, 512], BF16)
            rT_sb = bigp.tile([128, 16], F32)
            rT_bf = bigp.tile([128, 16], BF16)
            a_sb = bigp.tile([8, T_steps], BF16)

            # ================= s-branch (batchnormed aspect embedding) ======
            with (
                tc.tile_pool(name="swork", bufs=1) as sw,
                tc.tile_pool(name="spsum", bufs=1, space="PSUM") as sps,
            ):
                semb = sw.tile([64, 256], F32)
                nc.gpsimd.indirect_dma_start(
                    out=semb[:], out_offset=None, in_=emb_e[:],
                    in_offset=IndirectOffsetOnAxis(ap=sidx_sb[:, :1], axis=0))
                mu_ps = sps.tile([1, 256], F32, space="PSUM")
                nc.tensor.matmul(mu_ps[:], ones64[:], semb[:], start=True, stop=True)
                mu = sw.tile([1, 256], F32)
                nc.vector.tensor_scalar_mul(mu[:], mu_ps[:], 1.0 / 64)
                sq = sw.tile([64, 256], F32)
                nc.vector.tensor_mul(sq[:], semb[:], semb[:])
                ms_ps = sps.tile([1, 256], F32, space="PSUM")
                nc.tensor.matmul(ms_ps[:], ones64[:], sq[:], start=True, stop=True)
                msq = sw.tile([1, 256], F32)
                nc.vector.tensor_scalar_mul(msq[:], ms_ps[:], 1.0 / 64)
                mu2 = sw.tile([1, 256], F32)
                nc.vector.tensor_mul(mu2[:], mu[:], mu[:])
                var = sw.tile([1, 256], F32)
                nc.vector.tensor_tensor(var[:], msq[:], mu2[:], op=ALU.subtract)
                nc.vector.tensor_scalar_add(var[:], var[:], 1e-5)
                std = sw.tile([1, 256], F32)
                nc.scalar.sqrt(std[:], var[:])
                istd = sw.tile([1, 256], F32)
                nc.vector.reciprocal(istd[:], std[:])
                mub_ps = sps.tile([64, 256], F32, space="PSUM")
                nc.tensor.matmul(mub_ps[:], ones1w[:1, :64], mu[:], start=True, stop=True)
                ib_ps = sps.tile([64, 256], F32, space="PSUM")
                nc.tensor.matmul(ib_ps[:], ones1w[:1, :64], istd[:], start=True, stop=True)
                d8 = sw.tile([8, 256], F32)
                nc.vector.tensor_tensor(d8[:], semb[0:8, :], mub_ps[0:8, :], op=ALU.subtract)
                nc.vector.tensor_tensor(s2_sb[:, 0:256], d8[:], ib_ps[0:8, :], op=ALU.mult)
                nc.vector.tensor_copy(s2_sb[:, 256:512], s2_sb[:, 0:256])
                nc.gpsimd.dma_start(s2d[:], s2_sb[:])

            # ========= gather/xs/Weff/Y/score interleaved with recurrence ===
            xs_v = xsT[:].rearrange("p (t q) -> p t q", q=64)
            hT_v = hT[:].rearrange("p (t dj b) -> p dj b t", dj=2, b=8)
            weff_all = bigp.tile([128, 4096], BF16)
            nc.gpsimd.memset(hT[:, 0:16], 0.0)
            with (
                tc.tile_pool(name="gat", bufs=8) as gp,
                tc.tile_pool(name="xps", bufs=1, space="PSUM") as xpsm,
                tc.tile_pool(name="rec", bufs=3) as rp,
                tc.tile_pool(name="cst", bufs=3) as cpp,
                tc.tile_pool(name="rps", bufs=2, space="PSUM") as rpsm,
                tc.tile_pool(name="yps", bufs=2, space="PSUM") as ypsm,
                tc.tile_pool(name="ytp", bufs=3) as ytp,
                tc.tile_pool(name="scp", bufs=1, space="PSUM") as scpsm,
                tc.tile_pool(name="att", bufs=2) as ap_,
                tc.tile_pool(name="sm", bufs=1) as smp,
            ):
                sc_ps = scpsm.tile([8, T_steps], F32, space="PSUM")

                def emit_gather(g):
                    egath = gp.tile([128, 256], F32)
                    nc.gpsimd.indirect_dma_start(
                        out=egath[:], out_offset=None, in_=emb_e[:],
                        in_offset=IndirectOffsetOnAxis(ap=xp_sb[:, g:g + 1], axis=0))
                    return egath

                def emit_trcopy(egath, g, dc):
                    eT = eT0 if dc == 0 else eT1
                    tps = xpsm.tile([128, 128], F32, space="PSUM", tag="ps")
                    nc.tensor.transpose(tps[:], egath[:, dc * 128:(dc + 1) * 128], ident[:])
                    nc.vector.tensor_copy(eT[:, g * 128:(g + 1) * 128], tps[:])

                def emit_xs_mm(nch, gb, holder):
                    xps = xpsm.tile([128, 512], F32, space="PSUM", tag="ps")
                    nc.tensor.matmul(xps[:], wihT_sb[:, gb * 128:(gb + 1) * 128],
                                     eT0[:, nch * 512:(nch + 1) * 512],
                                     start=True, stop=False)
                    nc.tensor.matmul(xps[:], wihT_sb[:, 1024 + gb * 128:1024 + (gb + 1) * 128],
                                     eT1[:, nch * 512:(nch + 1) * 512],
                                     start=False, stop=True)
                    holder['xps'] = xps

                def emit_xs_ts(nch, gb, holder):
                    nc.vector.tensor_scalar(
                        xs_v[:, nch * 64:(nch + 1) * 64, gb * 8:(gb + 1) * 8],
                        holder['xps'], bl_sb[:, gb:gb + 1], None, op0=ALU.add)

                def emit_xs(nch, gb):
                    h = {}
                    emit_xs_mm(nch, gb, h)
                    emit_xs_ts(nch, gb, h)

                egaths = {}

                def chunk_items(k):
                    items = []
                    for g in range(4 * k, 4 * k + 4):
                        if variant == "g":
                            for dc in range(2):
                                items.append(lambda g=g, dc=dc: emit_trcopy(egaths.pop(g) if dc else egaths[g], g, dc))
                        else:
                            holder = {}
                            items.append(lambda g=g, h=holder: h.__setitem__('e', emit_gather(g)))
                            for dc in range(2):
                                items.append(lambda g=g, dc=dc, h=holder: emit_trcopy(h['e'], g, dc))
                    for gb in range(8):
                        h = {}
                        items.append(lambda k=k, gb=gb, h=h: emit_xs_mm(k, gb, h))
                        items.append(lambda k=k, gb=gb, h=h: emit_xs_ts(k, gb, h))
                    return items

                if variant == "g":
                    for g in range(NT):
                        egaths[g] = emit_gather(g)

                # --- Weff = C_b @ W_y (only needs the s-branch) ---
                def emit_cw(b, holder):
                    cw = ap_.tile([128, 384], BF16)
                    for j in range(3):
                        win = bass.AP(s2d[:].tensor, b * 512 + j * 128, [[1, 128], [1, 128]])
                        nc.gpsimd.dma_start(cw[:, j * 128:(j + 1) * 128], win)
                    holder['cw'] = cw

                def emit_weff_mm(b, mj, holder):
                    cw = holder['cw']
                    wps = ypsm.tile([128, 256], F32, space="PSUM", tag="yps")
                    for kc in range(2):
                        nc.tensor.matmul(wps[:], cw[:, (mj + kc) * 128:(mj + kc + 1) * 128],
                                         wy_sb[:, kc * 256:(kc + 1) * 256],
                                         start=(kc == 0), stop=(kc == 1))
                    holder['wps' + str(mj)] = wps

                def emit_weff_cp(b, mj, holder):
                    nc.vector.tensor_copy(weff_all[:, b * 512 + mj * 256: b * 512 + (mj + 1) * 256],
                                          holder['wps' + str(mj)])

                def emit_ygroup(k, b, ec):
                    yc = ypsm.tile([128, 64], F32, space="PSUM", tag="yps")
                    for kc in range(2):
                        nc.tensor.matmul(
                            yc[:], weff_all[:, b * 512 + kc * 256 + ec * 128: b * 512 + kc * 256 + (ec + 1) * 128],
                            hT_v[:, kc, b, 1 + 64 * k: 1 + 64 * (k + 1)],
                            start=(kc == 0), stop=(kc == 1))
                    yt = ytp.tile([128, 64], BF16)
                    nc.scalar.activation(yt[:], yc[:], AF.Tanh)
                    nc.tensor.matmul(sc_ps[:, 64 * k:64 * (k + 1)],
                                     wtoh_sb[:, ec * 64 + b * 8: ec * 64 + (b + 1) * 8],
                                     yt[:], start=(b == 0 and ec == 0), stop=(b == 7 and ec == 1))

                # chunk 0 of gather/xs up front; everything else trickles in
                for it in chunk_items(0):
                    it()
                xs_work = []
                for k in range(1, NCH):
                    xs_work.extend(chunk_items(k))
                y_work = []   # (avail_step, fn)
                for b in range(8):
                    holder = {}
                    y_work.append((0, lambda b=b, h=holder: emit_cw(b, h)))
                    for mj in range(2):
                        y_work.append((0, lambda b=b, mj=mj, h=holder: emit_weff_mm(b, mj, h)))
                        y_work.append((0, lambda b=b, mj=mj, h=holder: emit_weff_cp(b, mj, h)))
                for k in range(T_steps // 64):
                    for b in range(8):
                        for ec in range(2):
                            y_work.append((64 * (k + 1),
                                           lambda k=k, b=b, ec=ec: emit_ygroup(k, b, ec)))
                xi = yi = 0

                c_prev = cpp.tile([128, 16], F32)
                nc.vector.memset(c_prev[:], 0.0)
                def emit_imm(tt):
                    gi = rpsm.tile([128, 48], F32, space="PSUM", tag="gpi")
                    gg_ = rpsm.tile([128, 16], F32, space="PSUM", tag="gpg")
                    nc.tensor.matmul(gi[:], ident_bf[:], xsT[:, tt * 64: tt * 64 + 48],
                                     start=True, stop=False)
                    nc.tensor.matmul(gg_[:], ident_bf[:], xsT[:, tt * 64 + 48: tt * 64 + 64],
                                     start=True, stop=False)
                    return gi, gg_

                if variant == "d":
                    gpi, gpg = emit_imm(0)
                for t in range(T_steps):
                    if variant != "d":
                        gpi, gpg = emit_imm(t)
                    gb_order = (6, 7, 0, 1, 2, 3, 4, 5) if variant == "d" else (0, 1, 2, 3, 4, 5, 6, 7)
                    w_sb = whh8_sb if variant == "f" else whhT_sb
                    for gb in gb_order:
                        out = gpi[:, gb * 8:(gb + 1) * 8] if gb < 6 else gpg[:, (gb - 6) * 8:(gb - 5) * 8]
                        for dj in range(2):
                            nc.tensor.matmul(
                                out,
                                w_sb[:, dj * 1024 + gb * 128: dj * 1024 + (gb + 1) * 128],
                                hT[:, t * 16 + dj * 8: t * 16 + (dj + 1) * 8],
                                start=False, stop=(dj == 1 and gb in (5, 7)))
                    gpi_t, gpg_t = gpi, gpg
                    if variant == "d" and t + 1 < T_steps:
                        gpi, gpg = emit_imm(t + 1)
                    if variant == "d":
                        gg = rp.tile([128, 16], F32)
                        nc.scalar.activation(gg[:], gpg_t[:], AF.Tanh)
                        sig = rp.tile([128, 48], F32)
                        nc.scalar.activation(sig[:], gpi_t[:], AF.Sigmoid)
                    else:
                        sig = rp.tile([128, 48], F32)
                        nc.scalar.activation(sig[:], gpi_t[:], AF.Sigmoid)
                        gg = rp.tile([128, 16], F32)
                        nc.scalar.activation(gg[:], gpg_t[:], AF.Tanh)
                    m1 = rp.tile([128, 16], F32)
                    nc.vector.tensor_mul(m1[:], sig[:, 16:32], c_prev[:])
                    m2 = rp.tile([128, 16], F32)
                    nc.vector.tensor_mul(m2[:], sig[:, 0:16], gg[:])
                    c_new = cpp.tile([128, 16], F32)
                    nc.vector.tensor_tensor(c_new[:], m1[:], m2[:], op=ALU.add)
                    thc = rp.tile([128, 16], F32)
                    nc.scalar.activation(thc[:], c_new[:], AF.Tanh)
                    nc.vector.tensor_mul(hT[:, (t + 1) * 16:(t + 2) * 16], sig[:, 32:48], thc[:])
                    c_prev = c_new
                    if xi < len(xs_work):
                        xs_work[xi]()
                        xi += 1
                    if yi < len(y_work) and y_work[yi][0] <= t and (t >= 96 or t % 2 == 0):
                        y_work[yi][1]()
                        yi += 1
                while xi < len(xs_work):
                    xs_work[xi]()
                    xi += 1
                while yi < len(y_work):
                    y_work[yi][1]()
                    yi += 1

                # ---- softmax over T (free axis) ----
                mx = smp.tile([8, 1], F32)
                nc.vector.tensor_reduce(mx[:], sc_ps[:, 0:T_steps], axis=mybir.AxisListType.X, op=ALU.max)
                nmx = smp.tile([8, 1], F32)
                nc.vector.tensor_scalar_mul(nmx[:], mx[:], -1.0)
                esc = smp.tile([8, T_steps], F32)
                ssum = smp.tile([8, 1], F32)
                nc.scalar.activation(esc[:], sc_ps[:, 0:T_steps], AF.Exp,
                                     bias=nmx[:, 0:1], accum_out=ssum[:, 0:1])
                rcs = smp.tile([8, 1], F32)
                nc.vector.reciprocal(rcs[:], ssum[:])
                nc.scalar.activation(a_sb[:], esc[:], AF.Copy, scale=rcs[:, 0:1])
                # ---- r = sum_t a_t * h_t ----
                for b in range(8):
                    abc = ypsm.tile([128, T_steps], F32, space="PSUM", tag="yps")
                    nc.tensor.matmul(abc[:, 0:T_steps], sel_sb[:, b * 128:(b + 1) * 128],
                                     a_sb[:], start=True, stop=True)
                    for dj in range(2):
                        wt_ = ap_.tile([128, T_steps], F32)
                        nc.vector.tensor_tensor(wt_[:, 0:T_steps], hT_v[:, dj, b, 1:T_steps + 1],
                                                abc[:, 0:T_steps], op=ALU.mult)
                        nc.vector.tensor_reduce(rT_sb[:, dj * 8 + b: dj * 8 + b + 1],
                                                wt_[:, 0:T_steps], axis=mybir.AxisListType.X,
                                                op=ALU.add)
                nc.vector.tensor_copy(rT_bf[:], rT_sb[:])

            # ================= final MLP + softmax ==========================
            with (
                tc.tile_pool(name="fin", bufs=1) as fp,
                tc.tile_pool(name="fps", bufs=1, space="PSUM") as fpsm,
            ):
                rr_ps = fpsm.tile([128, 16], F32, space="PSUM")
                hlast = hT[:, T_steps * 16:(T_steps + 1) * 16]
                for oc in range(2):
                    for kc in range(2):
                        nc.tensor.matmul(rr_ps[:, oc * 8:(oc + 1) * 8],
                                         wpT_sb[:, kc * 256 + oc * 128: kc * 256 + (oc + 1) * 128],
                                         rT_bf[:, kc * 8:(kc + 1) * 8],
                                         start=(kc == 0), stop=False)
                    for kc in range(2):
                        nc.tensor.matmul(rr_ps[:, oc * 8:(oc + 1) * 8],
                                         wxT_sb[:, kc * 256 + oc * 128: kc * 256 + (oc + 1) * 128],
                                         hlast[:, kc * 8:(kc + 1) * 8],
                                         start=False, stop=(kc == 1))
                rrT = fp.tile([128, 16], BF16)
                nc.scalar.activation(rrT[:], rr_ps[:], AF.Tanh)
                z_ps = fpsm.tile([128, 16], F32, space="PSUM")
                for oc in range(2):
                    for kc in range(2):
                        nc.tensor.matmul(z_ps[:, oc * 8:(oc + 1) * 8],
                                         wfT_sb[:, kc * 256 + oc * 128: kc * 256 + (oc + 1) * 128],
                                         rrT[:, kc * 8:(kc + 1) * 8],
                                         start=(kc == 0), stop=(kc == 1))
                e_sb = fp.tile([128, 16], F32)
                for oc in range(2):
                    nc.scalar.activation(e_sb[:, oc * 8:(oc + 1) * 8], z_ps[:, oc * 8:(oc + 1) * 8],
                                         AF.Exp, bias=bf_sb[:, oc:oc + 1])
                cs_ps = fpsm.tile([1, 16], F32, space="PSUM")
                nc.tensor.matmul(cs_ps[:], ones128[:], e_sb[:], start=True, stop=True)
                cs_sb = fp.tile([1, 16], F32)
                nc.vector.tensor_copy(cs_sb[:], cs_ps[:])
                s8 = fp.tile([1, 8], F32)
                nc.vector.tensor_tensor(s8[:], cs_sb[0:1, 0:8], cs_sb[0:1, 8:16], op=ALU.add)
                rc8 = fp.tile([1, 8], F32)
                nc.vector.reciprocal(rc8[:], s8[:])
                rc16 = fp.tile([1, 16], F32)
                nc.vector.tensor_copy(rc16[:, 0:8], rc8[:])
                nc.vector.tensor_copy(rc16[:, 8:16], rc8[:])
                rbc_ps = fpsm.tile([128, 16], F32, space="PSUM")
                nc.tensor.matmul(rbc_ps[:], ones1w[:], rc16[:], start=True, stop=True)
                yT_sb = fp.tile([128, 16], F32)
                nc.vector.tensor_tensor(yT_sb[:], e_sb[:], rbc_ps[:], op=ALU.mult)
                ytr_ps = fpsm.tile([16, 128], F32, space="PSUM")
                nc.tensor.transpose(ytr_ps[:], yT_sb[:], ident[:])
                ynat = fp.tile([16, 128], F32)
                nc.vector.tensor_copy(ynat[:], ytr_ps[:])
                for oc in range(2):
                    nc.sync.dma_start(out_e[0:8, oc * 128:(oc + 1) * 128],
                                      ynat[oc * 8:(oc + 1) * 8, :])

    nc.compile()
    return nc


_CACHE = {}


def _get_nc(T_steps=512, variant=None):
    key = (T_steps, VARIANT if variant is None else variant)
    if key not in _CACHE:
        _CACHE[key] = build(T_steps, variant=key[1])
    return _CACHE[key]


def make_in_maps(x, s, emb, w_ih, w_hh, b_lstm, W_y, w_t, W_p, W_x, W_f, b_f,
                 T_steps=512):
    x = np.asarray(x).astype(np.int32)[:, :T_steps]
    s = np.asarray(s).astype(np.int32).reshape(64)
    emb = np.ascontiguousarray(np.asarray(emb, dtype=np.float32))
    wih_p = np.asarray(w_ih, dtype=np.float32)[_PERM]
    whh_p = np.asarray(w_hh, dtype=np.float32)[_PERM]
    bl_p = np.asarray(b_lstm, dtype=np.float32)[_PERM]

    def wt2sb(wT):  # [256, 1024] -> [128, 2048]
        return np.concatenate([wT[0:128], wT[128:256]], axis=1)

    wihT = wt2sb(wih_p.T).astype(bf16)
    whhT = wt2sb(whh_p.T).astype(bf16)
    bl_sb = bl_p.reshape(8, 128).T.copy().astype(np.float32)  # [128, 8]
    wy_sb = np.concatenate([np.asarray(W_y, np.float32)[0:128],
                            np.asarray(W_y, np.float32)[128:256]], axis=1).astype(bf16)
    w_t = np.asarray(w_t, np.float32)
    wtoh = np.zeros((128, 128), np.float32)
    for ec in range(2):
        for b in range(8):
            wtoh[:, ec * 64 + b * 8 + b] = w_t[ec * 128:(ec + 1) * 128]
    wtoh = wtoh.astype(bf16)
    sel = np.zeros((8, 1024), np.float32)
    for b in range(8):
        sel[b, b * 128:(b + 1) * 128] = 1.0
    sel = sel.astype(bf16)

    def t2sb(w):  # W [do, din] -> lhsT layout [128, 512] free=kc*256+do
        wT = np.asarray(w, np.float32).T  # [din, do]
        return np.concatenate([wT[0:128], wT[128:256]], axis=1).astype(bf16)

    wpT = t2sb(W_p)
    wxT = t2sb(W_x)
    wfT = t2sb(W_f)
    bf_sb = np.asarray(b_f, np.float32).reshape(2, 128).T.copy()

    whh8 = wt2sb(whh_p.T).astype(ml_dtypes.float8_e4m3)
    common = dict(emb=emb, wihT=wihT, whhT=whhT, whh8=whh8, bl=bl_sb, wy=wy_sb, wtoh=wtoh,
                  sel=sel, wpT=wpT, wxT=wxT, wfT=wfT, bf=bf_sb)
    in_maps = []
    for c in range(NCORES):
        xs_c = x[c * BL:(c + 1) * BL]                      # [8, T]
        xflat = xs_c.T.reshape(-1)                         # t-major tokens
        xp = xflat.reshape(-1, 128).T.copy()               # [128, NT]
        sidx = np.roll(s, -BL * c).reshape(64, 1).copy()
        in_maps.append(dict(xp=xp, sidx=sidx, **common))
    return in_maps


def _install_trace_shim():
    """The agent image lacks antenv.axon_hooks; recreate it and install the
    ctypes NTFF hook from trn_boot so run_bass_kernel_spmd(trace=True) works."""
    import sys, types
    if "antenv.axon_hooks" not in sys.modules:
        mod = types.ModuleType("antenv.axon_hooks")
        mod._hook = None
        mod.set_axon_ntff_profile_hook = lambda h: setattr(mod, "_hook", h)
        mod.get_axon_ntff_profile_hook = lambda: mod._hook
        sys.modules["antenv.axon_hooks"] = mod
        import antenv
        antenv.axon_hooks = mod
    import antenv.axon_hooks as ah
    if ah.get_axon_ntff_profile_hook() is None:
        from trn_agent_boot.trn_boot import _ntff_profile_via_ctypes
        ah.set_axon_ntff_profile_hook(_ntff_profile_via_ctypes("/opt/axon/libaxon_pjrt.so"))
    import concourse.bass_utils as bu
    bu.upload_artifacts = lambda tmpdir: ""


def run(in_maps, T_steps=512, trace=False, tmpdir=None, variant=None):
    nc = _get_nc(T_steps, variant)
    if trace:
        _install_trace_shim()
    return run_bass_kernel_spmd(nc, in_maps, core_ids=list(range(NCORES)),
                                trace=trace, tmpdir=tmpdir)


def kernel(x, s, emb, w_ih, w_hh, b_lstm, W_y, w_t, W_p, W_x, W_f, b_f):
    in_maps = make_in_maps(x, s, emb, w_ih, w_hh, b_lstm, W_y, w_t, W_p, W_x,
                           W_f, b_f)
    res = run(in_maps)
    return np.concatenate([res.results[i]["out"] for i in range(NCORES)], axis=0)


# revision 16
# speedup vs baseline: 1.1129x; 1.0344x over previous
"""AF-LSTM Trainium2 kernel: 8-way batch-parallel, no collectives.

Per core (8 batch rows): gather embeddings, LSTM recurrence in transposed
(gate-major) layout, circular-correlation attention via per-batch circulant
matmul folded into W_y, final MLP + softmax. Host concatenates per-core
[8,256] outputs.

Variant "h": latency-tuned recurrence chain — g-gates matmul first so
tanh(g) hides under sigma(i,f); packed [sig_i|sig_f] * [tanh_g|c] product;
sigma(o) off the critical path; all embedding gathers issued up-front so the
PE queue never head-of-line blocks on gather DMAs; s-branch and circulant
window DMAs moved off the GpSimd/PE critical queues; fused
tensor_tensor_reduce for the attention readout.
"""

import numpy as np
import ml_dtypes

import concourse.bacc as bacc
import concourse.tile as tile
from concourse import bass, mybir
from concourse.bass import IndirectOffsetOnAxis
from concourse.bass_utils import run_bass_kernel_spmd
from concourse.masks import make_identity

F32 = mybir.dt.float32
BF16 = mybir.dt.bfloat16
I32 = mybir.dt.int32
AF = mybir.ActivationFunctionType
ALU = mybir.AluOpType

V, D, B = 50000, 256, 64
NCORES, BL = 8, 8
G4 = 4 * D
bf16 = ml_dtypes.bfloat16

# gate blocks of 128 rows reordered to [i0,i1,f0,f1,o0,o1,g0,g1]
_PERM = np.concatenate([
    np.arange(0, 256),        # i
    np.arange(256, 512),      # f
    np.arange(768, 1024),     # o
    np.arange(512, 768),      # g
])


VARIANT = "i"

import os as _os
H_CW_SYNC = _os.environ.get("H_CW_SYNC", "1") == "1"
H_TTR = _os.environ.get("H_TTR", "0") == "1"


def _declare(nc, T_steps):
    NT = T_steps * BL // 128
    ten = {}
    ten['emb'] = nc.declare_dram_parameter("emb", [V, D], F32, isOutput=False)
    ten['xp'] = nc.declare_dram_parameter("xp", [128, NT], I32, isOutput=False)
    ten['sidx'] = nc.declare_dram_parameter("sidx", [64, 1], I32, isOutput=False)
    ten['wihT'] = nc.declare_dram_parameter("wihT", [128, 2048], BF16, isOutput=False)
    ten['whhT'] = nc.declare_dram_parameter("whhT", [128, 2048], BF16, isOutput=False)
    ten['whh8'] = nc.declare_dram_parameter("whh8", [128, 2048], mybir.dt.float8e4, isOutput=False)
    ten['bl'] = nc.declare_dram_parameter("bl", [128, 8], F32, isOutput=False)
    ten['wy'] = nc.declare_dram_parameter("wy", [128, 512], BF16, isOutput=False)
    ten['wtoh'] = nc.declare_dram_parameter("wtoh", [128, 128], BF16, isOutput=False)
    ten['sel'] = nc.declare_dram_parameter("sel", [8, 1024], BF16, isOutput=False)
    ten['wpT'] = nc.declare_dram_parameter("wpT", [128, 512], BF16, isOutput=False)
    ten['wxT'] = nc.declare_dram_parameter("wxT", [128, 512], BF16, isOutput=False)
    ten['wfT'] = nc.declare_dram_parameter("wfT", [128, 512], BF16, isOutput=False)
    ten['bf'] = nc.declare_dram_parameter("bf", [128, 2], F32, isOutput=False)
    ten['out'] = nc.declare_dram_parameter("out", [8, 256], F32, isOutput=True)
    ten['s2d'] = nc.dram_tensor("s2d", [8, 512], BF16)
    return ten


def build_i(T_steps=512):
    """Host-gathered variant: embeddings arrive pre-gathered/transposed in
    et0/et1, the normalized aspect rows in s2. No indirect DMA, no on-device
    batchnorm, no PE transposes."""
    nc = bacc.Bacc(None, target_bir_lowering=False)
    NCH = T_steps * BL // 512         # 512-token chunks for xs matmul
    et0_e = nc.declare_dram_parameter("et0", [128, T_steps * 8], BF16, isOutput=False)
    et1_e = nc.declare_dram_parameter("et1", [128, T_steps * 8], BF16, isOutput=False)
    s2_e = nc.declare_dram_parameter("s2", [8, 512], BF16, isOutput=False)
    wihT_e = nc.declare_dram_parameter("wihT", [128, 2048], BF16, isOutput=False)
    whhT_e = nc.declare_dram_parameter("whhT", [128, 2048], BF16, isOutput=False)
    bl_e = nc.declare_dram_parameter("bl", [128, 8], F32, isOutput=False)
    wy_e = nc.declare_dram_parameter("wy", [128, 512], BF16, isOutput=False)
    wtoh_e = nc.declare_dram_parameter("wtoh", [128, 128], BF16, isOutput=False)
    sel_e = nc.declare_dram_parameter("sel", [8, 1024], BF16, isOutput=False)
    wpT_e = nc.declare_dram_parameter("wpT", [128, 512], BF16, isOutput=False)
    wxT_e = nc.declare_dram_parameter("wxT", [128, 512], BF16, isOutput=False)
    wfT_e = nc.declare_dram_parameter("wfT", [128, 512], BF16, isOutput=False)
    bf_e = nc.declare_dram_parameter("bf", [128, 2], F32, isOutput=False)
    out_e = nc.declare_dram_parameter("out", [8, 256], F32, isOutput=True)

    with tile.TileContext(nc) as tc:
        with (
            tc.tile_pool(name="const", bufs=1) as cp,
            tc.tile_pool(name="big", bufs=1) as bigp,
        ):
            # ---- constants / weights to SBUF ----
            wihT_sb = cp.tile([128, 2048], BF16)
            whhT_sb = cp.tile([128, 2048], BF16)
            bl_sb = cp.tile([128, 8], F32)
            wy_sb = cp.tile([128, 512], BF16)
            wtoh_sb = cp.tile([128, 128], BF16)
            sel_sb = cp.tile([8, 1024], BF16)
            wpT_sb = cp.tile([128, 512], BF16)
            wxT_sb = cp.tile([128, 512], BF16)
            wfT_sb = cp.tile([128, 512], BF16)
            bf_sb = cp.tile([128, 2], F32)
            for dst, src in [(wihT_sb, wihT_e), (whhT_sb, whhT_e),
                             (bl_sb, bl_e), (wy_sb, wy_e),
                             (wtoh_sb, wtoh_e), (sel_sb, sel_e), (wpT_sb, wpT_e),
                             (wxT_sb, wxT_e), (wfT_sb, wfT_e), (bf_sb, bf_e)]:
                nc.sync.dma_start(dst[:], src[:])
            ident = cp.tile([128, 128], F32)
            make_identity(nc, ident[:])
            ident_bf = cp.tile([128, 128], BF16)
            nc.vector.tensor_copy(ident_bf[:], ident[:])
            ones1w = cp.tile([1, 128], F32)
            nc.gpsimd.memset(ones1w[:], 1.0)
            ones128 = cp.tile([128, 1], F32)
            nc.gpsimd.memset(ones128[:], 1.0)

            # ---- persistent big tensors ----
            eT0 = bigp.tile([128, T_steps * 8], BF16)
            eT1 = bigp.tile([128, T_steps * 8], BF16)
            xsT = bigp.tile([128, T_steps * 64], BF16)
            hT = bigp.tile([128, (T_steps + 1) * 16], BF16)
            rT_sb = bigp.tile([128, 16], F32)
            rT_bf = bigp.tile([128, 16], BF16)
            a_sb = bigp.tile([8, T_steps], BF16)
            weff_all = bigp.tile([128, 4096], BF16)
            cgA = bigp.tile([128, 32], F32)   # cols 0:16 tanh(g_t), 16:32 c
            cgB = bigp.tile([128, 32], F32)

            # chunk-0 embedding slices first so step 0 can start early
            nc.sync.dma_start(eT0[:, 0:512], et0_e[:, 0:512])
            nc.sync.dma_start(eT1[:, 0:512], et1_e[:, 0:512])
            if T_steps > 64:
                nc.sync.dma_start(eT0[:, 512:T_steps * 8], et0_e[:, 512:T_steps * 8])
                nc.sync.dma_start(eT1[:, 512:T_steps * 8], et1_e[:, 512:T_steps * 8])

            xs_v = xsT[:].rearrange("p (t q) -> p t q", q=64)
            hT_v = hT[:].rearrange("p (t dj b) -> p dj b t", dj=2, b=8)
            nc.vector.memset(hT[:, 0:16], 0.0)
            nc.vector.memset(cgA[:, 16:32], 0.0)

            with (
                tc.tile_pool(name="xps", bufs=1, space="PSUM") as xpsm,
                tc.tile_pool(name="rec", bufs=4) as rp,
                tc.tile_pool(name="rps", bufs=2, space="PSUM") as rpsm,
                tc.tile_pool(name="yps", bufs=2, space="PSUM") as ypsm,
                tc.tile_pool(name="ytp", bufs=3) as ytp,
                tc.tile_pool(name="scp", bufs=1, space="PSUM") as scpsm,
                tc.tile_pool(name="cwp", bufs=8) as cwp,
                tc.tile_pool(name="att", bufs=2) as ap_,
                tc.tile_pool(name="sm", bufs=1) as smp,
            ):
                sc_ps = scpsm.tile([8, T_steps], F32, space="PSUM")

                def emit_xs_mm(nch, gb, holder):
                    xps = xpsm.tile([128, 512], F32, space="PSUM", tag="ps")
                    nc.tensor.matmul(xps[:], wihT_sb[:, gb * 128:(gb + 1) * 128],
                                     eT0[:, nch * 512:(nch + 1) * 512],
                                     start=True, stop=False)
                    nc.tensor.matmul(xps[:], wihT_sb[:, 1024 + gb * 128:1024 + (gb + 1) * 128],
                                     eT1[:, nch * 512:(nch + 1) * 512],
                                     start=False, stop=True)
                    holder['xps'] = xps

                def emit_xs_ts_half(nch, gb, half, holder):
                    nc.vector.tensor_scalar(
                        xs_v[:, nch * 64 + half * 32:nch * 64 + (half + 1) * 32,
                             gb * 8:(gb + 1) * 8],
                        holder['xps'][:, half * 256:(half + 1) * 256],
                        bl_sb[:, gb:gb + 1], None, op0=ALU.add)

                def chunk_items(k):
                    items = []
                    for gb in range(8):
                        h = {}
                        items.append(lambda k=k, gb=gb, h=h: emit_xs_mm(k, gb, h))
                        for half in range(2):
                            items.append(lambda k=k, gb=gb, half=half, h=h:
                                         emit_xs_ts_half(k, gb, half, h))
                    return items

                # chunk-0 xs up front
                for it in chunk_items(0):
                    it()

                # circulant windows for all batch rows (sync DMA queue)
                cw_tiles = {}
                for b in range(8):
                    cw = cwp.tile([128, 384], BF16)
                    for j in range(3):
                        win = bass.AP(s2_e[:].tensor, b * 512 + j * 128, [[1, 128], [1, 128]])
                        nc.sync.dma_start(cw[:, j * 128:(j + 1) * 128], win)
                    cw_tiles[b] = cw

                # --- Weff = C_b @ W_y ---
                def emit_weff_mm(b, mj, holder):
                    cw = cw_tiles[b]
                    wps = ypsm.tile([128, 256], F32, space="PSUM", tag="yps")
                    for kc in range(2):
                        nc.tensor.matmul(wps[:], cw[:, (mj + kc) * 128:(mj + kc + 1) * 128],
                                         wy_sb[:, kc * 256:(kc + 1) * 256],
                                         start=(kc == 0), stop=(kc == 1))
                    holder['wps' + str(mj)] = wps

                def emit_weff_cp(b, mj, holder):
                    nc.vector.tensor_copy(
                        weff_all[:, b * 512 + mj * 256: b * 512 + (mj + 1) * 256],
                        holder['wps' + str(mj)])

                def emit_ygroup(k, b, ec):
                    yc = ypsm.tile([128, 64], F32, space="PSUM", tag="yps")
                    for kc in range(2):
                        nc.tensor.matmul(
                            yc[:], weff_all[:, b * 512 + kc * 256 + ec * 128:
                                            b * 512 + kc * 256 + (ec + 1) * 128],
                            hT_v[:, kc, b, 1 + 64 * k: 1 + 64 * (k + 1)],
                            start=(kc == 0), stop=(kc == 1))
                    yt = ytp.tile([128, 64], BF16)
                    nc.scalar.activation(yt[:], yc[:], AF.Tanh)
                    nc.tensor.matmul(sc_ps[:, 64 * k:64 * (k + 1)],
                                     wtoh_sb[:, ec * 64 + b * 8: ec * 64 + (b + 1) * 8],
                                     yt[:], start=(b == 0 and ec == 0), stop=(b == 7 and ec == 1))

                xs_work = []
                for k in range(1, NCH):
                    xs_work.extend(chunk_items(k))
                y_work = []   # (avail_step, fn)
                for b in range(8):
                    holder = {}
                    for mj in range(2):
                        y_work.append((16, lambda b=b, mj=mj, h=holder: emit_weff_mm(b, mj, h)))
                        y_work.append((16, lambda b=b, mj=mj, h=holder: emit_weff_cp(b, mj, h)))
                for k in range(T_steps // 64):
                    for b in range(8):
                        for ec in range(2):
                            y_work.append((64 * (k + 1),
                                           lambda k=k, b=b, ec=ec: emit_ygroup(k, b, ec)))
                xi = yi = 0

                def emit_imm(tt):
                    gi = rpsm.tile([128, 48], F32, space="PSUM", tag="gpi")
                    gg_ = rpsm.tile([128, 16], F32, space="PSUM", tag="gpg")
                    nc.tensor.matmul(gi[:], ident_bf[:], xsT[:, tt * 64: tt * 64 + 48],
                                     start=True, stop=False)
                    nc.tensor.matmul(gg_[:], ident_bf[:], xsT[:, tt * 64 + 48: tt * 64 + 64],
                                     start=True, stop=False)
                    return gi, gg_

                cg = [cgA, cgB]
                for t in range(T_steps):
                    gpi, gpg = emit_imm(t)
                    # g-gates first so tanh(g) hides under sigma(i,f)
                    for gb in (6, 7, 0, 1, 2, 3, 4, 5):
                        out = (gpi[:, gb * 8:(gb + 1) * 8] if gb < 6
                               else gpg[:, (gb - 6) * 8:(gb - 5) * 8])
                        for dj in range(2):
                            nc.tensor.matmul(
                                out,
                                whhT_sb[:, dj * 1024 + gb * 128: dj * 1024 + (gb + 1) * 128],
                                hT[:, t * 16 + dj * 8: t * 16 + (dj + 1) * 8],
                                start=False, stop=(dj == 1 and gb in (5, 7)))
                    cgt = cg[t % 2]
                    cgn = cg[(t + 1) % 2]
                    # Scalar queue: tanh(g), sigma(i,f), sigma(o), [tanh(c) later]
                    nc.scalar.activation(cgt[:, 0:16], gpg[:], AF.Tanh)
                    sig_if = rp.tile([128, 32], F32)
                    nc.scalar.activation(sig_if[:], gpi[:, 0:32], AF.Sigmoid)
                    sig_o = rp.tile([128, 16], F32)
                    nc.scalar.activation(sig_o[:], gpi[:, 32:48], AF.Sigmoid)
                    # DVE: packed product, pair-sum, then h
                    m12 = rp.tile([128, 32], F32)
                    nc.vector.tensor_mul(m12[:], sig_if[:], cgt[:])
                    nc.vector.tensor_tensor(cgn[:, 16:32], m12[:, 0:16], m12[:, 16:32],
                                            op=ALU.add)
                    thc = rp.tile([128, 16], F32)
                    nc.scalar.activation(thc[:], cgn[:, 16:32], AF.Tanh)
                    nc.vector.tensor_mul(hT[:, (t + 1) * 16:(t + 2) * 16], sig_o[:], thc[:])
                    if xi < len(xs_work):
                        xs_work[xi]()
                        xi += 1
                    if yi < len(y_work) and y_work[yi][0] <= t:
                        y_work[yi][1]()
                        yi += 1
                while xi < len(xs_work):
                    xs_work[xi]()
                    xi += 1
                while yi < len(y_work):
                    y_work[yi][1]()
                    yi += 1

                # ---- softmax over T (free axis) ----
                mx = smp.tile([8, 1], F32)
                nc.vector.tensor_reduce(mx[:], sc_ps[:, 0:T_steps], axis=mybir.AxisListType.X, op=ALU.max)
                nmx = smp.tile([8, 1], F32)
                nc.vector.tensor_scalar_mul(nmx[:], mx[:], -1.0)
                esc = smp.tile([8, T_steps], F32)
                ssum = smp.tile([8, 1], F32)
                nc.scalar.activation(esc[:], sc_ps[:, 0:T_steps], AF.Exp,
                                     bias=nmx[:, 0:1], accum_out=ssum[:, 0:1])
                rcs = smp.tile([8, 1], F32)
                nc.vector.reciprocal(rcs[:], ssum[:])
                nc.scalar.activation(a_sb[:], esc[:], AF.Copy, scale=rcs[:, 0:1])
                # ---- r = sum_t a_t * h_t ----
                for b in range(8):
                    abc = ypsm.tile([128, T_steps], F32, space="PSUM", tag="yps")
                    nc.tensor.matmul(abc[:, 0:T_steps], sel_sb[:, b * 128:(b + 1) * 128],
                                     a_sb[:], start=True, stop=True)
                    for dj in range(2):
                        wt_ = ap_.tile([128, T_steps], F32)
                        nc.vector.tensor_tensor(wt_[:, 0:T_steps], hT_v[:, dj, b, 1:T_steps + 1],
                                                abc[:, 0:T_steps], op=ALU.mult)
                        nc.vector.tensor_reduce(rT_sb[:, dj * 8 + b: dj * 8 + b + 1],
                                                wt_[:, 0:T_steps], axis=mybir.AxisListType.X,
                                                op=ALU.add)
                nc.vector.tensor_copy(rT_bf[:], rT_sb[:])

            # ================= final MLP + softmax ==========================
            with (
                tc.tile_pool(name="fin", bufs=1) as fp,
                tc.tile_pool(name="fps", bufs=1, space="PSUM") as fpsm,
            ):
                rr_ps = fpsm.tile([128, 16], F32, space="PSUM")
                hlast = hT[:, T_steps * 16:(T_steps + 1) * 16]
                for oc in range(2):
                    for kc in range(2):
                        nc.tensor.matmul(rr_ps[:, oc * 8:(oc + 1) * 8],
                                         wpT_sb[:, kc * 256 + oc * 128: kc * 256 + (oc + 1) * 128],
                                         rT_bf[:, kc * 8:(kc + 1) * 8],
                                         start=(kc == 0), stop=False)
                    for kc in range(2):
                        nc.tensor.matmul(rr_ps[:, oc * 8:(oc + 1) * 8],
                                         wxT_sb[:, kc * 256 + oc * 128: kc * 256 + (oc + 1) * 128],
                                         hlast[:, kc * 8:(kc + 1) * 8],
                                         start=False, stop=(kc == 1))
                rrT = fp.tile([128, 16], BF16)
                nc.scalar.activation(rrT[:], rr_ps[:], AF.Tanh)
                z_ps = fpsm.tile([128, 16], F32, space="PSUM")
                for oc in range(2):
                    for kc in range(2):
                        nc.tensor.matmul(z_ps[:, oc * 8:(oc + 1) * 8],
                                         wfT_sb[:, kc * 256 + oc * 128: kc * 256 + (oc + 1) * 128],
                                         rrT[:, kc * 8:(kc + 1) * 8],
                                         start=(kc == 0), stop=(kc == 1))
                e_sb = fp.tile([128, 16], F32)
                for oc in range(2):
                    nc.scalar.activation(e_sb[:, oc * 8:(oc + 1) * 8], z_ps[:, oc * 8:(oc + 1) * 8],
                                         AF.Exp, bias=bf_sb[:, oc:oc + 1])
                cs_ps = fpsm.tile([1, 16], F32, space="PSUM")
                nc.tensor.matmul(cs_ps[:], ones128[:], e_sb[:], start=True, stop=True)
                cs_sb = fp.tile([1, 16], F32)
                nc.vector.tensor_copy(cs_sb[:], cs_ps[:])
                s8 = fp.tile([1, 8], F32)
                nc.vector.tensor_tensor(s8[:], cs_sb[0:1, 0:8], cs_sb[0:1, 8:16], op=ALU.add)
                rc8 = fp.tile([1, 8], F32)
                nc.vector.reciprocal(rc8[:], s8[:])
                rc16 = fp.tile([1, 16], F32)
                nc.vector.tensor_copy(rc16[:, 0:8], rc8[:])
                nc.vector.tensor_copy(rc16[:, 8:16], rc8[:])
                rbc_ps = fpsm.tile([128, 16], F32, space="PSUM")
                nc.tensor.matmul(rbc_ps[:], ones1w[:], rc16[:], start=True, stop=True)
                yT_sb = fp.tile([128, 16], F32)
                nc.vector.tensor_tensor(yT_sb[:], e_sb[:], rbc_ps[:], op=ALU.mult)
                ytr_ps = fpsm.tile([16, 128], F32, space="PSUM")
                nc.tensor.transpose(ytr_ps[:], yT_sb[:], ident[:])
                ynat = fp.tile([16, 128], F32)
                nc.vector.tensor_copy(ynat[:], ytr_ps[:])
                for oc in range(2):
                    nc.sync.dma_start(out_e[0:8, oc * 128:(oc + 1) * 128],
                                      ynat[oc * 8:(oc + 1) * 8, :])

    nc.compile()
    return nc


def build_h(T_steps=512):
    nc = bacc.Bacc(None, target_bir_lowering=False)
    NT = T_steps * BL // 128          # gather tiles of 128 tokens
    NCH = T_steps * BL // 512         # 512-token chunks for xs matmul
    ten = _declare(nc, T_steps)
    emb_e, xp_e, sidx_e = ten['emb'], ten['xp'], ten['sidx']
    out_e, s2d = ten['out'], ten['s2d']

    with tile.TileContext(nc) as tc:
        with (
            tc.tile_pool(name="const", bufs=1) as cp,
            tc.tile_pool(name="big", bufs=1) as bigp,
        ):
            # ---- constants / weights to SBUF ----
            xp_sb = cp.tile([128, NT], I32)
            sidx_sb = cp.tile([64, 1], I32)
            wihT_sb = cp.tile([128, 2048], BF16)
            whhT_sb = cp.tile([128, 2048], BF16)
            bl_sb = cp.tile([128, 8], F32)
            wy_sb = cp.tile([128, 512], BF16)
            wtoh_sb = cp.tile([128, 128], BF16)
            sel_sb = cp.tile([8, 1024], BF16)
            wpT_sb = cp.tile([128, 512], BF16)
            wxT_sb = cp.tile([128, 512], BF16)
            wfT_sb = cp.tile([128, 512], BF16)
            bf_sb = cp.tile([128, 2], F32)
            for dst, src in [(xp_sb, ten['xp']), (sidx_sb, ten['sidx']),
                             (wihT_sb, ten['wihT']), (whhT_sb, ten['whhT']),
                             (bl_sb, ten['bl']), (wy_sb, ten['wy']),
                             (wtoh_sb, ten['wtoh']), (sel_sb, ten['sel']),
                             (wpT_sb, ten['wpT']), (wxT_sb, ten['wxT']),
                             (wfT_sb, ten['wfT']), (bf_sb, ten['bf'])]:
                nc.sync.dma_start(dst[:], src[:])
            ident = cp.tile([128, 128], F32)
            make_identity(nc, ident[:])
            ident_bf = cp.tile([128, 128], BF16)
            nc.vector.tensor_copy(ident_bf[:], ident[:])
            ones64 = cp.tile([64, 1], F32)
            nc.gpsimd.memset(ones64[:], 1.0)
            ones1w = cp.tile([1, 128], F32)
            nc.gpsimd.memset(ones1w[:], 1.0)
            ones128 = cp.tile([128, 1], F32)
            nc.gpsimd.memset(ones128[:], 1.0)

            # ---- persistent big tensors ----
            eT0 = bigp.tile([128, T_steps * 8], BF16)
            eT1 = bigp.tile([128, T_steps * 8], BF16)
            xsT = bigp.tile([128, T_steps * 64], BF16)
            hT = bigp.tile([128, (T_steps + 1) * 16], BF16)
            s2_sb = bigp.tile([8, 512], BF16)
            rT_sb = bigp.tile([128, 16], F32)
            rT_bf = bigp.tile([128, 16], BF16)
            a_sb = bigp.tile([8, T_steps], BF16)
            weff_all = bigp.tile([128, 4096], BF16)
            cgA = bigp.tile([128, 32], F32)   # cols 0:16 tanh(g_t), 16:32 c
            cgB = bigp.tile([128, 32], F32)

            xs_v = xsT[:].rearrange("p (t q) -> p t q", q=64)
            hT_v = hT[:].rearrange("p (t dj b) -> p dj b t", dj=2, b=8)
            nc.vector.memset(hT[:, 0:16], 0.0)
            nc.vector.memset(cgA[:, 16:32], 0.0)

            with (
                tc.tile_pool(name="gat", bufs=NT) as gp,
                tc.tile_pool(name="xps", bufs=1, space="PSUM") as xpsm,
                tc.tile_pool(name="rec", bufs=4) as rp,
                tc.tile_pool(name="rps", bufs=2, space="PSUM") as rpsm,
                tc.tile_pool(name="yps", bufs=2, space="PSUM") as ypsm,
                tc.tile_pool(name="ytp", bufs=3) as ytp,
                tc.tile_pool(name="scp", bufs=1, space="PSUM") as scpsm,
                tc.tile_pool(name="cwp", bufs=8) as cwp,
                tc.tile_pool(name="att", bufs=2) as ap_,
                tc.tile_pool(name="sm", bufs=1) as smp,
                tc.tile_pool(name="swork", bufs=1) as sw,
            ):
                sc_ps = scpsm.tile([8, T_steps], F32, space="PSUM")

                def emit_gather(g):
                    egath = gp.tile([128, 256], F32)
                    nc.gpsimd.indirect_dma_start(
                        out=egath[:], out_offset=None, in_=emb_e[:],
                        in_offset=IndirectOffsetOnAxis(ap=xp_sb[:, g:g + 1], axis=0))
                    return egath

                def emit_trcopy(egath, g, dc):
                    eT = eT0 if dc == 0 else eT1
                    tps = xpsm.tile([128, 128], F32, space="PSUM", tag="ps")
                    nc.tensor.transpose(tps[:], egath[:, dc * 128:(dc + 1) * 128], ident[:])
                    nc.vector.tensor_copy(eT[:, g * 128:(g + 1) * 128], tps[:])

                def emit_xs_mm(nch, gb, holder):
                    xps = xpsm.tile([128, 512], F32, space="PSUM", tag="ps")
                    nc.tensor.matmul(xps[:], wihT_sb[:, gb * 128:(gb + 1) * 128],
                                     eT0[:, nch * 512:(nch + 1) * 512],
                                     start=True, stop=False)
                    nc.tensor.matmul(xps[:], wihT_sb[:, 1024 + gb * 128:1024 + (gb + 1) * 128],
                                     eT1[:, nch * 512:(nch + 1) * 512],
                                     start=False, stop=True)
                    holder['xps'] = xps

                def emit_xs_ts_half(nch, gb, half, holder):
                    nc.vector.tensor_scalar(
                        xs_v[:, nch * 64 + half * 32:nch * 64 + (half + 1) * 32,
                             gb * 8:(gb + 1) * 8],
                        holder['xps'][:, half * 256:(half + 1) * 256],
                        bl_sb[:, gb:gb + 1], None, op0=ALU.add)

                egaths = {}

                def chunk_items(k):
                    items = []
                    for g in range(4 * k, 4 * k + 4):
                        for dc in range(2):
                            items.append(lambda g=g, dc=dc: emit_trcopy(
                                egaths.pop(g) if dc else egaths[g], g, dc))
                    for gb in range(8):
                        h = {}
                        items.append(lambda k=k, gb=gb, h=h: emit_xs_mm(k, gb, h))
                        for half in range(2):
                            items.append(lambda k=k, gb=gb, half=half, h=h:
                                         emit_xs_ts_half(k, gb, half, h))
                    return items

                # gathers: chunk-0 first, then aspect row, then the rest.
                for g in range(4):
                    egaths[g] = emit_gather(g)
                semb = sw.tile([64, 256], F32)
                nc.gpsimd.indirect_dma_start(
                    out=semb[:], out_offset=None, in_=emb_e[:],
                    in_offset=IndirectOffsetOnAxis(ap=sidx_sb[:, :1], axis=0))
                for g in range(4, NT):
                    egaths[g] = emit_gather(g)

                # chunk-0 transposes + xs up front
                for it in chunk_items(0):
                    it()

                # ---- s-branch (batchnormed aspect embedding); PSUM stats run
                # sequentially through the xpsm ring slot.
                mu_ps = xpsm.tile([64, 256], F32, space="PSUM", tag="ps")
                nc.tensor.matmul(mu_ps[0:1, :], ones64[:], semb[:], start=True, stop=True)
                mu = sw.tile([1, 256], F32)
                nc.vector.tensor_scalar_mul(mu[:], mu_ps[0:1, :], 1.0 / 64)
                sq = sw.tile([64, 256], F32)
                nc.vector.tensor_mul(sq[:], semb[:], semb[:])
                ms_ps = xpsm.tile([64, 256], F32, space="PSUM", tag="ps")
                nc.tensor.matmul(ms_ps[0:1, :], ones64[:], sq[:], start=True, stop=True)
                msq = sw.tile([1, 256], F32)
                nc.vector.tensor_scalar_mul(msq[:], ms_ps[0:1, :], 1.0 / 64)
                mu2 = sw.tile([1, 256], F32)
                nc.vector.tensor_mul(mu2[:], mu[:], mu[:])
                var = sw.tile([1, 256], F32)
                nc.vector.tensor_tensor(var[:], msq[:], mu2[:], op=ALU.subtract)
                nc.vector.tensor_scalar_add(var[:], var[:], 1e-5)
                std = sw.tile([1, 256], F32)
                nc.scalar.sqrt(std[:], var[:])
                istd = sw.tile([1, 256], F32)
                nc.vector.reciprocal(istd[:], std[:])
                mub_ps = xpsm.tile([64, 256], F32, space="PSUM", tag="ps")
                nc.tensor.matmul(mub_ps[:], ones1w[:1, :64], mu[:], start=True, stop=True)
                d8 = sw.tile([8, 256], F32)
                nc.vector.tensor_tensor(d8[:], semb[0:8, :], mub_ps[0:8, :], op=ALU.subtract)
                ib_ps = xpsm.tile([64, 256], F32, space="PSUM", tag="ps")
                nc.tensor.matmul(ib_ps[:], ones1w[:1, :64], istd[:], start=True, stop=True)
                nc.vector.tensor_tensor(s2_sb[:, 0:256], d8[:], ib_ps[0:8, :], op=ALU.mult)
                nc.vector.tensor_copy(s2_sb[:, 256:512], s2_sb[:, 0:256])
                nc.sync.dma_start(s2d[:], s2_sb[:])

                # circulant windows for all batch rows (sync DMA queue)
                cw_tiles = {}
                for b in range(8):
                    cw = cwp.tile([128, 384], BF16)
                    for j in range(3):
                        win = bass.AP(s2d[:].tensor, b * 512 + j * 128, [[1, 128], [1, 128]])
                        if H_CW_SYNC:
                            nc.sync.dma_start(cw[:, j * 128:(j + 1) * 128], win)
                        else:
                            nc.gpsimd.dma_start(cw[:, j * 128:(j + 1) * 128], win)
                    cw_tiles[b] = cw

                # --- Weff = C_b @ W_y ---
                def emit_weff_mm(b, mj, holder):
                    cw = cw_tiles[b]
                    wps = ypsm.tile([128, 256], F32, space="PSUM", tag="yps")
                    for kc in range(2):
                        nc.tensor.matmul(wps[:], cw[:, (mj + kc) * 128:(mj + kc + 1) * 128],
                                         wy_sb[:, kc * 256:(kc + 1) * 256],
                                         start=(kc == 0), stop=(kc == 1))
                    holder['wps' + str(mj)] = wps

                def emit_weff_cp(b, mj, holder):
                    nc.vector.tensor_copy(
                        weff_all[:, b * 512 + mj * 256: b * 512 + (mj + 1) * 256],
                        holder['wps' + str(mj)])

                def emit_ygroup(k, b, ec):
                    yc = ypsm.tile([128, 64], F32, space="PSUM", tag="yps")
                    for kc in range(2):
                        nc.tensor.matmul(
                            yc[:], weff_all[:, b * 512 + kc * 256 + ec * 128:
                                            b * 512 + kc * 256 + (ec + 1) * 128],
                            hT_v[:, kc, b, 1 + 64 * k: 1 + 64 * (k + 1)],
                            start=(kc == 0), stop=(kc == 1))
                    yt = ytp.tile([128, 64], BF16)
                    nc.scalar.activation(yt[:], yc[:], AF.Tanh)
                    nc.tensor.matmul(sc_ps[:, 64 * k:64 * (k + 1)],
                                     wtoh_sb[:, ec * 64 + b * 8: ec * 64 + (b + 1) * 8],
                                     yt[:], start=(b == 0 and ec == 0), stop=(b == 7 and ec == 1))

                xs_work = []
                for k in range(1, NCH):
                    xs_work.extend(chunk_items(k))
                y_work = []   # (avail_step, fn)
                for b in range(8):
                    holder = {}
                    for mj in range(2):
                        y_work.append((12, lambda b=b, mj=mj, h=holder: emit_weff_mm(b, mj, h)))
                        y_work.append((12, lambda b=b, mj=mj, h=holder: emit_weff_cp(b, mj, h)))
                for k in range(T_steps // 64):
                    for b in range(8):
                        for ec in range(2):
                            y_work.append((64 * (k + 1),
                                           lambda k=k, b=b, ec=ec: emit_ygroup(k, b, ec)))
                xi = yi = 0

                def emit_imm(tt):
                    gi = rpsm.tile([128, 48], F32, space="PSUM", tag="gpi")
                    gg_ = rpsm.tile([128, 16], F32, space="PSUM", tag="gpg")
                    nc.tensor.matmul(gi[:], ident_bf[:], xsT[:, tt * 64: tt * 64 + 48],
                                     start=True, stop=False)
                    nc.tensor.matmul(gg_[:], ident_bf[:], xsT[:, tt * 64 + 48: tt * 64 + 64],
                                     start=True, stop=False)
                    return gi, gg_

                cg = [cgA, cgB]
                for t in range(T_steps):
                    gpi, gpg = emit_imm(t)
                    # g-gates first so tanh(g) hides under sigma(i,f)
                    for gb in (6, 7, 0, 1, 2, 3, 4, 5):
                        out = (gpi[:, gb * 8:(gb + 1) * 8] if gb < 6
                               else gpg[:, (gb - 6) * 8:(gb - 5) * 8])
                        for dj in range(2):
                            nc.tensor.matmul(
                                out,
                                whhT_sb[:, dj * 1024 + gb * 128: dj * 1024 + (gb + 1) * 128],
                                hT[:, t * 16 + dj * 8: t * 16 + (dj + 1) * 8],
                                start=False, stop=(dj == 1 and gb in (5, 7)))
                    cgt = cg[t % 2]
                    cgn = cg[(t + 1) % 2]
                    # Scalar queue: tanh(g), sigma(i,f), sigma(o), [tanh(c) later]
                    nc.scalar.activation(cgt[:, 0:16], gpg[:], AF.Tanh)
                    sig_if = rp.tile([128, 32], F32)
                    nc.scalar.activation(sig_if[:], gpi[:, 0:32], AF.Sigmoid)
                    sig_o = rp.tile([128, 16], F32)
                    nc.scalar.activation(sig_o[:], gpi[:, 32:48], AF.Sigmoid)
                    # DVE: packed product, pair-sum, then h
                    m12 = rp.tile([128, 32], F32)
                    nc.vector.tensor_mul(m12[:], sig_if[:], cgt[:])
                    nc.vector.tensor_tensor(cgn[:, 16:32], m12[:, 0:16], m12[:, 16:32],
                                            op=ALU.add)
                    thc = rp.tile([128, 16], F32)
                    nc.scalar.activation(thc[:], cgn[:, 16:32], AF.Tanh)
                    nc.vector.tensor_mul(hT[:, (t + 1) * 16:(t + 2) * 16], sig_o[:], thc[:])
                    if xi < len(xs_work):
                        xs_work[xi]()
                        xi += 1
                    if yi < len(y_work) and y_work[yi][0] <= t:
                        y_work[yi][1]()
                        yi += 1
                while xi < len(xs_work):
                    xs_work[xi]()
                    xi += 1
                while yi < len(y_work):
                    y_work[yi][1]()
                    yi += 1

                # ---- softmax over T (free axis) ----
                mx = smp.tile([8, 1], F32)
                nc.vector.tensor_reduce(mx[:], sc_ps[:, 0:T_steps], axis=mybir.AxisListType.X, op=ALU.max)
                nmx = smp.tile([8, 1], F32)
                nc.vector.tensor_scalar_mul(nmx[:], mx[:], -1.0)
                esc = smp.tile([8, T_steps], F32)
                ssum = smp.tile([8, 1], F32)
                nc.scalar.activation(esc[:], sc_ps[:, 0:T_steps], AF.Exp,
                                     bias=nmx[:, 0:1], accum_out=ssum[:, 0:1])
                rcs = smp.tile([8, 1], F32)
                nc.vector.reciprocal(rcs[:], ssum[:])
                nc.scalar.activation(a_sb[:], esc[:], AF.Copy, scale=rcs[:, 0:1])
                # ---- r = sum_t a_t * h_t (fused multiply+reduce) ----
                for b in range(8):
                    abc = ypsm.tile([128, T_steps], F32, space="PSUM", tag="yps")
                    nc.tensor.matmul(abc[:, 0:T_steps], sel_sb[:, b * 128:(b + 1) * 128],
                                     a_sb[:], start=True, stop=True)
                    for dj in range(2):
                        wt_ = ap_.tile([128, T_steps], F32)
                        if H_TTR:
                            nc.vector.tensor_tensor_reduce(
                                out=wt_[:, 0:T_steps],
                                in0=hT_v[:, dj, b, 1:T_steps + 1],
                                in1=abc[:, 0:T_steps],
                                scale=1.0, scalar=0.0,
                                op0=ALU.mult, op1=ALU.add,
                                accum_out=rT_sb[:, dj * 8 + b: dj * 8 + b + 1])
                        else:
                            nc.vector.tensor_tensor(wt_[:, 0:T_steps], hT_v[:, dj, b, 1:T_steps + 1],
                                                    abc[:, 0:T_steps], op=ALU.mult)
                            nc.vector.tensor_reduce(rT_sb[:, dj * 8 + b: dj * 8 + b + 1],
                                                    wt_[:, 0:T_steps], axis=mybir.AxisListType.X,
                                                    op=ALU.add)
                nc.vector.tensor_copy(rT_bf[:], rT_sb[:])

            # ================= final MLP + softmax ==========================
            with (
                tc.tile_pool(name="fin", bufs=1) as fp,
                tc.tile_pool(name="fps", bufs=1, space="PSUM") as fpsm,
            ):
                rr_ps = fpsm.tile([128, 16], F32, space="PSUM")
                hlast = hT[:, T_steps * 16:(T_steps + 1) * 16]
                for oc in range(2):
                    for kc in range(2):
                        nc.tensor.matmul(rr_ps[:, oc * 8:(oc + 1) * 8],
                                         wpT_sb[:, kc * 256 + oc * 128: kc * 256 + (oc + 1) * 128],
                                         rT_bf[:, kc * 8:(kc + 1) * 8],
                                         start=(kc == 0), stop=False)
                    for kc in range(2):
                        nc.tensor.matmul(rr_ps[:, oc * 8:(oc + 1) * 8],
                                         wxT_sb[:, kc * 256 + oc * 128: kc * 256 + (oc + 1) * 128],
                                         hlast[:, kc * 8:(kc + 1) * 8],
                                         start=False, stop=(kc == 1))
                rrT = fp.tile([128, 16], BF16)
                nc.scalar.activation(rrT[:], rr_ps[:], AF.Tanh)
                z_ps = fpsm.tile([128, 16], F32, space="PSUM")
                for oc in range(2):
                    for kc in range(2):
                        nc.tensor.matmul(z_ps[:, oc * 8:(oc + 1) * 8],
                                         wfT_sb[:, kc * 256 + oc * 128: kc * 256 + (oc + 1) * 128],
                                         rrT[:, kc * 8:(kc + 1) * 8],
                                         start=(kc == 0), stop=(kc == 1))
                e_sb = fp.tile([128, 16], F32)
                for oc in range(2):
                    nc.scalar.activation(e_sb[:, oc * 8:(oc + 1) * 8], z_ps[:, oc * 8:(oc + 1) * 8],
                                         AF.Exp, bias=bf_sb[:, oc:oc + 1])
                cs_ps = fpsm.tile([1, 16], F32, space="PSUM")
                nc.tensor.matmul(cs_ps[:], ones128[:], e_sb[:], start=True, stop=True)
                cs_sb = fp.tile([1, 16], F32)
                nc.vector.tensor_copy(cs_sb[:], cs_ps[:])
                s8 = fp.tile([1, 8], F32)
                nc.vector.tensor_tensor(s8[:], cs_sb[0:1, 0:8], cs_sb[0:1, 8:16], op=ALU.add)
                rc8 = fp.tile([1, 8], F32)
                nc.vector.reciprocal(rc8[:], s8[:])
                rc16 = fp.tile([1, 16], F32)
                nc.vector.tensor_copy(rc16[:, 0:8], rc8[:])
                nc.vector.tensor_copy(rc16[:, 8:16], rc8[:])
                rbc_ps = fpsm.tile([128, 16], F32, space="PSUM")
                nc.tensor.matmul(rbc_ps[:], ones1w[:], rc16[:], start=True, stop=True)
                yT_sb = fp.tile([128, 16], F32)
                nc.vector.tensor_tensor(yT_sb[:], e_sb[:], rbc_ps[:], op=ALU.mult)
                ytr_ps = fpsm.tile([16, 128], F32, space="PSUM")
                nc.tensor.transpose(ytr_ps[:], yT_sb[:], ident[:])
                ynat = fp.tile([16, 128], F32)
                nc.vector.tensor_copy(ynat[:], ytr_ps[:])
                for oc in range(2):
                    nc.sync.dma_start(out_e[0:8, oc * 128:(oc + 1) * 128],
                                      ynat[oc * 8:(oc + 1) * 8, :])

    nc.compile()
    return nc


def build(T_steps=512, variant=None):
    variant = VARIANT if variant is None else variant
    if variant == "i":
        return build_i(T_steps)
    if variant == "h":
        return build_h(T_steps)
    nc = bacc.Bacc(None, target_bir_lowering=False)
    NT = T_steps * BL // 128          # gather tiles of 128 tokens
    NCH = T_steps * BL // 512         # 512-token chunks for xs matmul

    ten = _declare(nc, T_steps)
    emb_e, xp_e, sidx_e = ten['emb'], ten['xp'], ten['sidx']
    wihT_e, whhT_e, whh8_e, bl_e = ten['wihT'], ten['whhT'], ten['whh8'], ten['bl']
    wy_e, wtoh_e, sel_e = ten['wy'], ten['wtoh'], ten['sel']
    wpT_e, wxT_e, wfT_e, bf_e = ten['wpT'], ten['wxT'], ten['wfT'], ten['bf']
    out_e, s2d = ten['out'], ten['s2d']

    with tile.TileContext(nc) as tc:
        with (
            tc.tile_pool(name="const", bufs=1) as cp,
            tc.tile_pool(name="big", bufs=1) as bigp,
        ):
            # ---- constants / weights to SBUF ----
            xp_sb = cp.tile([128, NT], I32)
            sidx_sb = cp.tile([64, 1], I32)
            wihT_sb = cp.tile([128, 2048], BF16)
            whhT_sb = cp.tile([128, 2048], BF16)
            whh8_sb = cp.tile([128, 2048], mybir.dt.float8e4)
            bl_sb = cp.tile([128, 8], F32)
            wy_sb = cp.tile([128, 512], BF16)
            wtoh_sb = cp.tile([128, 128], BF16)
            sel_sb = cp.tile([8, 1024], BF16)
            wpT_sb = cp.tile([128, 512], BF16)
            wxT_sb = cp.tile([128, 512], BF16)
            wfT_sb = cp.tile([128, 512], BF16)
            bf_sb = cp.tile([128, 2], F32)
            for dst, src in [(xp_sb, xp_e), (sidx_sb, sidx_e), (wihT_sb, wihT_e),
                             (whhT_sb, whhT_e), (whh8_sb, whh8_e), (bl_sb, bl_e), (wy_sb, wy_e),
                             (wtoh_sb, wtoh_e), (sel_sb, sel_e), (wpT_sb, wpT_e),
                             (wxT_sb, wxT_e), (wfT_sb, wfT_e), (bf_sb, bf_e)]:
                nc.sync.dma_start(dst[:], src[:])
            ident = cp.tile([128, 128], F32)
            make_identity(nc, ident[:])
            ident_bf = cp.tile([128, 128], BF16)
            nc.vector.tensor_copy(ident_bf[:], ident[:])
            ones64 = cp.tile([64, 1], F32)
            nc.gpsimd.memset(ones64[:], 1.0)
            ones1w = cp.tile([1, 128], F32)
            nc.gpsimd.memset(ones1w[:], 1.0)
            ones128 = cp.tile([128, 1], F32)
            nc.gpsimd.memset(ones128[:], 1.0)

            # ---- persistent big tensors ----
            eT0 = bigp.tile([128, T_steps * 8], BF16)
            eT1 = bigp.tile([128, T_steps * 8], BF16)
            xsT = bigp.tile([128, T_steps * 64], BF16)
            hT = bigp.tile([128, (T_steps + 1) * 16], BF16)
            s2_sb = bigp.tile([8, 512], BF16)
            rT_sb = bigp.tile([128, 16], F32)
            rT_bf = bigp.tile([128, 16], BF16)
            a_sb = bigp.tile([8, T_steps], BF16)

            # ================= s-branch (batchnormed aspect embedding) ======
            with (
                tc.tile_pool(name="swork", bufs=1) as sw,
                tc.tile_pool(name="spsum", bufs=1, space="PSUM") as sps,
            ):
                semb = sw.tile([64, 256], F32)
                nc.gpsimd.indirect_dma_start(
                    out=semb[:], out_offset=None, in_=emb_e[:],
                    in_offset=IndirectOffsetOnAxis(ap=sidx_sb[:, :1], axis=0))
                mu_ps = sps.tile([1, 256], F32, space="PSUM")
                nc.tensor.matmul(mu_ps[:], ones64[:], semb[:], start=True, stop=True)
                mu = sw.tile([1, 256], F32)
                nc.vector.tensor_scalar_mul(mu[:], mu_ps[:], 1.0 / 64)
                sq = sw.tile([64, 256], F32)
                nc.vector.tensor_mul(sq[:], semb[:], semb[:])
                ms_ps = sps.tile([1, 256], F32, space="PSUM")
                nc.tensor.matmul(ms_ps[:], ones64[:], sq[:], start=True, stop=True)
                msq = sw.tile([1, 256], F32)
                nc.vector.tensor_scalar_mul(msq[:], ms_ps[:], 1.0 / 64)
                mu2 = sw.tile([1, 256], F32)
                nc.vector.tensor_mul(mu2[:], mu[:], mu[:])
                var = sw.tile([1, 256], F32)
                nc.vector.tensor_tensor(var[:], msq[:], mu2[:], op=ALU.subtract)
                nc.vector.tensor_scalar_add(var[:], var[:], 1e-5)
                std = sw.tile([1, 256], F32)
                nc.scalar.sqrt(std[:], var[:])
                istd = sw.tile([1, 256], F32)
                nc.vector.reciprocal(istd[:], std[:])
                mub_ps = sps.tile([64, 256], F32, space="PSUM")
                nc.tensor.matmul(mub_ps[:], ones1w[:1, :64], mu[:], start=True, stop=True)
                ib_ps = sps.tile([64, 256], F32, space="PSUM")
                nc.tensor.matmul(ib_ps[:], ones1w[:1, :64], istd[:], start=True, stop=True)
                d8 = sw.tile([8, 256], F32)
                nc.vector.tensor_tensor(d8[:], semb[0:8, :], mub_ps[0:8, :], op=ALU.subtract)
                nc.vector.tensor_tensor(s2_sb[:, 0:256], d8[:], ib_ps[0:8, :], op=ALU.mult)
                nc.vector.tensor_copy(s2_sb[:, 256:512], s2_sb[:, 0:256])
                nc.gpsimd.dma_start(s2d[:], s2_sb[:])

            # ========= gather/xs/Weff/Y/score interleaved with recurrence ===
            xs_v = xsT[:].rearrange("p (t q) -> p t q", q=64)
            hT_v = hT[:].rearrange("p (t dj b) -> p dj b t", dj=2, b=8)
            weff_all = bigp.tile([128, 4096], BF16)
            nc.gpsimd.memset(hT[:, 0:16], 0.0)
            with (
                tc.tile_pool(name="gat", bufs=8) as gp,
                tc.tile_pool(name="xps", bufs=1, space="PSUM") as xpsm,
                tc.tile_pool(name="rec", bufs=3) as rp,
                tc.tile_pool(name="cst", bufs=3) as cpp,
                tc.tile_pool(name="rps", bufs=2, space="PSUM") as rpsm,
                tc.tile_pool(name="yps", bufs=2, space="PSUM") as ypsm,
                tc.tile_pool(name="ytp", bufs=3) as ytp,
                tc.tile_pool(name="scp", bufs=1, space="PSUM") as scpsm,
                tc.tile_pool(name="att", bufs=2) as ap_,
                tc.tile_pool(name="sm", bufs=1) as smp,
            ):
                sc_ps = scpsm.tile([8, T_steps], F32, space="PSUM")

                def emit_gather(g):
                    egath = gp.tile([128, 256], F32)
                    nc.gpsimd.indirect_dma_start(
                        out=egath[:], out_offset=None, in_=emb_e[:],
                        in_offset=IndirectOffsetOnAxis(ap=xp_sb[:, g:g + 1], axis=0))
                    return egath

                def emit_trcopy(egath, g, dc):
                    eT = eT0 if dc == 0 else eT1
                    tps = xpsm.tile([128, 128], F32, space="PSUM", tag="ps")
                    nc.tensor.transpose(tps[:], egath[:, dc * 128:(dc + 1) * 128], ident[:])
                    nc.vector.tensor_copy(eT[:, g * 128:(g + 1) * 128], tps[:])

                def emit_xs_mm(nch, gb, holder):
                    xps = xpsm.tile([128, 512], F32, space="PSUM", tag="ps")
                    nc.tensor.matmul(xps[:], wihT_sb[:, gb * 128:(gb + 1) * 128],
                                     eT0[:, nch * 512:(nch + 1) * 512],
                                     start=True, stop=False)
                    nc.tensor.matmul(xps[:], wihT_sb[:, 1024 + gb * 128:1024 + (gb + 1) * 128],
                                     eT1[:, nch * 512:(nch + 1) * 512],
                                     start=False, stop=True)
                    holder['xps'] = xps

                def emit_xs_ts(nch, gb, holder):
                    nc.vector.tensor_scalar(
                        xs_v[:, nch * 64:(nch + 1) * 64, gb * 8:(gb + 1) * 8],
                        holder['xps'], bl_sb[:, gb:gb + 1], None, op0=ALU.add)

                def emit_xs(nch, gb):
                    h = {}
                    emit_xs_mm(nch, gb, h)
                    emit_xs_ts(nch, gb, h)

                egaths = {}

                def chunk_items(k):
                    items = []
                    for g in range(4 * k, 4 * k + 4):
                        if variant == "g":
                            for dc in range(2):
                                items.append(lambda g=g, dc=dc: emit_trcopy(egaths.pop(g) if dc else egaths[g], g, dc))
                        else:
                            holder = {}
                            items.append(lambda g=g, h=holder: h.__setitem__('e', emit_gather(g)))
                            for dc in range(2):
                                items.append(lambda g=g, dc=dc, h=holder: emit_trcopy(h['e'], g, dc))
                    for gb in range(8):
                        h = {}
                        items.append(lambda k=k, gb=gb, h=h: emit_xs_mm(k, gb, h))
                        items.append(lambda k=k, gb=gb, h=h: emit_xs_ts(k, gb, h))
                    return items

                if variant == "g":
                    for g in range(NT):
                        egaths[g] = emit_gather(g)

                # --- Weff = C_b @ W_y (only needs the s-branch) ---
                def emit_cw(b, holder):
                    cw = ap_.tile([128, 384], BF16)
                    for j in range(3):
                        win = bass.AP(s2d[:].tensor, b * 512 + j * 128, [[1, 128], [1, 128]])
                        nc.gpsimd.dma_start(cw[:, j * 128:(j + 1) * 128], win)
                    holder['cw'] = cw

                def emit_weff_mm(b, mj, holder):
                    cw = holder['cw']
                    wps = ypsm.tile([128, 256], F32, space="PSUM", tag="yps")
                    for kc in range(2):
                        nc.tensor.matmul(wps[:], cw[:, (mj + kc) * 128:(mj + kc + 1) * 128],
                                         wy_sb[:, kc * 256:(kc + 1) * 256],
                                         start=(kc == 0), stop=(kc == 1))
                    holder['wps' + str(mj)] = wps

                def emit_weff_cp(b, mj, holder):
                    nc.vector.tensor_copy(weff_all[:, b * 512 + mj * 256: b * 512 + (mj + 1) * 256],
                                          holder['wps' + str(mj)])

                def emit_ygroup(k, b, ec):
                    yc = ypsm.tile([128, 64], F32, space="PSUM", tag="yps")
                    for kc in range(2):
                        nc.tensor.matmul(
                            yc[:], weff_all[:, b * 512 + kc * 256 + ec * 128: b * 512 + kc * 256 + (ec + 1) * 128],
                            hT_v[:, kc, b, 1 + 64 * k: 1 + 64 * (k + 1)],
                            start=(kc == 0), stop=(kc == 1))
                    yt = ytp.tile([128, 64], BF16)
                    nc.scalar.activation(yt[:], yc[:], AF.Tanh)
                    nc.tensor.matmul(sc_ps[:, 64 * k:64 * (k + 1)],
                                     wtoh_sb[:, ec * 64 + b * 8: ec * 64 + (b + 1) * 8],
                                     yt[:], start=(b == 0 and ec == 0), stop=(b == 7 and ec == 1))

                # chunk 0 of gather/xs up front; everything else trickles in
                for it in chunk_items(0):
                    it()
                xs_work = []
                for k in range(1, NCH):
                    xs_work.extend(chunk_items(k))
                y_work = []   # (avail_step, fn)
                for b in range(8):
                    holder = {}
                    y_work.append((0, lambda b=b, h=holder: emit_cw(b, h)))
                    for mj in range(2):
                        y_work.append((0, lambda b=b, mj=mj, h=holder: emit_weff_mm(b, mj, h)))
                        y_work.append((0, lambda b=b, mj=mj, h=holder: emit_weff_cp(b, mj, h)))
                for k in range(T_steps // 64):
                    for b in range(8):
                        for ec in range(2):
                            y_work.append((64 * (k + 1),
                                           lambda k=k, b=b, ec=ec: emit_ygroup(k, b, ec)))
                xi = yi = 0

                c_prev = cpp.tile([128, 16], F32)
                nc.vector.memset(c_prev[:], 0.0)
                def emit_imm(tt):
                    gi = rpsm.tile([128, 48], F32, space="PSUM", tag="gpi")
                    gg_ = rpsm.tile([128, 16], F32, space="PSUM", tag="gpg")
                    nc.tensor.matmul(gi[:], ident_bf[:], xsT[:, tt * 64: tt * 64 + 48],
                                     start=True, stop=False)
                    nc.tensor.matmul(gg_[:], ident_bf[:], xsT[:, tt * 64 + 48: tt * 64 + 64],
                                     start=True, stop=False)
                    return gi, gg_

                if variant == "d":
                    gpi, gpg = emit_imm(0)
                for t in range(T_steps):
                    if variant != "d":
                        gpi, gpg = emit_imm(t)
                    gb_order = (6, 7, 0, 1, 2, 3, 4, 5) if variant == "d" else (0, 1, 2, 3, 4, 5, 6, 7)
                    w_sb = whh8_sb if variant == "f" else whhT_sb
                    for gb in gb_order:
                        out = gpi[:, gb * 8:(gb + 1) * 8] if gb < 6 else gpg[:, (gb - 6) * 8:(gb - 5) * 8]
                        for dj in range(2):
                            nc.tensor.matmul(
                                out,
                                w_sb[:, dj * 1024 + gb * 128: dj * 1024 + (gb + 1) * 128],
                                hT[:, t * 16 + dj * 8: t * 16 + (dj + 1) * 8],
                                start=False, stop=(dj == 1 and gb in (5, 7)))
                    gpi_t, gpg_t = gpi, gpg
                    if variant == "d" and t + 1 < T_steps:
                        gpi, gpg = emit_imm(t + 1)
                    if variant == "d":
                        gg = rp.tile([128, 16], F32)
                        nc.scalar.activation(gg[:], gpg_t[:], AF.Tanh)
                        sig = rp.tile([128, 48], F32)
                        nc.scalar.activation(sig[:], gpi_t[:], AF.Sigmoid)
                    else:
                        sig = rp.tile([128, 48], F32)
                        nc.scalar.activation(sig[:], gpi_t[:], AF.Sigmoid)
                        gg = rp.tile([128, 16], F32)
                        nc.scalar.activation(gg[:], gpg_t[:], AF.Tanh)
                    m1 = rp.tile([128, 16], F32)
                    nc.vector.tensor_mul(m1[:], sig[:, 16:32], c_prev[:])
                    m2 = rp.tile([128, 16], F32)
                    nc.vector.tensor_mul(m2[:], sig[:, 0:16], gg[:])
                    c_new = cpp.tile([128, 16], F32)
                    nc.vector.tensor_tensor(c_new[:], m1[:], m2[:], op=ALU.add)
                    thc = rp.tile([128, 16], F32)
                    nc.scalar.activation(thc[:], c_new[:], AF.Tanh)
                    nc.vector.tensor_mul(hT[:, (t + 1) * 16:(t + 2) * 16], sig[:, 32:48], thc[:])
                    c_prev = c_new
                    if xi < len(xs_work):
                        xs_work[xi]()
                        xi += 1
                    if yi < len(y_work) and y_work[yi][0] <= t and (t >= 96 or t % 2 == 0):
                        y_work[yi][1]()
                        yi += 1
                while xi < len(xs_work):
                    xs_work[xi]()
                    xi += 1
                while yi < len(y_work):
                    y_work[yi][1]()
                    yi += 1

                # ---- softmax over T (free axis) ----
                mx = smp.tile([8, 1], F32)
                nc.vector.tensor_reduce(mx[:], sc_ps[:, 0:T_steps], axis=mybir.AxisListType.X, op=ALU.max)
                nmx = smp.tile([8, 1], F32)
                nc.vector.tensor_scalar_mul(nmx[:], mx[:], -1.0)
                esc = smp.tile([8, T_steps], F32)
                ssum = smp.tile([8, 1], F32)
                nc.scalar.activation(esc[:], sc_ps[:, 0:T_steps], AF.Exp,
                                     bias=nmx[:, 0:1], accum_out=ssum[:, 0:1])
                rcs = smp.tile([8, 1], F32)
                nc.vector.reciprocal(rcs[:], ssum[:])
                nc.scalar.activation(a_sb[:], esc[:], AF.Copy, scale=rcs[:, 0:1])
                # ---- r = sum_t a_t * h_t ----
                for b in range(8):
                    abc = ypsm.tile([128, T_steps], F32, space="PSUM", tag="yps")
                    nc.tensor.matmul(abc[:, 0:T_steps], sel_sb[:, b * 128:(b + 1) * 128],
                                     a_sb[:], start=True, stop=True)
                    for dj in range(2):
                        wt_ = ap_.tile([128, T_steps], F32)
                        nc.vector.tensor_tensor(wt_[:, 0:T_steps], hT_v[:, dj, b, 1:T_steps + 1],
                                                abc[:, 0:T_steps], op=ALU.mult)
                        nc.vector.tensor_reduce(rT_sb[:, dj * 8 + b: dj * 8 + b + 1],
                                                wt_[:, 0:T_steps], axis=mybir.AxisListType.X,
                                                op=ALU.add)
                nc.vector.tensor_copy(rT_bf[:], rT_sb[:])

            # ================= final MLP + softmax ==========================
            with (
                tc.tile_pool(name="fin", bufs=1) as fp,
                tc.tile_pool(name="fps", bufs=1, space="PSUM") as fpsm,
            ):
                rr_ps = fpsm.tile([128, 16], F32, space="PSUM")
                hlast = hT[:, T_steps * 16:(T_steps + 1) * 16]
                for oc in range(2):
                    for kc in range(2):
                        nc.tensor.matmul(rr_ps[:, oc * 8:(oc + 1) * 8],
                                         wpT_sb[:, kc * 256 + oc * 128: kc * 256 + (oc + 1) * 128],
                                         rT_bf[:, kc * 8:(kc + 1) * 8],
                                         start=(kc == 0), stop=False)
                    for kc in range(2):
                        nc.tensor.matmul(rr_ps[:, oc * 8:(oc + 1) * 8],
                                         wxT_sb[:, kc * 256 + oc * 128: kc * 256 + (oc + 1) * 128],
                                         hlast[:, kc * 8:(kc + 1) * 8],
                                         start=False, stop=(kc == 1))
                rrT = fp.tile([128, 16], BF16)
                nc.scalar.activation(rrT[:], rr_ps[:], AF.Tanh)
                z_ps = fpsm.tile([128, 16], F32, space="PSUM")
                for oc in range(2):
                    for kc in range(2):
                        nc.tensor.matmul(z_ps[:, oc * 8:(oc + 1) * 8],
                                         wfT_sb[:, kc * 256 + oc * 128: kc * 256 + (oc + 1) * 128],
                                         rrT[:, kc * 8:(kc + 1) * 8],
                                         start=(kc == 0), stop=(kc == 1))
                e_sb = fp.tile([128, 16], F32)
                for oc in range(2):
                    nc.scalar.activation(e_sb[:, oc * 8:(oc + 1) * 8], z_ps[:, oc * 8:(oc + 1) * 8],
                                         AF.Exp, bias=bf_sb[:, oc:oc + 1])
                cs_ps = fpsm.tile([1, 16], F32, space="PSUM")
                nc.tensor.matmul(cs_ps[:], ones128[:], e_sb[:], start=True, stop=True)
                cs_sb = fp.tile([1, 16], F32)
                nc.vector.tensor_copy(cs_sb[:], cs_ps[:])
                s8 = fp.tile([1, 8], F32)
                nc.vector.tensor_tensor(s8[:], cs_sb[0:1, 0:8], cs_sb[0:1, 8:16], op=ALU.add)
                rc8 = fp.tile([1, 8], F32)
                nc.vector.reciprocal(rc8[:], s8[:])
                rc16 = fp.tile([1, 16], F32)
                nc.vector.tensor_copy(rc16[:, 0:8], rc8[:])
                nc.vector.tensor_copy(rc16[:, 8:16], rc8[:])
                rbc_ps = fpsm.tile([128, 16], F32, space="PSUM")
                nc.tensor.matmul(rbc_ps[:], ones1w[:], rc16[:], start=True, stop=True)
                yT_sb = fp.tile([128, 16], F32)
                nc.vector.tensor_tensor(yT_sb[:], e_sb[:], rbc_ps[:], op=ALU.mult)
                ytr_ps = fpsm.tile([16, 128], F32, space="PSUM")
                nc.tensor.transpose(ytr_ps[:], yT_sb[:], ident[:])
                ynat = fp.tile([16, 128], F32)
                nc.vector.tensor_copy(ynat[:], ytr_ps[:])
                for oc in range(2):
                    nc.sync.dma_start(out_e[0:8, oc * 128:(oc + 1) * 128],
                                      ynat[oc * 8:(oc + 1) * 8, :])

    nc.compile()
    return nc


_CACHE = {}


def _get_nc(T_steps=512, variant=None):
    key = (T_steps, VARIANT if variant is None else variant)
    if key not in _CACHE:
        _CACHE[key] = build(T_steps, variant=key[1])
    return _CACHE[key]


def make_in_maps(x, s, emb, w_ih, w_hh, b_lstm, W_y, w_t, W_p, W_x, W_f, b_f,
                 T_steps=512, variant=None):
    variant = VARIANT if variant is None else variant
    x = np.asarray(x).astype(np.int32)[:, :T_steps]
    s = np.asarray(s).astype(np.int32).reshape(64)
    emb = np.ascontiguousarray(np.asarray(emb, dtype=np.float32))
    wih_p = np.asarray(w_ih, dtype=np.float32)[_PERM]
    whh_p = np.asarray(w_hh, dtype=np.float32)[_PERM]
    bl_p = np.asarray(b_lstm, dtype=np.float32)[_PERM]

    def wt2sb(wT):  # [256, 1024] -> [128, 2048]
        return np.concatenate([wT[0:128], wT[128:256]], axis=1)

    wihT = wt2sb(wih_p.T).astype(bf16)
    whhT = wt2sb(whh_p.T).astype(bf16)
    bl_sb = bl_p.reshape(8, 128).T.copy().astype(np.float32)  # [128, 8]
    wy_sb = np.concatenate([np.asarray(W_y, np.float32)[0:128],
                            np.asarray(W_y, np.float32)[128:256]], axis=1).astype(bf16)
    w_t = np.asarray(w_t, np.float32)
    wtoh = np.zeros((128, 128), np.float32)
    for ec in range(2):
        for b in range(8):
            wtoh[:, ec * 64 + b * 8 + b] = w_t[ec * 128:(ec + 1) * 128]
    wtoh = wtoh.astype(bf16)
    sel = np.zeros((8, 1024), np.float32)
    for b in range(8):
        sel[b, b * 128:(b + 1) * 128] = 1.0
    sel = sel.astype(bf16)

    def t2sb(w):  # W [do, din] -> lhsT layout [128, 512] free=kc*256+do
        wT = np.asarray(w, np.float32).T  # [din, do]
        return np.concatenate([wT[0:128], wT[128:256]], axis=1).astype(bf16)

    wpT = t2sb(W_p)
    wxT = t2sb(W_x)
    wfT = t2sb(W_f)
    bf_sb = np.asarray(b_f, np.float32).reshape(2, 128).T.copy()

    if variant == "i":
        # host-side gather + transpose + batchnorm
        semb = emb[s]                                      # [64, 256]
        mu = semb.mean(0, keepdims=True)
        var = semb.var(0, keepdims=True)
        sn = (semb - mu) / np.sqrt(var + 1e-5)             # [64, 256]
        common = dict(wihT=wihT, whhT=whhT, bl=bl_sb, wy=wy_sb, wtoh=wtoh,
                      sel=sel, wpT=wpT, wxT=wxT, wfT=wfT, bf=bf_sb)
        in_maps = []
        for c in range(NCORES):
            xs_c = x[c * BL:(c + 1) * BL]                  # [8, T]
            ex = emb[xs_c]                                 # [8, T, 256] f32
            exT = ex.transpose(2, 1, 0)                    # [256, T, 8]
            et0 = np.ascontiguousarray(exT[0:128]).reshape(128, T_steps * 8).astype(bf16)
            et1 = np.ascontiguousarray(exT[128:256]).reshape(128, T_steps * 8).astype(bf16)
            rows = sn[c * BL:(c + 1) * BL]                 # [8, 256]
            s2 = np.concatenate([rows, rows], axis=1).astype(bf16)  # [8, 512]
            in_maps.append(dict(et0=et0, et1=et1, s2=s2, **common))
        return in_maps

    whh8 = wt2sb(whh_p.T).astype(ml_dtypes.float8_e4m3)
    common = dict(emb=emb, wihT=wihT, whhT=whhT, whh8=whh8, bl=bl_sb, wy=wy_sb, wtoh=wtoh,
                  sel=sel, wpT=wpT, wxT=wxT, wfT=wfT, bf=bf_sb)
    in_maps = []
    for c in range(NCORES):
        xs_c = x[c * BL:(c + 1) * BL]                      # [8, T]
        xflat = xs_c.T.reshape(-1)                         # t-major tokens
        xp = xflat.reshape(-1, 128).T.copy()               # [128, NT]
        sidx = np.roll(s, -BL * c).reshape(64, 1).copy()
        in_maps.append(dict(xp=xp, sidx=sidx, **common))
    return in_maps


def _install_trace_shim():
    """The agent image lacks antenv.axon_hooks; recreate it and install the
    ctypes NTFF hook from trn_boot so run_bass_kernel_spmd(trace=True) works."""
    import sys, types
    if "antenv.axon_hooks" not in sys.modules:
        mod = types.ModuleType("antenv.axon_hooks")
        mod._hook = None
        mod.set_axon_ntff_profile_hook = lambda h: setattr(mod, "_hook", h)
        mod.get_axon_ntff_profile_hook = lambda: mod._hook
        sys.modules["antenv.axon_hooks"] = mod
        import antenv
        antenv.axon_hooks = mod
    import antenv.axon_hooks as ah
    if ah.get_axon_ntff_profile_hook() is None:
        from trn_agent_boot.trn_boot import _ntff_profile_via_ctypes
        ah.set_axon_ntff_profile_hook(_ntff_profile_via_ctypes("/opt/axon/libaxon_pjrt.so"))
    import concourse.bass_utils as bu
    bu.upload_artifacts = lambda tmpdir: ""


def run(in_maps, T_steps=512, trace=False, tmpdir=None, variant=None):
    nc = _get_nc(T_steps, variant)
    if trace:
        _install_trace_shim()
    return run_bass_kernel_spmd(nc, in_maps, core_ids=list(range(NCORES)),
                                trace=trace, tmpdir=tmpdir)


def kernel(x, s, emb, w_ih, w_hh, b_lstm, W_y, w_t, W_p, W_x, W_f, b_f):
    in_maps = make_in_maps(x, s, emb, w_ih, w_hh, b_lstm, W_y, w_t, W_p, W_x,
                           W_f, b_f)
    res = run(in_maps)
    return np.concatenate([res.results[i]["out"] for i in range(NCORES)], axis=0)


# revision 26
# speedup vs baseline: 1.1303x; 1.0156x over previous
"""AF-LSTM Trainium2 kernel: 8-way batch-parallel, no collectives.

Per core (8 batch rows): gather embeddings, LSTM recurrence in transposed
(gate-major) layout, circular-correlation attention via per-batch circulant
matmul folded into W_y, final MLP + softmax. Host concatenates per-core
[8,256] outputs.

Variant "h": latency-tuned recurrence chain — g-gates matmul first so
tanh(g) hides under sigma(i,f); packed [sig_i|sig_f] * [tanh_g|c] product;
sigma(o) off the critical path; all embedding gathers issued up-front so the
PE queue never head-of-line blocks on gather DMAs; s-branch and circulant
window DMAs moved off the GpSimd/PE critical queues; fused
tensor_tensor_reduce for the attention readout.
"""

import numpy as np
import ml_dtypes

import concourse.bacc as bacc
import concourse.tile as tile
from concourse import bass, mybir
from concourse.bass import IndirectOffsetOnAxis
from concourse.bass_utils import run_bass_kernel_spmd
from concourse.masks import make_identity

F32 = mybir.dt.float32
BF16 = mybir.dt.bfloat16
I32 = mybir.dt.int32
AF = mybir.ActivationFunctionType
ALU = mybir.AluOpType

V, D, B = 50000, 256, 64
NCORES, BL = 8, 8
G4 = 4 * D
bf16 = ml_dtypes.bfloat16

# gate blocks of 128 rows reordered to [i0,i1,f0,f1,o0,o1,g0,g1]
_PERM = np.concatenate([
    np.arange(0, 256),        # i
    np.arange(256, 512),      # f
    np.arange(768, 1024),     # o
    np.arange(512, 768),      # g
])


VARIANT = "i"

import os as _os
H_CW_SYNC = _os.environ.get("H_CW_SYNC", "1") == "1"
H_TTR = _os.environ.get("H_TTR", "0") == "1"
H_STT = _os.environ.get("H_STT", "1") == "1"


def _declare(nc, T_steps):
    NT = T_steps * BL // 128
    ten = {}
    ten['emb'] = nc.declare_dram_parameter("emb", [V, D], F32, isOutput=False)
    ten['xp'] = nc.declare_dram_parameter("xp", [128, NT], I32, isOutput=False)
    ten['sidx'] = nc.declare_dram_parameter("sidx", [64, 1], I32, isOutput=False)
    ten['wihT'] = nc.declare_dram_parameter("wihT", [128, 2048], BF16, isOutput=False)
    ten['whhT'] = nc.declare_dram_parameter("whhT", [128, 2048], BF16, isOutput=False)
    ten['whh8'] = nc.declare_dram_parameter("whh8", [128, 2048], mybir.dt.float8e4, isOutput=False)
    ten['bl'] = nc.declare_dram_parameter("bl", [128, 8], F32, isOutput=False)
    ten['wy'] = nc.declare_dram_parameter("wy", [128, 512], BF16, isOutput=False)
    ten['wtoh'] = nc.declare_dram_parameter("wtoh", [128, 128], BF16, isOutput=False)
    ten['sel'] = nc.declare_dram_parameter("sel", [8, 1024], BF16, isOutput=False)
    ten['wpT'] = nc.declare_dram_parameter("wpT", [128, 512], BF16, isOutput=False)
    ten['wxT'] = nc.declare_dram_parameter("wxT", [128, 512], BF16, isOutput=False)
    ten['wfT'] = nc.declare_dram_parameter("wfT", [128, 512], BF16, isOutput=False)
    ten['bf'] = nc.declare_dram_parameter("bf", [128, 2], F32, isOutput=False)
    ten['out'] = nc.declare_dram_parameter("out", [8, 256], F32, isOutput=True)
    ten['s2d'] = nc.dram_tensor("s2d", [8, 512], BF16)
    return ten


def build_i(T_steps=512, fp8=False):
    """Host-gathered variant: embeddings arrive pre-gathered/transposed in
    et0/et1, the normalized aspect rows in s2. No indirect DMA, no on-device
    batchnorm, no PE transposes. fp8=True runs the W_hh recurrence matmuls in
    fp8e4m3 DoubleRow mode (K=256 per pass, 8 gate matmuls per step)."""
    nc = bacc.Bacc(None, target_bir_lowering=False)
    NCH = T_steps * BL // 512         # 512-token chunks for xs matmul
    KW = 32                           # score-block width in steps
    et0_e = nc.declare_dram_parameter("et0", [128, T_steps * 8], BF16, isOutput=False)
    et1_e = nc.declare_dram_parameter("et1", [128, T_steps * 8], BF16, isOutput=False)
    s2_e = nc.declare_dram_parameter("s2", [8, 512], BF16, isOutput=False)
    if fp8:
        whh8_e = nc.declare_dram_parameter("whh8", [128, 2048], mybir.dt.float8e4, isOutput=False)
    wihT_e = nc.declare_dram_parameter("wihT", [128, 2048], BF16, isOutput=False)
    whhT_e = nc.declare_dram_parameter("whhT", [128, 2048], BF16, isOutput=False)
    bl_e = nc.declare_dram_parameter("bl", [128, 8], F32, isOutput=False)
    wy_e = nc.declare_dram_parameter("wy", [128, 512], BF16, isOutput=False)
    wtoh_e = nc.declare_dram_parameter("wtoh", [128, 128], BF16, isOutput=False)
    sel_e = nc.declare_dram_parameter("sel", [8, 1024], BF16, isOutput=False)
    wpT_e = nc.declare_dram_parameter("wpT", [128, 512], BF16, isOutput=False)
    wxT_e = nc.declare_dram_parameter("wxT", [128, 512], BF16, isOutput=False)
    wfT_e = nc.declare_dram_parameter("wfT", [128, 512], BF16, isOutput=False)
    bf_e = nc.declare_dram_parameter("bf", [128, 2], F32, isOutput=False)
    out_e = nc.declare_dram_parameter("out", [8, 256], F32, isOutput=True)

    with tile.TileContext(nc) as tc:
        with (
            tc.tile_pool(name="const", bufs=1) as cp,
            tc.tile_pool(name="big", bufs=1) as bigp,
        ):
            # ---- constants / weights to SBUF ----
            wihT_sb = cp.tile([128, 2048], BF16)
            whhT_sb = cp.tile([128, 2048], BF16)
            bl_sb = cp.tile([128, 8], F32)
            wy_sb = cp.tile([128, 512], BF16)
            wtoh_sb = cp.tile([128, 128], BF16)
            sel_sb = cp.tile([8, 1024], BF16)
            wpT_sb = cp.tile([128, 512], BF16)
            wxT_sb = cp.tile([128, 512], BF16)
            wfT_sb = cp.tile([128, 512], BF16)
            bf_sb = cp.tile([128, 2], F32)
            dmas = [(wihT_sb, wihT_e), (whhT_sb, whhT_e),
                    (bl_sb, bl_e), (wy_sb, wy_e),
                    (wtoh_sb, wtoh_e), (sel_sb, sel_e), (wpT_sb, wpT_e),
                    (wxT_sb, wxT_e), (wfT_sb, wfT_e), (bf_sb, bf_e)]
            if fp8:
                whh8_sb = cp.tile([128, 2048], mybir.dt.float8e4)
                dmas.append((whh8_sb, whh8_e))
            for dst, src in dmas:
                nc.sync.dma_start(dst[:], src[:])
            ident = cp.tile([128, 128], F32)
            make_identity(nc, ident[:])
            ident_bf = cp.tile([128, 128], BF16)
            nc.vector.tensor_copy(ident_bf[:], ident[:])
            ones1w = cp.tile([1, 128], F32)
            nc.gpsimd.memset(ones1w[:], 1.0)
            ones128 = cp.tile([128, 1], F32)
            nc.gpsimd.memset(ones128[:], 1.0)

            # ---- persistent big tensors ----
            eT0 = bigp.tile([128, T_steps * 8], BF16)
            eT1 = bigp.tile([128, T_steps * 8], BF16)
            xsT = bigp.tile([128, T_steps * 64], BF16)
            hT = bigp.tile([128, (T_steps + 1) * 16], BF16)
            rT_sb = bigp.tile([128, 16], F32)
            rT_bf = bigp.tile([128, 16], BF16)
            a_sb = bigp.tile([8, T_steps], BF16)
            weff_all = bigp.tile([128, 4096], BF16)
            cgA = bigp.tile([128, 32], F32)   # cols 0:16 tanh(g_t), 16:32 c
            cgB = bigp.tile([128, 32], F32)

            # chunk-0 embedding slices first so step 0 can start early
            nc.sync.dma_start(eT0[:, 0:512], et0_e[:, 0:512])
            nc.sync.dma_start(eT1[:, 0:512], et1_e[:, 0:512])
            if T_steps > 64:
                nc.sync.dma_start(eT0[:, 512:T_steps * 8], et0_e[:, 512:T_steps * 8])
                nc.sync.dma_start(eT1[:, 512:T_steps * 8], et1_e[:, 512:T_steps * 8])

            xs_v = xsT[:].rearrange("p (t q) -> p t q", q=64)
            hT_v = hT[:].rearrange("p (t dj b) -> p dj b t", dj=2, b=8)
            nc.vector.memset(hT[:, 0:16], 0.0)
            nc.vector.memset(cgA[:, 16:32], 0.0)
            if fp8:
                hT8 = bigp.tile([128, (T_steps + 1) * 16], mybir.dt.float8e4)
                whh8_v = whh8_sb[:].rearrange("p (r x) -> p r x", r=2)
                nc.vector.memset(hT8[:, 0:16], 0.0)

            with (
                tc.tile_pool(name="xps", bufs=1, space="PSUM") as xpsm,
                tc.tile_pool(name="rec", bufs=4) as rp,
                tc.tile_pool(name="rps", bufs=2, space="PSUM") as rpsm,
                tc.tile_pool(name="yps", bufs=2, space="PSUM") as ypsm,
                tc.tile_pool(name="ytp", bufs=3) as ytp,
                tc.tile_pool(name="scp", bufs=1, space="PSUM") as scpsm,
                tc.tile_pool(name="cwp", bufs=8) as cwp,
                tc.tile_pool(name="att", bufs=2) as ap_,
                tc.tile_pool(name="sm", bufs=1) as smp,
            ):
                sc_ps = scpsm.tile([8, T_steps], F32, space="PSUM")

                def emit_xs_mm(nch, gb, holder):
                    pool, tag = (ypsm, "yps") if nch == 0 else (xpsm, "ps")
                    xps = pool.tile([128, 512], F32, space="PSUM", tag=tag)
                    nc.tensor.matmul(xps[:], wihT_sb[:, gb * 128:(gb + 1) * 128],
                                     eT0[:, nch * 512:(nch + 1) * 512],
                                     start=True, stop=False)
                    nc.tensor.matmul(xps[:], wihT_sb[:, 1024 + gb * 128:1024 + (gb + 1) * 128],
                                     eT1[:, nch * 512:(nch + 1) * 512],
                                     start=False, stop=True)
                    holder['xps'] = xps

                def emit_xs_ts_half(nch, gb, half, holder):
                    nc.vector.tensor_scalar(
                        xs_v[:, nch * 64 + half * 32:nch * 64 + (half + 1) * 32,
                             gb * 8:(gb + 1) * 8],
                        holder['xps'][:, half * 256:(half + 1) * 256],
                        bl_sb[:, gb:gb + 1], None, op0=ALU.add)

                def chunk_items(k):
                    items = []
                    for gb in range(8):
                        h = {}
                        items.append(lambda k=k, gb=gb, h=h: emit_xs_mm(k, gb, h))
                        for half in range(2):
                            items.append(lambda k=k, gb=gb, half=half, h=h:
                                         emit_xs_ts_half(k, gb, half, h))
                    return items

                # chunk-0 xs up front
                for it in chunk_items(0):
                    it()

                # circulant windows for all batch rows (sync DMA queue)
                cw_tiles = {}
                for b in range(8):
                    cw = cwp.tile([128, 384], BF16)
                    for j in range(3):
                        win = bass.AP(s2_e[:].tensor, b * 512 + j * 128, [[1, 128], [1, 128]])
                        nc.sync.dma_start(cw[:, j * 128:(j + 1) * 128], win)
                    cw_tiles[b] = cw

                # --- Weff = C_b @ W_y ---
                def emit_weff_mm(b, mj, holder):
                    cw = cw_tiles[b]
                    wps = ypsm.tile([128, 256], F32, space="PSUM", tag="yps")
                    for kc in range(2):
                        nc.tensor.matmul(wps[:], cw[:, (mj + kc) * 128:(mj + kc + 1) * 128],
                                         wy_sb[:, kc * 256:(kc + 1) * 256],
                                         start=(kc == 0), stop=(kc == 1))
                    holder['wps' + str(mj)] = wps

                def emit_weff_cp(b, mj, holder):
                    nc.vector.tensor_copy(
                        weff_all[:, b * 512 + mj * 256: b * 512 + (mj + 1) * 256],
                        holder['wps' + str(mj)])

                def emit_ygroup(k, b, ec):
                    yc = ypsm.tile([128, KW], F32, space="PSUM", tag="yps")
                    for kc in range(2):
                        nc.tensor.matmul(
                            yc[:], weff_all[:, b * 512 + kc * 256 + ec * 128:
                                            b * 512 + kc * 256 + (ec + 1) * 128],
                            hT_v[:, kc, b, 1 + KW * k: 1 + KW * (k + 1)],
                            start=(kc == 0), stop=(kc == 1))
                    yt = ytp.tile([128, KW], BF16)
                    nc.scalar.activation(yt[:], yc[:], AF.Tanh)
                    nc.tensor.matmul(sc_ps[:, KW * k:KW * (k + 1)],
                                     wtoh_sb[:, ec * 64 + b * 8: ec * 64 + (b + 1) * 8],
                                     yt[:], start=(b == 0 and ec == 0), stop=(b == 7 and ec == 1))

                xs_work = []
                for k in range(1, NCH):
                    xs_work.extend(chunk_items(k))
                y_work = []   # (avail_step, fn)
                for b in range(8):
                    holder = {}
                    for mj in range(2):
                        y_work.append((16, lambda b=b, mj=mj, h=holder: emit_weff_mm(b, mj, h)))
                        y_work.append((16, lambda b=b, mj=mj, h=holder: emit_weff_cp(b, mj, h)))
                for k in range(T_steps // KW):
                    for b in range(8):
                        for ec in range(2):
                            y_work.append((KW * (k + 1),
                                           lambda k=k, b=b, ec=ec: emit_ygroup(k, b, ec)))
                xi = yi = 0

                def emit_imm(tt):
                    gi = rpsm.tile([128, 48], F32, space="PSUM", tag="gpi")
                    gg_ = rpsm.tile([128, 16], F32, space="PSUM", tag="gpg")
                    nc.tensor.matmul(gi[:], ident_bf[:], xsT[:, tt * 64: tt * 64 + 48],
                                     start=True, stop=False)
                    nc.tensor.matmul(gg_[:], ident_bf[:], xsT[:, tt * 64 + 48: tt * 64 + 64],
                                     start=True, stop=False)
                    return gi, gg_

                if fp8:
                    hT8_v = hT8[:].rearrange("p (t r b) -> p t r b", r=2, b=8)
                cg = [cgA, cgB]
                for t in range(T_steps):
                    gpi, gpg = emit_imm(t)
                    # g-gates first so tanh(g) hides under sigma(i,f)
                    for gb in (6, 7, 0, 1, 2, 3, 4, 5):
                        out = (gpi[:, gb * 8:(gb + 1) * 8] if gb < 6
                               else gpg[:, (gb - 6) * 8:(gb - 5) * 8])
                        if fp8:
                            nc.tensor.matmul(
                                out,
                                whh8_v[:, :, gb * 128:(gb + 1) * 128],
                                hT8_v[:, t],
                                start=False, stop=(gb in (5, 7)),
                                perf_mode=mybir.MatmulPerfMode.DoubleRow)
                        else:
                            for dj in range(2):
                                nc.tensor.matmul(
                                    out,
                                    whhT_sb[:, dj * 1024 + gb * 128: dj * 1024 + (gb + 1) * 128],
                                    hT[:, t * 16 + dj * 8: t * 16 + (dj + 1) * 8],
                                    start=False, stop=(dj == 1 and gb in (5, 7)))
                    cgt = cg[t % 2]
                    cgn = cg[(t + 1) % 2]
                    # Scalar queue: tanh(g), sigma(i,f), sigma(o), [tanh(c) later]
                    nc.scalar.activation(cgt[:, 0:16], gpg[:], AF.Tanh)
                    sig_if = rp.tile([128, 32], F32)
                    nc.scalar.activation(sig_if[:], gpi[:, 0:32], AF.Sigmoid)
                    sig_o = rp.tile([128, 16], F32)
                    nc.scalar.activation(sig_o[:], gpi[:, 32:48], AF.Sigmoid)
                    # DVE: packed product, pair-sum, then h
                    m12 = rp.tile([128, 32], F32)
                    nc.vector.tensor_mul(m12[:], sig_if[:], cgt[:])
                    nc.vector.tensor_tensor(cgn[:, 16:32], m12[:, 0:16], m12[:, 16:32],
                                            op=ALU.add)
                    thc = rp.tile([128, 16], F32)
                    nc.scalar.activation(thc[:], cgn[:, 16:32], AF.Tanh)
                    if fp8:
                        nc.vector.tensor_mul(hT8[:, (t + 1) * 16:(t + 2) * 16], sig_o[:], thc[:])
                    nc.vector.tensor_mul(hT[:, (t + 1) * 16:(t + 2) * 16], sig_o[:], thc[:])
                    if xi < len(xs_work):
                        xs_work[xi]()
                        xi += 1
                    if yi < len(y_work) and y_work[yi][0] <= t:
                        y_work[yi][1]()
                        yi += 1
                while xi < len(xs_work):
                    xs_work[xi]()
                    xi += 1
                while yi < len(y_work):
                    y_work[yi][1]()
                    yi += 1

                # ---- softmax over T (free axis) ----
                mx = smp.tile([8, 1], F32)
                nc.vector.tensor_reduce(mx[:], sc_ps[:, 0:T_steps], axis=mybir.AxisListType.X, op=ALU.max)
                nmx = smp.tile([8, 1], F32)
                nc.vector.tensor_scalar_mul(nmx[:], mx[:], -1.0)
                esc = smp.tile([8, T_steps], F32)
                ssum = smp.tile([8, 1], F32)
                nc.scalar.activation(esc[:], sc_ps[:, 0:T_steps], AF.Exp,
                                     bias=nmx[:, 0:1], accum_out=ssum[:, 0:1])
                rcs = smp.tile([8, 1], F32)
                nc.vector.reciprocal(rcs[:], ssum[:])
                nc.scalar.activation(a_sb[:], esc[:], AF.Copy, scale=rcs[:, 0:1])
                # ---- r = sum_t a_t * h_t (fused multiply+accumulate) ----
                for b in range(8):
                    abc = ypsm.tile([128, T_steps], F32, space="PSUM", tag="yps")
                    nc.tensor.matmul(abc[:, 0:T_steps], sel_sb[:, b * 128:(b + 1) * 128],
                                     a_sb[:], start=True, stop=True)
                    for dj in range(2):
                        wt_ = ap_.tile([128, T_steps], F32)
                        if H_STT:
                            nc.vector.scalar_tensor_tensor(
                                out=wt_[:, 0:T_steps],
                                in0=hT_v[:, dj, b, 1:T_steps + 1],
                                scalar=1.0,
                                in1=abc[:, 0:T_steps],
                                op0=ALU.mult, op1=ALU.mult,
                                accum_out=rT_sb[:, dj * 8 + b: dj * 8 + b + 1])
                        else:
                            nc.vector.tensor_tensor(wt_[:, 0:T_steps], hT_v[:, dj, b, 1:T_steps + 1],
                                                    abc[:, 0:T_steps], op=ALU.mult)
                            nc.vector.tensor_reduce(rT_sb[:, dj * 8 + b: dj * 8 + b + 1],
                                                    wt_[:, 0:T_steps], axis=mybir.AxisListType.X,
                                                    op=ALU.add)
                nc.vector.tensor_copy(rT_bf[:], rT_sb[:])

            # ================= final MLP + softmax ==========================
            with (
                tc.tile_pool(name="fin", bufs=1) as fp,
                tc.tile_pool(name="fps", bufs=1, space="PSUM") as fpsm,
            ):
                rr_ps = fpsm.tile([128, 16], F32, space="PSUM")
                hlast = hT[:, T_steps * 16:(T_steps + 1) * 16]
                for oc in range(2):
                    for kc in range(2):
                        nc.tensor.matmul(rr_ps[:, oc * 8:(oc + 1) * 8],
                                         wpT_sb[:, kc * 256 + oc * 128: kc * 256 + (oc + 1) * 128],
                                         rT_bf[:, kc * 8:(kc + 1) * 8],
                                         start=(kc == 0), stop=False)
                    for kc in range(2):
                        nc.tensor.matmul(rr_ps[:, oc * 8:(oc + 1) * 8],
                                         wxT_sb[:, kc * 256 + oc * 128: kc * 256 + (oc + 1) * 128],
                                         hlast[:, kc * 8:(kc + 1) * 8],
                                         start=False, stop=(kc == 1))
                rrT = fp.tile([128, 16], BF16)
                nc.scalar.activation(rrT[:], rr_ps[:], AF.Tanh)
                z_ps = fpsm.tile([128, 16], F32, space="PSUM")
                for oc in range(2):
                    for kc in range(2):
                        nc.tensor.matmul(z_ps[:, oc * 8:(oc + 1) * 8],
                                         wfT_sb[:, kc * 256 + oc * 128: kc * 256 + (oc + 1) * 128],
                                         rrT[:, kc * 8:(kc + 1) * 8],
                                         start=(kc == 0), stop=(kc == 1))
                e_sb = fp.tile([128, 16], F32)
                for oc in range(2):
                    nc.scalar.activation(e_sb[:, oc * 8:(oc + 1) * 8], z_ps[:, oc * 8:(oc + 1) * 8],
                                         AF.Exp, bias=bf_sb[:, oc:oc + 1])
                cs_ps = fpsm.tile([1, 16], F32, space="PSUM")
                nc.tensor.matmul(cs_ps[:], ones128[:], e_sb[:], start=True, stop=True)
                cs_sb = fp.tile([1, 16], F32)
                nc.vector.tensor_copy(cs_sb[:], cs_ps[:])
                s8 = fp.tile([1, 8], F32)
                nc.vector.tensor_tensor(s8[:], cs_sb[0:1, 0:8], cs_sb[0:1, 8:16], op=ALU.add)
                rc8 = fp.tile([1, 8], F32)
                nc.vector.reciprocal(rc8[:], s8[:])
                rc16 = fp.tile([1, 16], F32)
                nc.vector.tensor_copy(rc16[:, 0:8], rc8[:])
                nc.vector.tensor_copy(rc16[:, 8:16], rc8[:])
                rbc_ps = fpsm.tile([128, 16], F32, space="PSUM")
                nc.tensor.matmul(rbc_ps[:], ones1w[:], rc16[:], start=True, stop=True)
                yT_sb = fp.tile([128, 16], F32)
                nc.vector.tensor_tensor(yT_sb[:], e_sb[:], rbc_ps[:], op=ALU.mult)
                ytr_ps = fpsm.tile([16, 128], F32, space="PSUM")
                nc.tensor.transpose(ytr_ps[:], yT_sb[:], ident[:])
                ynat = fp.tile([16, 128], F32)
                nc.vector.tensor_copy(ynat[:], ytr_ps[:])
                for oc in range(2):
                    nc.sync.dma_start(out_e[0:8, oc * 128:(oc + 1) * 128],
                                      ynat[oc * 8:(oc + 1) * 8, :])

    nc.compile()
    return nc


def build_h(T_steps=512):
    nc = bacc.Bacc(None, target_bir_lowering=False)
    NT = T_steps * BL // 128          # gather tiles of 128 tokens
    NCH = T_steps * BL // 512         # 512-token chunks for xs matmul
    ten = _declare(nc, T_steps)
    emb_e, xp_e, sidx_e = ten['emb'], ten['xp'], ten['sidx']
    out_e, s2d = ten['out'], ten['s2d']

    with tile.TileContext(nc) as tc:
        with (
            tc.tile_pool(name="const", bufs=1) as cp,
            tc.tile_pool(name="big", bufs=1) as bigp,
        ):
            # ---- constants / weights to SBUF ----
            xp_sb = cp.tile([128, NT], I32)
            sidx_sb = cp.tile([64, 1], I32)
            wihT_sb = cp.tile([128, 2048], BF16)
            whhT_sb = cp.tile([128, 2048], BF16)
            bl_sb = cp.tile([128, 8], F32)
            wy_sb = cp.tile([128, 512], BF16)
            wtoh_sb = cp.tile([128, 128], BF16)
            sel_sb = cp.tile([8, 1024], BF16)
            wpT_sb = cp.tile([128, 512], BF16)
            wxT_sb = cp.tile([128, 512], BF16)
            wfT_sb = cp.tile([128, 512], BF16)
            bf_sb = cp.tile([128, 2], F32)
            for dst, src in [(xp_sb, ten['xp']), (sidx_sb, ten['sidx']),
                             (wihT_sb, ten['wihT']), (whhT_sb, ten['whhT']),
                             (bl_sb, ten['bl']), (wy_sb, ten['wy']),
                             (wtoh_sb, ten['wtoh']), (sel_sb, ten['sel']),
                             (wpT_sb, ten['wpT']), (wxT_sb, ten['wxT']),
                             (wfT_sb, ten['wfT']), (bf_sb, ten['bf'])]:
                nc.sync.dma_start(dst[:], src[:])
            ident = cp.tile([128, 128], F32)
            make_identity(nc, ident[:])
            ident_bf = cp.tile([128, 128], BF16)
            nc.vector.tensor_copy(ident_bf[:], ident[:])
            ones64 = cp.tile([64, 1], F32)
            nc.gpsimd.memset(ones64[:], 1.0)
            ones1w = cp.tile([1, 128], F32)
            nc.gpsimd.memset(ones1w[:], 1.0)
            ones128 = cp.tile([128, 1], F32)
            nc.gpsimd.memset(ones128[:], 1.0)

            # ---- persistent big tensors ----
            eT0 = bigp.tile([128, T_steps * 8], BF16)
            eT1 = bigp.tile([128, T_steps * 8], BF16)
            xsT = bigp.tile([128, T_steps * 64], BF16)
            hT = bigp.tile([128, (T_steps + 1) * 16], BF16)
            s2_sb = bigp.tile([8, 512], BF16)
            rT_sb = bigp.tile([128, 16], F32)
            rT_bf = bigp.tile([128, 16], BF16)
            a_sb = bigp.tile([8, T_steps], BF16)
            weff_all = bigp.tile([128, 4096], BF16)
            cgA = bigp.tile([128, 32], F32)   # cols 0:16 tanh(g_t), 16:32 c
            cgB = bigp.tile([128, 32], F32)

            xs_v = xsT[:].rearrange("p (t q) -> p t q", q=64)
            hT_v = hT[:].rearrange("p (t dj b) -> p dj b t", dj=2, b=8)
            nc.vector.memset(hT[:, 0:16], 0.0)
            nc.vector.memset(cgA[:, 16:32], 0.0)

            with (
                tc.tile_pool(name="gat", bufs=NT) as gp,
                tc.tile_pool(name="xps", bufs=1, space="PSUM") as xpsm,
                tc.tile_pool(name="rec", bufs=4) as rp,
                tc.tile_pool(name="rps", bufs=2, space="PSUM") as rpsm,
                tc.tile_pool(name="yps", bufs=2, space="PSUM") as ypsm,
                tc.tile_pool(name="ytp", bufs=3) as ytp,
                tc.tile_pool(name="scp", bufs=1, space="PSUM") as scpsm,
                tc.tile_pool(name="cwp", bufs=8) as cwp,
                tc.tile_pool(name="att", bufs=2) as ap_,
                tc.tile_pool(name="sm", bufs=1) as smp,
                tc.tile_pool(name="swork", bufs=1) as sw,
            ):
                sc_ps = scpsm.tile([8, T_steps], F32, space="PSUM")

                def emit_gather(g):
                    egath = gp.tile([128, 256], F32)
                    nc.gpsimd.indirect_dma_start(
                        out=egath[:], out_offset=None, in_=emb_e[:],
                        in_offset=IndirectOffsetOnAxis(ap=xp_sb[:, g:g + 1], axis=0))
                    return egath

                def emit_trcopy(egath, g, dc):
                    eT = eT0 if dc == 0 else eT1
                    tps = xpsm.tile([128, 128], F32, space="PSUM", tag="ps")
                    nc.tensor.transpose(tps[:], egath[:, dc * 128:(dc + 1) * 128], ident[:])
                    nc.vector.tensor_copy(eT[:, g * 128:(g + 1) * 128], tps[:])

                def emit_xs_mm(nch, gb, holder):
                    xps = xpsm.tile([128, 512], F32, space="PSUM", tag="ps")
                    nc.tensor.matmul(xps[:], wihT_sb[:, gb * 128:(gb + 1) * 128],
                                     eT0[:, nch * 512:(nch + 1) * 512],
                                     start=True, stop=False)
                    nc.tensor.matmul(xps[:], wihT_sb[:, 1024 + gb * 128:1024 + (gb + 1) * 128],
                                     eT1[:, nch * 512:(nch + 1) * 512],
                                     start=False, stop=True)
                    holder['xps'] = xps

                def emit_xs_ts_half(nch, gb, half, holder):
                    nc.vector.tensor_scalar(
                        xs_v[:, nch * 64 + half * 32:nch * 64 + (half + 1) * 32,
                             gb * 8:(gb + 1) * 8],
                        holder['xps'][:, half * 256:(half + 1) * 256],
                        bl_sb[:, gb:gb + 1], None, op0=ALU.add)

                egaths = {}

                def chunk_items(k):
                    items = []
                    for g in range(4 * k, 4 * k + 4):
                        for dc in range(2):
                            items.append(lambda g=g, dc=dc: emit_trcopy(
                                egaths.pop(g) if dc else egaths[g], g, dc))
                    for gb in range(8):
                        h = {}
                        items.append(lambda k=k, gb=gb, h=h: emit_xs_mm(k, gb, h))
                        for half in range(2):
                            items.append(lambda k=k, gb=gb, half=half, h=h:
                                         emit_xs_ts_half(k, gb, half, h))
                    return items

                # gathers: chunk-0 first, then aspect row, then the rest.
                for g in range(4):
                    egaths[g] = emit_gather(g)
                semb = sw.tile([64, 256], F32)
                nc.gpsimd.indirect_dma_start(
                    out=semb[:], out_offset=None, in_=emb_e[:],
                    in_offset=IndirectOffsetOnAxis(ap=sidx_sb[:, :1], axis=0))
                for g in range(4, NT):
                    egaths[g] = emit_gather(g)

                # chunk-0 transposes + xs up front
                for it in chunk_items(0):
                    it()

                # ---- s-branch (batchnormed aspect embedding); PSUM stats run
                # sequentially through the xpsm ring slot.
                mu_ps = xpsm.tile([64, 256], F32, space="PSUM", tag="ps")
                nc.tensor.matmul(mu_ps[0:1, :], ones64[:], semb[:], start=True, stop=True)
                mu = sw.tile([1, 256], F32)
                nc.vector.tensor_scalar_mul(mu[:], mu_ps[0:1, :], 1.0 / 64)
                sq = sw.tile([64, 256], F32)
                nc.vector.tensor_mul(sq[:], semb[:], semb[:])
                ms_ps = xpsm.tile([64, 256], F32, space="PSUM", tag="ps")
                nc.tensor.matmul(ms_ps[0:1, :], ones64[:], sq[:], start=True, stop=True)
                msq = sw.tile([1, 256], F32)
                nc.vector.tensor_scalar_mul(msq[:], ms_ps[0:1, :], 1.0 / 64)
                mu2 = sw.tile([1, 256], F32)
                nc.vector.tensor_mul(mu2[:], mu[:], mu[:])
                var = sw.tile([1, 256], F32)
                nc.vector.tensor_tensor(var[:], msq[:], mu2[:], op=ALU.subtract)
                nc.vector.tensor_scalar_add(var[:], var[:], 1e-5)
                std = sw.tile([1, 256], F32)
                nc.scalar.sqrt(std[:], var[:])
                istd = sw.tile([1, 256], F32)
                nc.vector.reciprocal(istd[:], std[:])
                mub_ps = xpsm.tile([64, 256], F32, space="PSUM", tag="ps")
                nc.tensor.matmul(mub_ps[:], ones1w[:1, :64], mu[:], start=True, stop=True)
                d8 = sw.tile([8, 256], F32)
                nc.vector.tensor_tensor(d8[:], semb[0:8, :], mub_ps[0:8, :], op=ALU.subtract)
                ib_ps = xpsm.tile([64, 256], F32, space="PSUM", tag="ps")
                nc.tensor.matmul(ib_ps[:], ones1w[:1, :64], istd[:], start=True, stop=True)
                nc.vector.tensor_tensor(s2_sb[:, 0:256], d8[:], ib_ps[0:8, :], op=ALU.mult)
                nc.vector.tensor_copy(s2_sb[:, 256:512], s2_sb[:, 0:256])
                nc.sync.dma_start(s2d[:], s2_sb[:])

                # circulant windows for all batch rows (sync DMA queue)
                cw_tiles = {}
                for b in range(8):
                    cw = cwp.tile([128, 384], BF16)
                    for j in range(3):
                        win = bass.AP(s2d[:].tensor, b * 512 + j * 128, [[1, 128], [1, 128]])
                        if H_CW_SYNC:
                            nc.sync.dma_start(cw[:, j * 128:(j + 1) * 128], win)
                        else:
                            nc.gpsimd.dma_start(cw[:, j * 128:(j + 1) * 128], win)
                    cw_tiles[b] = cw

                # --- Weff = C_b @ W_y ---
                def emit_weff_mm(b, mj, holder):
                    cw = cw_tiles[b]
                    wps = ypsm.tile([128, 256], F32, space="PSUM", tag="yps")
                    for kc in range(2):
                        nc.tensor.matmul(wps[:], cw[:, (mj + kc) * 128:(mj + kc + 1) * 128],
                                         wy_sb[:, kc * 256:(kc + 1) * 256],
                                         start=(kc == 0), stop=(kc == 1))
                    holder['wps' + str(mj)] = wps

                def emit_weff_cp(b, mj, holder):
                    nc.vector.tensor_copy(
                        weff_all[:, b * 512 + mj * 256: b * 512 + (mj + 1) * 256],
                        holder['wps' + str(mj)])

                def emit_ygroup(k, b, ec):
                    yc = ypsm.tile([128, 64], F32, space="PSUM", tag="yps")
                    for kc in range(2):
                        nc.tensor.matmul(
                            yc[:], weff_all[:, b * 512 + kc * 256 + ec * 128:
                                            b * 512 + kc * 256 + (ec + 1) * 128],
                            hT_v[:, kc, b, 1 + 64 * k: 1 + 64 * (k + 1)],
                            start=(kc == 0), stop=(kc == 1))
                    yt = ytp.tile([128, 64], BF16)
                    nc.scalar.activation(yt[:], yc[:], AF.Tanh)
                    nc.tensor.matmul(sc_ps[:, 64 * k:64 * (k + 1)],
                                     wtoh_sb[:, ec * 64 + b * 8: ec * 64 + (b + 1) * 8],
                                     yt[:], start=(b == 0 and ec == 0), stop=(b == 7 and ec == 1))

                xs_work = []
                for k in range(1, NCH):
                    xs_work.extend(chunk_items(k))
                y_work = []   # (avail_step, fn)
                for b in range(8):
                    holder = {}
                    for mj in range(2):
                        y_work.append((12, lambda b=b, mj=mj, h=holder: emit_weff_mm(b, mj, h)))
                        y_work.append((12, lambda b=b, mj=mj, h=holder: emit_weff_cp(b, mj, h)))
                for k in range(T_steps // 64):
                    for b in range(8):
                        for ec in range(2):
                            y_work.append((64 * (k + 1),
                                           lambda k=k, b=b, ec=ec: emit_ygroup(k, b, ec)))
                xi = yi = 0

                def emit_imm(tt):
                    gi = rpsm.tile([128, 48], F32, space="PSUM", tag="gpi")
                    gg_ = rpsm.tile([128, 16], F32, space="PSUM", tag="gpg")
                    nc.tensor.matmul(gi[:], ident_bf[:], xsT[:, tt * 64: tt * 64 + 48],
                                     start=True, stop=False)
                    nc.tensor.matmul(gg_[:], ident_bf[:], xsT[:, tt * 64 + 48: tt * 64 + 64],
                                     start=True, stop=False)
                    return gi, gg_

                cg = [cgA, cgB]
                for t in range(T_steps):
                    gpi, gpg = emit_imm(t)
                    # g-gates first so tanh(g) hides under sigma(i,f)
                    for gb in (6, 7, 0, 1, 2, 3, 4, 5):
                        out = (gpi[:, gb * 8:(gb + 1) * 8] if gb < 6
                               else gpg[:, (gb - 6) * 8:(gb - 5) * 8])
                        for dj in range(2):
                            nc.tensor.matmul(
                                out,
                                whhT_sb[:, dj * 1024 + gb * 128: dj * 1024 + (gb + 1) * 128],
                                hT[:, t * 16 + dj * 8: t * 16 + (dj + 1) * 8],
                                start=False, stop=(dj == 1 and gb in (5, 7)))
                    cgt = cg[t % 2]
                    cgn = cg[(t + 1) % 2]
                    # Scalar queue: tanh(g), sigma(i,f), sigma(o), [tanh(c) later]
                    nc.scalar.activation(cgt[:, 0:16], gpg[:], AF.Tanh)
                    sig_if = rp.tile([128, 32], F32)
                    nc.scalar.activation(sig_if[:], gpi[:, 0:32], AF.Sigmoid)
                    sig_o = rp.tile([128, 16], F32)
                    nc.scalar.activation(sig_o[:], gpi[:, 32:48], AF.Sigmoid)
                    # DVE: packed product, pair-sum, then h
                    m12 = rp.tile([128, 32], F32)
                    nc.vector.tensor_mul(m12[:], sig_if[:], cgt[:])
                    nc.vector.tensor_tensor(cgn[:, 16:32], m12[:, 0:16], m12[:, 16:32],
                                            op=ALU.add)
                    thc = rp.tile([128, 16], F32)
                    nc.scalar.activation(thc[:], cgn[:, 16:32], AF.Tanh)
                    nc.vector.tensor_mul(hT[:, (t + 1) * 16:(t + 2) * 16], sig_o[:], thc[:])
                    if xi < len(xs_work):
                        xs_work[xi]()
                        xi += 1
                    if yi < len(y_work) and y_work[yi][0] <= t:
                        y_work[yi][1]()
                        yi += 1
                while xi < len(xs_work):
                    xs_work[xi]()
                    xi += 1
                while yi < len(y_work):
                    y_work[yi][1]()
                    yi += 1

                # ---- softmax over T (free axis) ----
                mx = smp.tile([8, 1], F32)
                nc.vector.tensor_reduce(mx[:], sc_ps[:, 0:T_steps], axis=mybir.AxisListType.X, op=ALU.max)
                nmx = smp.tile([8, 1], F32)
                nc.vector.tensor_scalar_mul(nmx[:], mx[:], -1.0)
                esc = smp.tile([8, T_steps], F32)
                ssum = smp.tile([8, 1], F32)
                nc.scalar.activation(esc[:], sc_ps[:, 0:T_steps], AF.Exp,
                                     bias=nmx[:, 0:1], accum_out=ssum[:, 0:1])
                rcs = smp.tile([8, 1], F32)
                nc.vector.reciprocal(rcs[:], ssum[:])
                nc.scalar.activation(a_sb[:], esc[:], AF.Copy, scale=rcs[:, 0:1])
                # ---- r = sum_t a_t * h_t (fused multiply+reduce) ----
                for b in range(8):
                    abc = ypsm.tile([128, T_steps], F32, space="PSUM", tag="yps")
                    nc.tensor.matmul(abc[:, 0:T_steps], sel_sb[:, b * 128:(b + 1) * 128],
                                     a_sb[:], start=True, stop=True)
                    for dj in range(2):
                        wt_ = ap_.tile([128, T_steps], F32)
                        if H_TTR:
                            nc.vector.tensor_tensor_reduce(
                                out=wt_[:, 0:T_steps],
                                in0=hT_v[:, dj, b, 1:T_steps + 1],
                                in1=abc[:, 0:T_steps],
                                scale=1.0, scalar=0.0,
                                op0=ALU.mult, op1=ALU.add,
                                accum_out=rT_sb[:, dj * 8 + b: dj * 8 + b + 1])
                        else:
                            nc.vector.tensor_tensor(wt_[:, 0:T_steps], hT_v[:, dj, b, 1:T_steps + 1],
                                                    abc[:, 0:T_steps], op=ALU.mult)
                            nc.vector.tensor_reduce(rT_sb[:, dj * 8 + b: dj * 8 + b + 1],
                                                    wt_[:, 0:T_steps], axis=mybir.AxisListType.X,
                                                    op=ALU.add)
                nc.vector.tensor_copy(rT_bf[:], rT_sb[:])

            # ================= final MLP + softmax ==========================
            with (
                tc.tile_pool(name="fin", bufs=1) as fp,
                tc.tile_pool(name="fps", bufs=1, space="PSUM") as fpsm,
            ):
                rr_ps = fpsm.tile([128, 16], F32, space="PSUM")
                hlast = hT[:, T_steps * 16:(T_steps + 1) * 16]
                for oc in range(2):
                    for kc in range(2):
                        nc.tensor.matmul(rr_ps[:, oc * 8:(oc + 1) * 8],
                                         wpT_sb[:, kc * 256 + oc * 128: kc * 256 + (oc + 1) * 128],
                                         rT_bf[:, kc * 8:(kc + 1) * 8],
                                         start=(kc == 0), stop=False)
                    for kc in range(2):
                        nc.tensor.matmul(rr_ps[:, oc * 8:(oc + 1) * 8],
                                         wxT_sb[:, kc * 256 + oc * 128: kc * 256 + (oc + 1) * 128],
                                         hlast[:, kc * 8:(kc + 1) * 8],
                                         start=False, stop=(kc == 1))
                rrT = fp.tile([128, 16], BF16)
                nc.scalar.activation(rrT[:], rr_ps[:], AF.Tanh)
                z_ps = fpsm.tile([128, 16], F32, space="PSUM")
                for oc in range(2):
                    for kc in range(2):
                        nc.tensor.matmul(z_ps[:, oc * 8:(oc + 1) * 8],
                                         wfT_sb[:, kc * 256 + oc * 128: kc * 256 + (oc + 1) * 128],
                                         rrT[:, kc * 8:(kc + 1) * 8],
                                         start=(kc == 0), stop=(kc == 1))
                e_sb = fp.tile([128, 16], F32)
                for oc in range(2):
                    nc.scalar.activation(e_sb[:, oc * 8:(oc + 1) * 8], z_ps[:, oc * 8:(oc + 1) * 8],
                                         AF.Exp, bias=bf_sb[:, oc:oc + 1])
                cs_ps = fpsm.tile([1, 16], F32, space="PSUM")
                nc.tensor.matmul(cs_ps[:], ones128[:], e_sb[:], start=True, stop=True)
                cs_sb = fp.tile([1, 16], F32)
                nc.vector.tensor_copy(cs_sb[:], cs_ps[:])
                s8 = fp.tile([1, 8], F32)
                nc.vector.tensor_tensor(s8[:], cs_sb[0:1, 0:8], cs_sb[0:1, 8:16], op=ALU.add)
                rc8 = fp.tile([1, 8], F32)
                nc.vector.reciprocal(rc8[:], s8[:])
                rc16 = fp.tile([1, 16], F32)
                nc.vector.tensor_copy(rc16[:, 0:8], rc8[:])
                nc.vector.tensor_copy(rc16[:, 8:16], rc8[:])
                rbc_ps = fpsm.tile([128, 16], F32, space="PSUM")
                nc.tensor.matmul(rbc_ps[:], ones1w[:], rc16[:], start=True, stop=True)
                yT_sb = fp.tile([128, 16], F32)
                nc.vector.tensor_tensor(yT_sb[:], e_sb[:], rbc_ps[:], op=ALU.mult)
                ytr_ps = fpsm.tile([16, 128], F32, space="PSUM")
                nc.tensor.transpose(ytr_ps[:], yT_sb[:], ident[:])
                ynat = fp.tile([16, 128], F32)
                nc.vector.tensor_copy(ynat[:], ytr_ps[:])
                for oc in range(2):
                    nc.sync.dma_start(out_e[0:8, oc * 128:(oc + 1) * 128],
                                      ynat[oc * 8:(oc + 1) * 8, :])

    nc.compile()
    return nc


def build(T_steps=512, variant=None):
    variant = VARIANT if variant is None else variant
    if variant == "i":
        return build_i(T_steps)
    if variant == "j":
        return build_i(T_steps, fp8=True)
    if variant == "h":
        return build_h(T_steps)
    nc = bacc.Bacc(None, target_bir_lowering=False)
    NT = T_steps * BL // 128          # gather tiles of 128 tokens
    NCH = T_steps * BL // 512         # 512-token chunks for xs matmul

    ten = _declare(nc, T_steps)
    emb_e, xp_e, sidx_e = ten['emb'], ten['xp'], ten['sidx']
    wihT_e, whhT_e, whh8_e, bl_e = ten['wihT'], ten['whhT'], ten['whh8'], ten['bl']
    wy_e, wtoh_e, sel_e = ten['wy'], ten['wtoh'], ten['sel']
    wpT_e, wxT_e, wfT_e, bf_e = ten['wpT'], ten['wxT'], ten['wfT'], ten['bf']
    out_e, s2d = ten['out'], ten['s2d']

    with tile.TileContext(nc) as tc:
        with (
            tc.tile_pool(name="const", bufs=1) as cp,
            tc.tile_pool(name="big", bufs=1) as bigp,
        ):
            # ---- constants / weights to SBUF ----
            xp_sb = cp.tile([128, NT], I32)
            sidx_sb = cp.tile([64, 1], I32)
            wihT_sb = cp.tile([128, 2048], BF16)
            whhT_sb = cp.tile([128, 2048], BF16)
            whh8_sb = cp.tile([128, 2048], mybir.dt.float8e4)
            bl_sb = cp.tile([128, 8], F32)
            wy_sb = cp.tile([128, 512], BF16)
            wtoh_sb = cp.tile([128, 128], BF16)
            sel_sb = cp.tile([8, 1024], BF16)
            wpT_sb = cp.tile([128, 512], BF16)
            wxT_sb = cp.tile([128, 512], BF16)
            wfT_sb = cp.tile([128, 512], BF16)
            bf_sb = cp.tile([128, 2], F32)
            for dst, src in [(xp_sb, xp_e), (sidx_sb, sidx_e), (wihT_sb, wihT_e),
                             (whhT_sb, whhT_e), (whh8_sb, whh8_e), (bl_sb, bl_e), (wy_sb, wy_e),
                             (wtoh_sb, wtoh_e), (sel_sb, sel_e), (wpT_sb, wpT_e),
                             (wxT_sb, wxT_e), (wfT_sb, wfT_e), (bf_sb, bf_e)]:
                nc.sync.dma_start(dst[:], src[:])
            ident = cp.tile([128, 128], F32)
            make_identity(nc, ident[:])
            ident_bf = cp.tile([128, 128], BF16)
            nc.vector.tensor_copy(ident_bf[:], ident[:])
            ones64 = cp.tile([64, 1], F32)
            nc.gpsimd.memset(ones64[:], 1.0)
            ones1w = cp.tile([1, 128], F32)
            nc.gpsimd.memset(ones1w[:], 1.0)
            ones128 = cp.tile([128, 1], F32)
            nc.gpsimd.memset(ones128[:], 1.0)

            # ---- persistent big tensors ----
            eT0 = bigp.tile([128, T_steps * 8], BF16)
            eT1 = bigp.tile([128, T_steps * 8], BF16)
            xsT = bigp.tile([128, T_steps * 64], BF16)
            hT = bigp.tile([128, (T_steps + 1) * 16], BF16)
            s2_sb = bigp.tile([8, 512], BF16)
            rT_sb = bigp.tile([128, 16], F32)
            rT_bf = bigp.tile([128, 16], BF16)
            a_sb = bigp.tile([8, T_steps], BF16)

            # ================= s-branch (batchnormed aspect embedding) ======
            with (
                tc.tile_pool(name="swork", bufs=1) as sw,
                tc.tile_pool(name="spsum", bufs=1, space="PSUM") as sps,
            ):
                semb = sw.tile([64, 256], F32)
                nc.gpsimd.indirect_dma_start(
                    out=semb[:], out_offset=None, in_=emb_e[:],
                    in_offset=IndirectOffsetOnAxis(ap=sidx_sb[:, :1], axis=0))
                mu_ps = sps.tile([1, 256], F32, space="PSUM")
                nc.tensor.matmul(mu_ps[:], ones64[:], semb[:], start=True, stop=True)
                mu = sw.tile([1, 256], F32)
                nc.vector.tensor_scalar_mul(mu[:], mu_ps[:], 1.0 / 64)
                sq = sw.tile([64, 256], F32)
                nc.vector.tensor_mul(sq[:], semb[:], semb[:])
                ms_ps = sps.tile([1, 256], F32, space="PSUM")
                nc.tensor.matmul(ms_ps[:], ones64[:], sq[:], start=True, stop=True)
                msq = sw.tile([1, 256], F32)
                nc.vector.tensor_scalar_mul(msq[:], ms_ps[:], 1.0 / 64)
                mu2 = sw.tile([1, 256], F32)
                nc.vector.tensor_mul(mu2[:], mu[:], mu[:])
                var = sw.tile([1, 256], F32)
                nc.vector.tensor_tensor(var[:], msq[:], mu2[:], op=ALU.subtract)
                nc.vector.tensor_scalar_add(var[:], var[:], 1e-5)
                std = sw.tile([1, 256], F32)
                nc.scalar.sqrt(std[:], var[:])
                istd = sw.tile([1, 256], F32)
                nc.vector.reciprocal(istd[:], std[:])
                mub_ps = sps.tile([64, 256], F32, space="PSUM")
                nc.tensor.matmul(mub_ps[:], ones1w[:1, :64], mu[:], start=True, stop=True)
                ib_ps = sps.tile([64, 256], F32, space="PSUM")
                nc.tensor.matmul(ib_ps[:], ones1w[:1, :64], istd[:], start=True, stop=True)
                d8 = sw.tile([8, 256], F32)
                nc.vector.tensor_tensor(d8[:], semb[0:8, :], mub_ps[0:8, :], op=ALU.subtract)
                nc.vector.tensor_tensor(s2_sb[:, 0:256], d8[:], ib_ps[0:8, :], op=ALU.mult)
                nc.vector.tensor_copy(s2_sb[:, 256:512], s2_sb[:, 0:256])
                nc.gpsimd.dma_start(s2d[:], s2_sb[:])

            # ========= gather/xs/Weff/Y/score interleaved with recurrence ===
            xs_v = xsT[:].rearrange("p (t q) -> p t q", q=64)
            hT_v = hT[:].rearrange("p (t dj b) -> p dj b t", dj=2, b=8)
            weff_all = bigp.tile([128, 4096], BF16)
            nc.gpsimd.memset(hT[:, 0:16], 0.0)
            with (
                tc.tile_pool(name="gat", bufs=8) as gp,
                tc.tile_pool(name="xps", bufs=1, space="PSUM") as xpsm,
                tc.tile_pool(name="rec", bufs=3) as rp,
                tc.tile_pool(name="cst", bufs=3) as cpp,
                tc.tile_pool(name="rps", bufs=2, space="PSUM") as rpsm,
                tc.tile_pool(name="yps", bufs=2, space="PSUM") as ypsm,
                tc.tile_pool(name="ytp", bufs=3) as ytp,
                tc.tile_pool(name="scp", bufs=1, space="PSUM") as scpsm,
                tc.tile_pool(name="att", bufs=2) as ap_,
                tc.tile_pool(name="sm", bufs=1) as smp,
            ):
                sc_ps = scpsm.tile([8, T_steps], F32, space="PSUM")

                def emit_gather(g):
                    egath = gp.tile([128, 256], F32)
                    nc.gpsimd.indirect_dma_start(
                        out=egath[:], out_offset=None, in_=emb_e[:],
                        in_offset=IndirectOffsetOnAxis(ap=xp_sb[:, g:g + 1], axis=0))
                    return egath

                def emit_trcopy(egath, g, dc):
                    eT = eT0 if dc == 0 else eT1
                    tps = xpsm.tile([128, 128], F32, space="PSUM", tag="ps")
                    nc.tensor.transpose(tps[:], egath[:, dc * 128:(dc + 1) * 128], ident[:])
                    nc.vector.tensor_copy(eT[:, g * 128:(g + 1) * 128], tps[:])

                def emit_xs_mm(nch, gb, holder):
                    xps = xpsm.tile([128, 512], F32, space="PSUM", tag="ps")
                    nc.tensor.matmul(xps[:], wihT_sb[:, gb * 128:(gb + 1) * 128],
                                     eT0[:, nch * 512:(nch + 1) * 512],
                                     start=True, stop=False)
                    nc.tensor.matmul(xps[:], wihT_sb[:, 1024 + gb * 128:1024 + (gb + 1) * 128],
                                     eT1[:, nch * 512:(nch + 1) * 512],
                                     start=False, stop=True)
                    holder['xps'] = xps

                def emit_xs_ts(nch, gb, holder):
                    nc.vector.tensor_scalar(
                        xs_v[:, nch * 64:(nch + 1) * 64, gb * 8:(gb + 1) * 8],
                        holder['xps'], bl_sb[:, gb:gb + 1], None, op0=ALU.add)

                def emit_xs(nch, gb):
                    h = {}
                    emit_xs_mm(nch, gb, h)
                    emit_xs_ts(nch, gb, h)

                egaths = {}

                def chunk_items(k):
                    items = []
                    for g in range(4 * k, 4 * k + 4):
                        if variant == "g":
                            for dc in range(2):
                                items.append(lambda g=g, dc=dc: emit_trcopy(egaths.pop(g) if dc else egaths[g], g, dc))
                        else:
                            holder = {}
                            items.append(lambda g=g, h=holder: h.__setitem__('e', emit_gather(g)))
                            for dc in range(2):
                                items.append(lambda g=g, dc=dc, h=holder: emit_trcopy(h['e'], g, dc))
                    for gb in range(8):
                        h = {}
                        items.append(lambda k=k, gb=gb, h=h: emit_xs_mm(k, gb, h))
                        items.append(lambda k=k, gb=gb, h=h: emit_xs_ts(k, gb, h))
                    return items

                if variant == "g":
                    for g in range(NT):
                        egaths[g] = emit_gather(g)

                # --- Weff = C_b @ W_y (only needs the s-branch) ---
                def emit_cw(b, holder):
                    cw = ap_.tile([128, 384], BF16)
                    for j in range(3):
                        win = bass.AP(s2d[:].tensor, b * 512 + j * 128, [[1, 128], [1, 128]])
                        nc.gpsimd.dma_start(cw[:, j * 128:(j + 1) * 128], win)
                    holder['cw'] = cw

                def emit_weff_mm(b, mj, holder):
                    cw = holder['cw']
                    wps = ypsm.tile([128, 256], F32, space="PSUM", tag="yps")
                    for kc in range(2):
                        nc.tensor.matmul(wps[:], cw[:, (mj + kc) * 128:(mj + kc + 1) * 128],
                                         wy_sb[:, kc * 256:(kc + 1) * 256],
                                         start=(kc == 0), stop=(kc == 1))
                    holder['wps' + str(mj)] = wps

                def emit_weff_cp(b, mj, holder):
                    nc.vector.tensor_copy(weff_all[:, b * 512 + mj * 256: b * 512 + (mj + 1) * 256],
                                          holder['wps' + str(mj)])

                def emit_ygroup(k, b, ec):
                    yc = ypsm.tile([128, 64], F32, space="PSUM", tag="yps")
                    for kc in range(2):
                        nc.tensor.matmul(
                            yc[:], weff_all[:, b * 512 + kc * 256 + ec * 128: b * 512 + kc * 256 + (ec + 1) * 128],
                            hT_v[:, kc, b, 1 + 64 * k: 1 + 64 * (k + 1)],
                            start=(kc == 0), stop=(kc == 1))
                    yt = ytp.tile([128, 64], BF16)
                    nc.scalar.activation(yt[:], yc[:], AF.Tanh)
                    nc.tensor.matmul(sc_ps[:, 64 * k:64 * (k + 1)],
                                     wtoh_sb[:, ec * 64 + b * 8: ec * 64 + (b + 1) * 8],
                                     yt[:], start=(b == 0 and ec == 0), stop=(b == 7 and ec == 1))

                # chunk 0 of gather/xs up front; everything else trickles in
                for it in chunk_items(0):
                    it()
                xs_work = []
                for k in range(1, NCH):
                    xs_work.extend(chunk_items(k))
                y_work = []   # (avail_step, fn)
                for b in range(8):
                    holder = {}
                    y_work.append((0, lambda b=b, h=holder: emit_cw(b, h)))
                    for mj in range(2):
                        y_work.append((0, lambda b=b, mj=mj, h=holder: emit_weff_mm(b, mj, h)))
                        y_work.append((0, lambda b=b, mj=mj, h=holder: emit_weff_cp(b, mj, h)))
                for k in range(T_steps // 64):
                    for b in range(8):
                        for ec in range(2):
                            y_work.append((64 * (k + 1),
                                           lambda k=k, b=b, ec=ec: emit_ygroup(k, b, ec)))
                xi = yi = 0

                c_prev = cpp.tile([128, 16], F32)
                nc.vector.memset(c_prev[:], 0.0)
                def emit_imm(tt):
                    gi = rpsm.tile([128, 48], F32, space="PSUM", tag="gpi")
                    gg_ = rpsm.tile([128, 16], F32, space="PSUM", tag="gpg")
                    nc.tensor.matmul(gi[:], ident_bf[:], xsT[:, tt * 64: tt * 64 + 48],
                                     start=True, stop=False)
                    nc.tensor.matmul(gg_[:], ident_bf[:], xsT[:, tt * 64 + 48: tt * 64 + 64],
                                     start=True, stop=False)
                    return gi, gg_

                if variant == "d":
                    gpi, gpg = emit_imm(0)
                for t in range(T_steps):
                    if variant != "d":
                        gpi, gpg = emit_imm(t)
                    gb_order = (6, 7, 0, 1, 2, 3, 4, 5) if variant == "d" else (0, 1, 2, 3, 4, 5, 6, 7)
                    w_sb = whh8_sb if variant == "f" else whhT_sb
                    for gb in gb_order:
                        out = gpi[:, gb * 8:(gb + 1) * 8] if gb < 6 else gpg[:, (gb - 6) * 8:(gb - 5) * 8]
                        for dj in range(2):
                            nc.tensor.matmul(
                                out,
                                w_sb[:, dj * 1024 + gb * 128: dj * 1024 + (gb + 1) * 128],
                                hT[:, t * 16 + dj * 8: t * 16 + (dj + 1) * 8],
                                start=False, stop=(dj == 1 and gb in (5, 7)))
                    gpi_t, gpg_t = gpi, gpg
                    if variant == "d" and t + 1 < T_steps:
                        gpi, gpg = emit_imm(t + 1)
                    if variant == "d":
                        gg = rp.tile([128, 16], F32)
                        nc.scalar.activation(gg[:], gpg_t[:], AF.Tanh)
                        sig = rp.tile([128, 48], F32)
                        nc.scalar.activation(sig[:], gpi_t[:], AF.Sigmoid)
                    else:
                        sig = rp.tile([128, 48], F32)
                        nc.scalar.activation(sig[:], gpi_t[:], AF.Sigmoid)
                        gg = rp.tile([128, 16], F32)
                        nc.scalar.activation(gg[:], gpg_t[:], AF.Tanh)
                    m1 = rp.tile([128, 16], F32)
                    nc.vector.tensor_mul(m1[:], sig[:, 16:32], c_prev[:])
                    m2 = rp.tile([128, 16], F32)
                    nc.vector.tensor_mul(m2[:], sig[:, 0:16], gg[:])
                    c_new = cpp.tile([128, 16], F32)
                    nc.vector.tensor_tensor(c_new[:], m1[:], m2[:], op=ALU.add)
                    thc = rp.tile([128, 16], F32)
                    nc.scalar.activation(thc[:], c_new[:], AF.Tanh)
                    nc.vector.tensor_mul(hT[:, (t + 1) * 16:(t + 2) * 16], sig[:, 32:48], thc[:])
                    c_prev = c_new
                    if xi < len(xs_work):
                        xs_work[xi]()
                        xi += 1
                    if yi < len(y_work) and y_work[yi][0] <= t and (t >= 96 or t % 2 == 0):
                        y_work[yi][1]()
                        yi += 1
                while xi < len(xs_work):
                    xs_work[xi]()
                    xi += 1
                while yi < len(y_work):
                    y_work[yi][1]()
                    yi += 1

                # ---- softmax over T (free axis) ----
                mx = smp.tile([8, 1], F32)
                nc.vector.tensor_reduce(mx[:], sc_ps[:, 0:T_steps], axis=mybir.AxisListType.X, op=ALU.max)
                nmx = smp.tile([8, 1], F32)
                nc.vector.tensor_scalar_mul(nmx[:], mx[:], -1.0)
                esc = smp.tile([8, T_steps], F32)
                ssum = smp.tile([8, 1], F32)
                nc.scalar.activation(esc[:], sc_ps[:, 0:T_steps], AF.Exp,
                                     bias=nmx[:, 0:1], accum_out=ssum[:, 0:1])
                rcs = smp.tile([8, 1], F32)
                nc.vector.reciprocal(rcs[:], ssum[:])
                nc.scalar.activation(a_sb[:], esc[:], AF.Copy, scale=rcs[:, 0:1])
                # ---- r = sum_t a_t * h_t ----
                for b in range(8):
                    abc = ypsm.tile([128, T_steps], F32, space="PSUM", tag="yps")
                    nc.tensor.matmul(abc[:, 0:T_steps], sel_sb[:, b * 128:(b + 1) * 128],
                                     a_sb[:], start=True, stop=True)
                    for dj in range(2):
                        wt_ = ap_.tile([128, T_steps], F32)
                        nc.vector.tensor_tensor(wt_[:, 0:T_steps], hT_v[:, dj, b, 1:T_steps + 1],
                                                abc[:, 0:T_steps], op=ALU.mult)
                        nc.vector.tensor_reduce(rT_sb[:, dj * 8 + b: dj * 8 + b + 1],
                                                wt_[:, 0:T_steps], axis=mybir.AxisListType.X,
                                                op=ALU.add)
                nc.vector.tensor_copy(rT_bf[:], rT_sb[:])

            # ================= final MLP + softmax ==========================
            with (
                tc.tile_pool(name="fin", bufs=1) as fp,
                tc.tile_pool(name="fps", bufs=1, space="PSUM") as fpsm,
            ):
                rr_ps = fpsm.tile([128, 16], F32, space="PSUM")
                hlast = hT[:, T_steps * 16:(T_steps + 1) * 16]
                for oc in range(2):
                    for kc in range(2):
                        nc.tensor.matmul(rr_ps[:, oc * 8:(oc + 1) * 8],
                                         wpT_sb[:, kc * 256 + oc * 128: kc * 256 + (oc + 1) * 128],
                                         rT_bf[:, kc * 8:(kc + 1) * 8],
                                         start=(kc == 0), stop=False)
                    for kc in range(2):
                        nc.tensor.matmul(rr_ps[:, oc * 8:(oc + 1) * 8],
                                         wxT_sb[:, kc * 256 + oc * 128: kc * 256 + (oc + 1) * 128],
                                         hlast[:, kc * 8:(kc + 1) * 8],
                                         start=False, stop=(kc == 1))
                rrT = fp.tile([128, 16], BF16)
                nc.scalar.activation(rrT[:], rr_ps[:], AF.Tanh)
                z_ps = fpsm.tile([128, 16], F32, space="PSUM")
                for oc in range(2):
                    for kc in range(2):
                        nc.tensor.matmul(z_ps[:, oc * 8:(oc + 1) * 8],
                                         wfT_sb[:, kc * 256 + oc * 128: kc * 256 + (oc + 1) * 128],
                                         rrT[:, kc * 8:(kc + 1) * 8],
                                         start=(kc == 0), stop=(kc == 1))
                e_sb = fp.tile([128, 16], F32)
                for oc in range(2):
                    nc.scalar.activation(e_sb[:, oc * 8:(oc + 1) * 8], z_ps[:, oc * 8:(oc + 1) * 8],
                                         AF.Exp, bias=bf_sb[:, oc:oc + 1])
                cs_ps = fpsm.tile([1, 16], F32, space="PSUM")
                nc.tensor.matmul(cs_ps[:], ones128[:], e_sb[:], start=True, stop=True)
                cs_sb = fp.tile([1, 16], F32)
                nc.vector.tensor_copy(cs_sb[:], cs_ps[:])
                s8 = fp.tile([1, 8], F32)
                nc.vector.tensor_tensor(s8[:], cs_sb[0:1, 0:8], cs_sb[0:1, 8:16], op=ALU.add)
                rc8 = fp.tile([1, 8], F32)
                nc.vector.reciprocal(rc8[:], s8[:])
                rc16 = fp.tile([1, 16], F32)
                nc.vector.tensor_copy(rc16[:, 0:8], rc8[:])
                nc.vector.tensor_copy(rc16[:, 8:16], rc8[:])
                rbc_ps = fpsm.tile([128, 16], F32, space="PSUM")
                nc.tensor.matmul(rbc_ps[:], ones1w[:], rc16[:], start=True, stop=True)
                yT_sb = fp.tile([128, 16], F32)
                nc.vector.tensor_tensor(yT_sb[:], e_sb[:], rbc_ps[:], op=ALU.mult)
                ytr_ps = fpsm.tile([16, 128], F32, space="PSUM")
                nc.tensor.transpose(ytr_ps[:], yT_sb[:], ident[:])
                ynat = fp.tile([16, 128], F32)
                nc.vector.tensor_copy(ynat[:], ytr_ps[:])
                for oc in range(2):
                    nc.sync.dma_start(out_e[0:8, oc * 128:(oc + 1) * 128],
                                      ynat[oc * 8:(oc + 1) * 8, :])

    nc.compile()
    return nc


_CACHE = {}


def _get_nc(T_steps=512, variant=None):
    key = (T_steps, VARIANT if variant is None else variant)
    if key not in _CACHE:
        _CACHE[key] = build(T_steps, variant=key[1])
    return _CACHE[key]


def make_in_maps(x, s, emb, w_ih, w_hh, b_lstm, W_y, w_t, W_p, W_x, W_f, b_f,
                 T_steps=512, variant=None):
    variant = VARIANT if variant is None else variant
    x = np.asarray(x).astype(np.int32)[:, :T_steps]
    s = np.asarray(s).astype(np.int32).reshape(64)
    emb = np.ascontiguousarray(np.asarray(emb, dtype=np.float32))
    wih_p = np.asarray(w_ih, dtype=np.float32)[_PERM]
    whh_p = np.asarray(w_hh, dtype=np.float32)[_PERM]
    bl_p = np.asarray(b_lstm, dtype=np.float32)[_PERM]

    def wt2sb(wT):  # [256, 1024] -> [128, 2048]
        return np.concatenate([wT[0:128], wT[128:256]], axis=1)

    wihT = wt2sb(wih_p.T).astype(bf16)
    whhT = wt2sb(whh_p.T).astype(bf16)
    bl_sb = bl_p.reshape(8, 128).T.copy().astype(np.float32)  # [128, 8]
    wy_sb = np.concatenate([np.asarray(W_y, np.float32)[0:128],
                            np.asarray(W_y, np.float32)[128:256]], axis=1).astype(bf16)
    w_t = np.asarray(w_t, np.float32)
    wtoh = np.zeros((128, 128), np.float32)
    for ec in range(2):
        for b in range(8):
            wtoh[:, ec * 64 + b * 8 + b] = w_t[ec * 128:(ec + 1) * 128]
    wtoh = wtoh.astype(bf16)
    sel = np.zeros((8, 1024), np.float32)
    for b in range(8):
        sel[b, b * 128:(b + 1) * 128] = 1.0
    sel = sel.astype(bf16)

    def t2sb(w):  # W [do, din] -> lhsT layout [128, 512] free=kc*256+do
        wT = np.asarray(w, np.float32).T  # [din, do]
        return np.concatenate([wT[0:128], wT[128:256]], axis=1).astype(bf16)

    wpT = t2sb(W_p)
    wxT = t2sb(W_x)
    wfT = t2sb(W_f)
    bf_sb = np.asarray(b_f, np.float32).reshape(2, 128).T.copy()

    if variant in ("i", "j"):
        # host-side gather + transpose + batchnorm
        semb = emb[s]                                      # [64, 256]
        mu = semb.mean(0, keepdims=True)
        var = semb.var(0, keepdims=True)
        sn = (semb - mu) / np.sqrt(var + 1e-5)             # [64, 256]
        common = dict(wihT=wihT, whhT=whhT, bl=bl_sb, wy=wy_sb, wtoh=wtoh,
                      sel=sel, wpT=wpT, wxT=wxT, wfT=wfT, bf=bf_sb)
        if variant == "j":
            common['whh8'] = wt2sb(whh_p.T).astype(ml_dtypes.float8_e4m3)
        in_maps = []
        for c in range(NCORES):
            xs_c = x[c * BL:(c + 1) * BL]                  # [8, T]
            ex = emb[xs_c]                                 # [8, T, 256] f32
            exT = ex.transpose(2, 1, 0)                    # [256, T, 8]
            et0 = np.ascontiguousarray(exT[0:128]).reshape(128, T_steps * 8).astype(bf16)
            et1 = np.ascontiguousarray(exT[128:256]).reshape(128, T_steps * 8).astype(bf16)
            rows = sn[c * BL:(c + 1) * BL]                 # [8, 256]
            s2 = np.concatenate([rows, rows], axis=1).astype(bf16)  # [8, 512]
            in_maps.append(dict(et0=et0, et1=et1, s2=s2, **common))
        return in_maps

    whh8 = wt2sb(whh_p.T).astype(ml_dtypes.float8_e4m3)
    common = dict(emb=emb, wihT=wihT, whhT=whhT, whh8=whh8, bl=bl_sb, wy=wy_sb, wtoh=wtoh,
                  sel=sel, wpT=wpT, wxT=wxT, wfT=wfT, bf=bf_sb)
    in_maps = []
    for c in range(NCORES):
        xs_c = x[c * BL:(c + 1) * BL]                      # [8, T]
        xflat = xs_c.T.reshape(-1)                         # t-major tokens
        xp = xflat.reshape(-1, 128).T.copy()               # [128, NT]
        sidx = np.roll(s, -BL * c).reshape(64, 1).copy()
        in_maps.append(dict(xp=xp, sidx=sidx, **common))
    return in_maps


def _install_trace_shim():
    """The agent image lacks antenv.axon_hooks; recreate it and install the
    ctypes NTFF hook from trn_boot so run_bass_kernel_spmd(trace=True) works."""
    import sys, types
    if "antenv.axon_hooks" not in sys.modules:
        mod = types.ModuleType("antenv.axon_hooks")
        mod._hook = None
        mod.set_axon_ntff_profile_hook = lambda h: setattr(mod, "_hook", h)
        mod.get_axon_ntff_profile_hook = lambda: mod._hook
        sys.modules["antenv.axon_hooks"] = mod
        import antenv
        antenv.axon_hooks = mod
    import antenv.axon_hooks as ah
    if ah.get_axon_ntff_profile_hook() is None:
        from trn_agent_boot.trn_boot import _ntff_profile_via_ctypes
        ah.set_axon_ntff_profile_hook(_ntff_profile_via_ctypes("/opt/axon/libaxon_pjrt.so"))
    import concourse.bass_utils as bu
    bu.upload_artifacts = lambda tmpdir: ""


def run(in_maps, T_steps=512, trace=False, tmpdir=None, variant=None):
    nc = _get_nc(T_steps, variant)
    if trace:
        _install_trace_shim()
    return run_bass_kernel_spmd(nc, in_maps, core_ids=list(range(NCORES)),
                                trace=trace, tmpdir=tmpdir)


def kernel(x, s, emb, w_ih, w_hh, b_lstm, W_y, w_t, W_p, W_x, W_f, b_f):
    in_maps = make_in_maps(x, s, emb, w_ih, w_hh, b_lstm, W_y, w_t, W_p, W_x,
                           W_f, b_f)
    res = run(in_maps)
    return np.concatenate([res.results[i]["out"] for i in range(NCORES)], axis=0)
